# revision 24
# baseline (speedup 1.0000x reference)
"""DeepSeek-V2 MoE gate (group-limited greedy top-k routing) on 8 trn2 NeuronCores.

Reference computation (per token t over E=160 experts in G=8 groups of 20):
    logits = x @ W^T                       [T, E]
    scores = softmax(logits)
    group_scores[g] = max over group g of scores
    keep top-3 groups; mask scores of other groups to 0
    topk_weight, topk_idx = top_k(masked scores, 6); topk_weight *= 16.0

Sharding: tokens (B*S = 16384) split evenly across the 8 cores; the small
[160, 5120] gate weight is replicated (pre-arranged host-side).

The kernel is DMA-bound: each core must read its 41.9MB x shard once, and
the SBUF fabric ceiling (~435 GB/s) puts the floor near 100us. Everything
else is arranged to stay under that roofline:

- The tensor engine contracts over the partition axis, so both matmul
  operands need hidden (H=5120) on partitions. Host-side prep lays the
  shard out as xp[p, tile, j, t] = x[tile*128 + t, p*40 + j], making every
  token-tile load one fully contiguous 20KB-per-partition run (line rate)
  and every per-k-tile stationary slice contiguous in SBUF.
- Precision comes from a 3-term bf16 split (x = hi + lo, W = Whi + Wlo;
  logits = hi@Whi + hi@Wlo + lo@Whi accumulated in fp32 PSUM, error
  ~2^-18). Single-pass float32r would be ~10% faster on paper but its
  ~11-bit operand truncation flips too many near-tie expert picks
  (rel err 1.99e-2 vs the 2e-2 gate).
- The winning mode "hilo3g" fuses Whi|Wlo into one N=320 moving operand
  (2 matmuls per k-tile instead of 3), interleaves the hi and lo term
  streams into a single PSUM accumulation group per bank (psh[0:160] =
  hi@Whi + lo@Whi, psh[160:320] = hi@Wlo, folded with one copy+add),
  keeps the weight tile resident, fuses each token-tile pair's x load
  into a single 5.24MB DMA, rotates tile buffers seamlessly across
  repeat boundaries, and sends outputs on the scalar HWDGE ring so the
  sync ring stays dedicated to x prefetches. Measured ~90us/core vs the
  278.6us baseline; 6/98304 near-tie index swaps, rel err 4.8e-3.

Selection runs on raw logits (softmax is monotonic; the top-3-group test by
max-score equals the test by max-logit), so only the final 6 weights and the
softmax denominator need exp().
"""

import numpy as np

import concourse.bacc as bacc
import concourse.mybir as mybir
from concourse import bass_utils
from concourse.tile import TileContext

# Problem constants (hardcoded per the harness contract).
B, S, H = 4, 4096, 5120
E = 160                 # experts
G = 8                   # groups
EG = E // G             # experts per group (20)
TOP_K = 6
TOPK_GROUP = 3
ROUTED_SCALING = 16.0
N_CORES = 8
T_TOTAL = B * S         # 16384
T_CORE = T_TOTAL // N_CORES  # 2048
P = 128                 # SBUF partitions
J = H // P              # hidden values per partition (40) = number of k-tiles
NEG_BIG = -1.0e30

F32 = mybir.dt.float32
F32R = mybir.dt.float32r  # fp32 the PE streams at bf16 rate (moving dim
                          # >=256) but with ~11-bit operand truncation
BF16 = mybir.dt.bfloat16
F16 = mybir.dt.float16
F8E3 = mybir.dt.float8e3  # e3m4: 5 mantissa bits, range [2^-6, 15.5]
U32 = mybir.dt.uint32
ALU = mybir.AluOpType
ACTF = mybir.ActivationFunctionType
AX = mybir.AxisListType


def emit_gate(tc, x_ap, w_ap, oi_ap, ow_ap):
    """Emit the gate kernel body into TileContext `tc`.

    x_ap:  [T, H] f32 DRAM (T % 128 == 0)
    w_ap:  [P, J*E] f32 DRAM (pre-permuted weight, see module docstring)
    oi_ap: [T, TOP_K] u32 DRAM out (expert indices)
    ow_ap: [T, TOP_K] f32 DRAM out (routing weights)
    """
    nc = tc.nc
    T = x_ap.shape[0]
    assert T % P == 0
    n_tiles = T // P

    with (
        tc.tile_pool(name="wpool", bufs=1) as wpool,
        tc.tile_pool(name="xpool", bufs=3) as xpool,
        tc.tile_pool(name="psum", bufs=4, space="PSUM") as psum_pool,
        tc.tile_pool(name="small", bufs=6) as small,
        tc.tile_pool(name="bigt", bufs=3) as bigt,
    ):
        w_sb = wpool.tile([P, J * E], F32)
        nc.sync.dma_start(w_sb[:], w_ap)

        for tt in range(n_tiles):
            # x tile: [p, t*J + j] = x[t0 + t, p*J + j]
            xt = xpool.tile([P, P * J], F32)
            src = x_ap[tt * P : (tt + 1) * P, :].rearrange("t (p j) -> p t j", p=P)
            nc.sync.dma_start(xt[:].rearrange("p (t j) -> p t j", j=J), src)
            xt3 = xt[:].rearrange("p (t j) -> p t j", j=J)

            # logits[t, e] accumulated over the 40 k-tiles
            ps = psum_pool.tile([P, E], F32)
            for j in range(J):
                nc.tensor.matmul(
                    ps[:],
                    xt3[:, :, j],                  # stationary [128h, 128t]
                    w_sb[:, j * E : (j + 1) * E],  # moving     [128h, 160e]
                    start=(j == 0),
                    stop=(j == J - 1),
                )

            ps3 = ps[:].rearrange("p (g i) -> p g i", i=EG)

            # group max of logits -> top-3-group additive penalty mask
            gmax = small.tile([P, G], F32)
            nc.vector.tensor_reduce(gmax[:], ps3, axis=AX.X, op=ALU.max)
            gsort = small.tile([P, 8], F32)
            nc.vector.max(gsort[:], gmax[:])
            gpen = small.tile([P, G], F32)  # 0 for kept groups, NEG_BIG for dropped
            nc.vector.tensor_scalar(
                gpen[:], gmax[:], gsort[:, TOPK_GROUP - 1 : TOPK_GROUP], NEG_BIG,
                op0=ALU.is_lt, op1=ALU.mult,
            )

            # masked logits = logits + penalty(group)
            masked = bigt.tile([P, E], F32)
            nc.vector.scalar_tensor_tensor(
                masked[:].rearrange("p (g i) -> p g i", i=EG),
                ps3,
                1.0,
                gpen[:, :, None].to_broadcast((P, G, EG)),
                op0=ALU.mult,
                op1=ALU.add,
            )

            # top-8 masked logits (descending) + expert indices
            v8 = small.tile([P, 8], F32)
            nc.vector.max(v8[:], masked[:])
            i8 = small.tile([P, 8], U32)
            nc.vector.max_index(i8[:], v8[:], masked[:])

            # softmax pieces: global max logit is v8[:,0] (the best group holds it)
            nrmax = small.tile([P, 1], F32)
            nc.vector.tensor_scalar_mul(nrmax[:], v8[:, 0:1], -1.0)
            exps = bigt.tile([P, E], F32)
            ssum = small.tile([P, 1], F32)
            nc.scalar.activation(
                exps[:], ps[:], ACTF.Exp, bias=nrmax[:], scale=1.0, accum_out=ssum[:]
            )
            rcp = small.tile([P, 1], F32)
            nc.vector.reciprocal(rcp[:], ssum[:])
            scl = small.tile([P, 1], F32)
            nc.vector.tensor_scalar_mul(scl[:], rcp[:], ROUTED_SCALING)

            # weights = exp(v6 - rmax) * 16 / ssum
            e6 = small.tile([P, TOP_K], F32)
            nc.scalar.activation(e6[:], v8[:, 0:TOP_K], ACTF.Exp, bias=nrmax[:], scale=1.0)
            w6 = small.tile([P, TOP_K], F32)
            nc.vector.tensor_scalar_mul(w6[:], e6[:], scl[:])

            nc.sync.dma_start(oi_ap[tt * P : (tt + 1) * P, :], i8[:, 0:TOP_K])
            nc.sync.dma_start(ow_ap[tt * P : (tt + 1) * P, :], w6[:])


E_PAD = 256  # experts padded so the f32r moving operand is >=256 wide

# Fast-DMA activation layout, shared by the f32r and hilo3f modes:
# xp[p, ((tile*J) + j)*P + t] = x[tile*P + t, p*J + j]. Each token-tile's
# DMA is one fully contiguous 20KB run per partition (line rate), and the
# per-k-tile stationary slice xt[:, j*P:(j+1)*P] is contiguous in SBUF
# (for bf16 this lets the compiler's Fast Weight Load engage; a strided
# stationary AP defeats it and the kernel goes LDWEIGHTS-bound).


def emit_gate_f32r(tc, x_ap, w_ap, oi_ap, ow_ap):
    """Single-pass float32r gate.

    float32r is fp32 data the PE streams at bf16 rate (1 cycle/row) when the
    moving free dim is >=256 — below that it falls to 1/4 rate. The weight is
    therefore zero-padded from 160 to 256 experts; the epilogue only ever
    reads logits[:, :160] so the pad never enters selection.

    MEASURED: 116.9us (= the ~117us HBM roofline for the 41.9MB/core x
    read), but the f32r datapath truncates operands to ~11 mantissa bits:
    rel err 1.99e-2 vs the 2e-2 gate (hundreds of flipped near-tie 6th
    picks). Too risky to ship; kept for reference.
    """
    nc = tc.nc
    T = x_ap.shape[1] // (P * J) * P
    n_tiles = T // P

    with (
        tc.tile_pool(name="wpool", bufs=1) as wpool,
        tc.tile_pool(name="xpool", bufs=3) as xpool,
        tc.tile_pool(name="psum", bufs=4, space="PSUM") as psum_pool,
        tc.tile_pool(name="small", bufs=6) as small,
        tc.tile_pool(name="bigt", bufs=3) as bigt,
    ):
        w_sb = wpool.tile([P, J * E_PAD], F32R)
        nc.sync.dma_start(w_sb[:], w_ap)

        for tt0 in range(0, n_tiles, 2):
            pair = [tt0, tt0 + 1] if tt0 + 1 < n_tiles else [tt0]
            xts, pss = [], []
            for tt in pair:
                xt = xpool.tile([P, P * J], F32R)
                nc.sync.dma_start(
                    xt[:], x_ap[:, tt * P * J : (tt + 1) * P * J]
                )
                xts.append(xt[:])
                pss.append(psum_pool.tile([P, E_PAD], F32, name="ps", tag=f"ps{len(pss)}"))

            for j in range(J):
                for k in range(len(pair)):
                    nc.tensor.matmul(
                        pss[k][:],
                        xts[k][:, j * P : (j + 1) * P],
                        w_sb[:, j * E_PAD : (j + 1) * E_PAD],
                        start=(j == 0),
                        stop=(j == J - 1),
                    )

            for k, tt in enumerate(pair):
                _emit_epilogue(tc, small, bigt, pss[k][:, 0:E], oi_ap, ow_ap, tt)


def emit_gate_hilo3f(tc, x_ap, whi_ap, wlo_ap, oi_ap, ow_ap):
    """3-term bf16 split gate on the fast-DMA [p, tile, j, t] layout.

    logits = hi@Whi + hi@Wlo + lo@Whi, fp32 PSUM accumulation, error
    ~2^-18 (the dropped lo@Wlo term). The contiguous per-j stationary
    slice keeps LDWEIGHTS on the Fast-Weight-Load path (~53ns < the 67ns
    N=160 stream), so the PE runs at the 3x160x40 streaming floor
    (~128us/core) instead of the LDW-bound ~205us the strided layout
    gives. DMA is at line rate (~117us/core), fully overlapped.
    """
    nc = tc.nc
    T = x_ap.shape[1] // J
    n_tiles = T // P

    with (
        tc.tile_pool(name="wpool", bufs=1) as wpool,
        tc.tile_pool(name="xpool", bufs=3) as xpool,
        tc.tile_pool(name="hpool", bufs=3) as hpool,
        tc.tile_pool(name="lpool", bufs=3) as lpool,
        tc.tile_pool(name="psum", bufs=4, space="PSUM") as psum_pool,
        tc.tile_pool(name="small", bufs=6) as small,
        tc.tile_pool(name="bigt", bufs=3) as bigt,
    ):
        whi_sb = wpool.tile([P, J * E], BF16)
        nc.sync.dma_start(whi_sb[:], whi_ap)
        wlo_sb = wpool.tile([P, J * E], BF16)
        nc.sync.dma_start(wlo_sb[:], wlo_ap)

        for tt0 in range(0, n_tiles, 2):
            pair = [tt0, tt0 + 1] if tt0 + 1 < n_tiles else [tt0]
            his, los, pss = [], [], []
            for tt in pair:
                xt = xpool.tile([P, P * J], F32)
                nc.sync.dma_start(
                    xt[:], x_ap[:, tt * P * J : (tt + 1) * P * J]
                )
                hi = hpool.tile([P, P * J], BF16)
                nc.scalar.copy(hi[:], xt[:])
                lo = lpool.tile([P, P * J], BF16)
                nc.vector.scalar_tensor_tensor(
                    lo[:], xt[:], 1.0, hi[:], op0=ALU.mult, op1=ALU.subtract
                )
                his.append(hi[:])
                los.append(lo[:])
                pss.append(
                    psum_pool.tile([P, E], F32, name="ps", tag=f"ps{len(pss)}")
                )

            for j in range(J):
                xsl = slice(j * P, (j + 1) * P)
                wsl = slice(j * E, (j + 1) * E)
                ops = [(his, whi_sb), (his, wlo_sb), (los, whi_sb)]
                for oi, (xs, wsb) in enumerate(ops):
                    last = j == J - 1 and oi == len(ops) - 1
                    for k in range(len(pair)):
                        nc.tensor.matmul(
                            pss[k][:], xs[k][:, xsl], wsb[:, wsl],
                            start=(j == 0 and oi == 0), stop=last,
                        )

            for k, tt in enumerate(pair):
                _emit_epilogue(tc, small, bigt, pss[k][:], oi_ap, ow_ap, tt)


def emit_gate_hilo3w(tc, x_ap, wc_ap, oi_ap, ow_ap):
    """Like hilo3f but with Whi|Wlo concatenated per j into one N=320
    moving operand: per k-tile, 2 matmuls (hi@[Whi|Wlo], lo@Whi) instead
    of 3, cutting LDWEIGHTS/instruction count by a third at identical
    streamed-row count. logits = ps_h[:,0:160] + ps_h[:,160:320] + ps_l,
    folded with two DVE adds. wc_ap: [P, J*2E] bf16,
    wc[p, j*2E + e] = Whi[e] for e<160 else Wlo[e-160]."""
    nc = tc.nc
    T = x_ap.shape[1] // J
    n_tiles = T // P
    E2 = 2 * E

    with (
        tc.tile_pool(name="wpool", bufs=1) as wpool,
        tc.tile_pool(name="xpool", bufs=3) as xpool,
        tc.tile_pool(name="hpool", bufs=3) as hpool,
        tc.tile_pool(name="lpool", bufs=3) as lpool,
        tc.tile_pool(name="psum", bufs=2, space="PSUM") as psum_pool,
        tc.tile_pool(name="small", bufs=6) as small,
        tc.tile_pool(name="bigt", bufs=4) as bigt,
    ):
        wc_sb = wpool.tile([P, J * E2], BF16)
        nc.sync.dma_start(wc_sb[:], wc_ap)

        for tt0 in range(0, n_tiles, 2):
            pair = [tt0, tt0 + 1] if tt0 + 1 < n_tiles else [tt0]
            his, los, psh, psl = [], [], [], []
            for tt in pair:
                xt = xpool.tile([P, P * J], F32)
                nc.sync.dma_start(
                    xt[:], x_ap[:, tt * P * J : (tt + 1) * P * J]
                )
                hi = hpool.tile([P, P * J], BF16)
                nc.scalar.copy(hi[:], xt[:])
                lo = lpool.tile([P, P * J], BF16)
                nc.vector.scalar_tensor_tensor(
                    lo[:], xt[:], 1.0, hi[:], op0=ALU.mult, op1=ALU.subtract
                )
                his.append(hi[:])
                los.append(lo[:])
                # full-bank tiles so the two accumulation groups can never
                # share a PSUM bank (a group's start clears its whole bank)
                psh.append(
                    psum_pool.tile([P, 512], F32, name="psh", tag=f"psh{len(psh)}")
                )
                psl.append(
                    psum_pool.tile([P, 512], F32, name="psl", tag=f"psl{len(psl)}")
                )

            for j in range(J):
                xsl = slice(j * P, (j + 1) * P)
                for k in range(len(pair)):
                    nc.tensor.matmul(
                        psh[k][:, 0:E2], his[k][:, xsl],
                        wc_sb[:, j * E2 : (j + 1) * E2],
                        start=(j == 0), stop=(j == J - 1),
                    )
                    nc.tensor.matmul(
                        psl[k][:, 0:E], los[k][:, xsl],
                        wc_sb[:, j * E2 : j * E2 + E],
                        start=(j == 0), stop=(j == J - 1),
                    )

            for k, tt in enumerate(pair):
                # DVE/ACT may read at most one PSUM input per instruction
                hb = bigt.tile([P, E], F32)
                nc.scalar.copy(hb[:], psh[k][:, E:E2])
                ha = bigt.tile([P, E], F32)
                nc.vector.tensor_add(ha[:], psh[k][:, 0:E], hb[:])
                lg = bigt.tile([P, E], F32)
                nc.vector.tensor_add(lg[:], ha[:], psl[k][:, 0:E])
                _emit_epilogue(tc, small, bigt, lg[:], oi_ap, ow_ap, tt)


def emit_gate_hilo3g(tc, x_ap, wc_sb, oi_ap, ow_ap, repeat=1):
    """hilo3w with a resident weight tile (loaded once per NEFF, shared
    across repeats) and one fused 5.24MB DMA per token-tile pair.

    The repeat loop runs INSIDE the open tile pools so buffer rotation
    flows seamlessly across repeat boundaries (repeat r+1's first DMA
    prefetches during repeat r's tail) — the repeat-slope then measures
    pure steady-state pipeline rate, which is also the real back-to-back
    invocation throughput.

    wc_sb: [P, J*2E] bf16 SBUF AP, already loaded.
    """
    nc = tc.nc
    T = x_ap.shape[1] // J
    n_tiles = T // P
    E2 = 2 * E

    with (
        tc.tile_pool(name="xpool", bufs=2) as xpool,
        tc.tile_pool(name="hpool", bufs=2) as hpool,
        tc.tile_pool(name="lpool", bufs=2) as lpool,
        tc.tile_pool(name="psum", bufs=3, space="PSUM") as psum_pool,
        tc.tile_pool(name="small", bufs=6) as small,
        tc.tile_pool(name="bigt", bufs=4) as bigt,
    ):
        for _rep in range(repeat):
            for tt0 in range(0, n_tiles, 2):
                npair = 2 if tt0 + 1 < n_tiles else 1
                xt = xpool.tile([P, npair * P * J], F32)
                nc.sync.dma_start(
                    xt[:], x_ap[:, tt0 * P * J : (tt0 + npair) * P * J]
                )
                hi = hpool.tile([P, npair * P * J], BF16)
                nc.scalar.copy(hi[:], xt[:])
                lo = lpool.tile([P, npair * P * J], BF16)
                nc.vector.scalar_tensor_tensor(
                    lo[:], xt[:], 1.0, hi[:], op0=ALU.mult, op1=ALU.subtract
                )
                psh = [
                    psum_pool.tile([P, 512], F32, name="psh", tag=f"psh{k}")
                    for k in range(npair)
                ]

                # hi@[Whi|Wlo] (N=320) and lo@Whi (N=160) interleave in ONE
                # accumulation group per bank: psh[0:160] accumulates
                # hi@Whi + lo@Whi, psh[160:320] accumulates hi@Wlo. start
                # clears the whole bank on the first matmul only.
                for j in range(J):
                    for k in range(npair):
                        xsl = slice((k * J + j) * P, (k * J + j + 1) * P)
                        nc.tensor.matmul(
                            psh[k][:, 0:E2], hi[:, xsl],
                            wc_sb[:, j * E2 : (j + 1) * E2],
                            start=(j == 0), stop=False,
                        )
                        nc.tensor.matmul(
                            psh[k][:, 0:E], lo[:, xsl],
                            wc_sb[:, j * E2 : j * E2 + E],
                            start=False, stop=(j == J - 1),
                        )

                for k in range(npair):
                    tt = tt0 + k
                    hb = bigt.tile([P, E], F32)
                    nc.scalar.copy(hb[:], psh[k][:, E:E2])
                    lg = bigt.tile([P, E], F32)
                    nc.vector.tensor_add(lg[:], psh[k][:, 0:E], hb[:])
                    _emit_epilogue(tc, small, bigt, lg[:], oi_ap, ow_ap, tt)


def emit_gate_hilo3h(tc, x_ap, wc_sb, oi_ap, ow_ap):
    """hilo3g with the bf16 hi/lo split done host-side: x_ap is
    [P, n_pairs * 4*P*J] bf16 laid out per token-tile pair as
    [hi(tile0) hi(tile1) lo(tile0) lo(tile1)], so each pair is one
    5.24MB contiguous DMA and the ACT cast / DVE subtract disappear
    from the device entirely (same total DMA bytes as f32 x).
    """
    nc = tc.nc
    TJ4 = 4 * P * J
    n_pairs = x_ap.shape[1] // TJ4
    E2 = 2 * E

    with (
        tc.tile_pool(name="xpool", bufs=3) as xpool,
        tc.tile_pool(name="psum", bufs=2, space="PSUM") as psum_pool,
        tc.tile_pool(name="small", bufs=6) as small,
        tc.tile_pool(name="bigt", bufs=4) as bigt,
    ):
        for q in range(n_pairs):
            xc = xpool.tile([P, TJ4], BF16)
            nc.sync.dma_start(xc[:], x_ap[:, q * TJ4 : (q + 1) * TJ4])
            psh = [
                psum_pool.tile([P, 512], F32, name="psh", tag=f"psh{k}")
                for k in range(2)
            ]
            psl = [
                psum_pool.tile([P, 512], F32, name="psl", tag=f"psl{k}")
                for k in range(2)
            ]

            for j in range(J):
                for k in range(2):
                    hsl = slice((k * J + j) * P, (k * J + j + 1) * P)
                    lsl = slice(
                        (2 * J + k * J + j) * P, (2 * J + k * J + j + 1) * P
                    )
                    nc.tensor.matmul(
                        psh[k][:, 0:E2], xc[:, hsl],
                        wc_sb[:, j * E2 : (j + 1) * E2],
                        start=(j == 0), stop=(j == J - 1),
                    )
                    nc.tensor.matmul(
                        psl[k][:, 0:E], xc[:, lsl],
                        wc_sb[:, j * E2 : j * E2 + E],
                        start=(j == 0), stop=(j == J - 1),
                    )

            for k in range(2):
                tt = 2 * q + k
                hb = bigt.tile([P, E], F32)
                nc.scalar.copy(hb[:], psh[k][:, E:E2])
                ha = bigt.tile([P, E], F32)
                nc.vector.tensor_add(ha[:], psh[k][:, 0:E], hb[:])
                lg = bigt.tile([P, E], F32)
                nc.vector.tensor_add(lg[:], ha[:], psl[k][:, 0:E])
                _emit_epilogue(tc, small, bigt, lg[:], oi_ap, ow_ap, tt)


def emit_gate_f16(tc, x_ap, wc_sb, oi_ap, ow_ap, repeat=1, chunk=4):
    """fp16 host-split gate: x arrives as fp16 (half the DMA bytes of f32),
    W as [Whi_f16 | Wlo_f16 * 2^11] fused into one N=320 moving operand
    (Wlo is pre-scaled into fp16 normal range host-side; the epilogue's
    ACT copy un-scales it). One matmul per (tile, k-tile): 320 moving
    cols vs the bf16 3-term's 480 — and no on-chip hi/lo split at all.

    logits = xh @ Whi + 2^-11 * (xh @ (Wlo*2^11)); error ~2^-11 from the
    fp16 rounding of x only (W split exact to ~2^-22).

    x_ap: [P, T*J] f16 in the fast-DMA layout
    xp[p, ((tile*J)+j)*P + t] = fp16(x[tile*P + t, p*J + j]).
    `chunk` token-tiles are fetched per DMA (chunk=4: 5.24MB transfers)
    and processed as `chunk` interleaved PSUM accumulation chains.
    """
    nc = tc.nc
    T = x_ap.shape[1] // J
    n_tiles = T // P
    E2 = 2 * E

    with (
        tc.tile_pool(name="xpool", bufs=2) as xpool,
        tc.tile_pool(name="psum", bufs=2, space="PSUM") as psum_pool,
        tc.tile_pool(name="small", bufs=6) as small,
        tc.tile_pool(name="bigt", bufs=4) as bigt,
    ):
        for _rep in range(repeat):
            for tt0 in range(0, n_tiles, chunk):
                nch = min(chunk, n_tiles - tt0)
                xt = xpool.tile([P, nch * P * J], F16)
                nc.sync.dma_start(
                    xt[:], x_ap[:, tt0 * P * J : (tt0 + nch) * P * J]
                )
                psh = [
                    psum_pool.tile([P, 512], F32, name="psh", tag=f"psh{k}")
                    for k in range(nch)
                ]
                for j in range(J):
                    for k in range(nch):
                        xsl = slice((k * J + j) * P, (k * J + j + 1) * P)
                        nc.tensor.matmul(
                            psh[k][:, 0:E2], xt[:, xsl],
                            wc_sb[:, j * E2 : (j + 1) * E2],
                            start=(j == 0), stop=(j == J - 1),
                        )
                for k in range(nch):
                    tt = tt0 + k
                    # un-scale the Wlo half (2^-11) while folding: ACT copy
                    # with scale, then one DVE add (<=1 PSUM operand each)
                    hb = bigt.tile([P, E], F32)
                    nc.scalar.activation(
                        hb[:], psh[k][:, E:E2], ACTF.Copy, scale=2.0 ** -11
                    )
                    lg = bigt.tile([P, E], F32)
                    nc.vector.tensor_add(lg[:], psh[k][:, 0:E], hb[:])
                    _emit_epilogue(tc, small, bigt, lg[:], oi_ap, ow_ap, tt)


LO_FOLD = 2.0 ** -18  # lo stored as e3m4(lo*2^12), W8 as e3m4(W*2^6)


def emit_gate_f16l(tc, x_ap, lo_ap, wc_sb, w8_sb, oi_ap, ow_ap, repeat=1, chunk=4):
    """f16h plus an fp8e3m4 lo-correction term: x = fp16(x) + lo, with
    lo shipped as e3m4(lo * 2^12) (1 byte) and W for the lo term as
    e3m4(W * 2^6). x error drops from 2^-11 (f16h, fails the gate) to
    ~2^-16. The lo matmuls share the hi accumulation group, landing in
    cols 320:480 of the same PSUM bank; the epilogue folds all three
    column ranges with the 2^-18 un-scale."""
    nc = tc.nc
    T = x_ap.shape[1] // J
    n_tiles = T // P
    E2 = 2 * E
    E3 = 3 * E

    with (
        tc.tile_pool(name="xpool", bufs=2) as xpool,
        tc.tile_pool(name="lpool", bufs=2) as lpool,
        tc.tile_pool(name="psum", bufs=2, space="PSUM") as psum_pool,
        tc.tile_pool(name="small", bufs=6) as small,
        tc.tile_pool(name="bigt", bufs=4) as bigt,
    ):
        for _rep in range(repeat):
            for tt0 in range(0, n_tiles, chunk):
                nch = min(chunk, n_tiles - tt0)
                xt = xpool.tile([P, nch * P * J], F16)
                nc.sync.dma_start(
                    xt[:], x_ap[:, tt0 * P * J : (tt0 + nch) * P * J]
                )
                lt = lpool.tile([P, nch * P * J], F8E3)
                nc.scalar.dma_start(
                    lt[:], lo_ap[:, tt0 * P * J : (tt0 + nch) * P * J]
                )
                psh = [
                    psum_pool.tile([P, 512], F32, name="psh", tag=f"psh{k}")
                    for k in range(nch)
                ]
                for j in range(J):
                    for k in range(nch):
                        xsl = slice((k * J + j) * P, (k * J + j + 1) * P)
                        nc.tensor.matmul(
                            psh[k][:, 0:E2], xt[:, xsl],
                            wc_sb[:, j * E2 : (j + 1) * E2],
                            start=(j == 0), stop=False,
                        )
                        nc.tensor.matmul(
                            psh[k][:, E2:E3], lt[:, xsl],
                            w8_sb[:, j * E : (j + 1) * E],
                            start=False, stop=(j == J - 1),
                        )
                for k in range(nch):
                    tt = tt0 + k
                    hb = bigt.tile([P, E], F32)
                    nc.scalar.activation(
                        hb[:], psh[k][:, E:E2], ACTF.Copy, scale=2.0 ** -11
                    )
                    t1 = bigt.tile([P, E], F32)
                    nc.vector.scalar_tensor_tensor(
                        t1[:], psh[k][:, E2:E3], LO_FOLD, hb[:],
                        op0=ALU.mult, op1=ALU.add,
                    )
                    lg = bigt.tile([P, E], F32)
                    nc.vector.tensor_add(lg[:], psh[k][:, 0:E], t1[:])
                    _emit_epilogue(tc, small, bigt, lg[:], oi_ap, ow_ap, tt)


def emit_gate_f16x(tc, x_ap, lo_ap, wh_sb, wl8_sb, w8_sb, oi_ap, ow_ap,
                   repeat=1, chunk=4):
    """Scheme X: like f16l but the Wlo correction stream rides fp8e3m4 on
    the SAME xh stationary (mixed-dtype matmul), so the fp16 moving stream
    shrinks to N=160:
        psh[0:160]   += xh  @ Whi_f16            (fp16 moving)
        psh[160:320] += xh  @ e3m4(Wlo*2^17)     (fp8 moving, same lhsT)
        psh[320:480] += lo8 @ e3m4(W*2^6)        (fp8 moving)
    Wins if fp8 moving columns stream faster than fp16 ones."""
    nc = tc.nc
    T = x_ap.shape[1] // J
    n_tiles = T // P
    E2 = 2 * E
    E3 = 3 * E

    with (
        tc.tile_pool(name="xpool", bufs=2) as xpool,
        tc.tile_pool(name="lpool", bufs=2) as lpool,
        tc.tile_pool(name="psum", bufs=2, space="PSUM") as psum_pool,
        tc.tile_pool(name="small", bufs=6) as small,
        tc.tile_pool(name="bigt", bufs=4) as bigt,
    ):
        for _rep in range(repeat):
            for tt0 in range(0, n_tiles, chunk):
                nch = min(chunk, n_tiles - tt0)
                xt = xpool.tile([P, nch * P * J], F16)
                nc.sync.dma_start(
                    xt[:], x_ap[:, tt0 * P * J : (tt0 + nch) * P * J]
                )
                lt = lpool.tile([P, nch * P * J], F8E3)
                nc.scalar.dma_start(
                    lt[:], lo_ap[:, tt0 * P * J : (tt0 + nch) * P * J]
                )
                psh = [
                    psum_pool.tile([P, 512], F32, name="psh", tag=f"psh{k}")
                    for k in range(nch)
                ]
                for j in range(J):
                    for k in range(nch):
                        xsl = slice((k * J + j) * P, (k * J + j + 1) * P)
                        wsl = slice(j * E, (j + 1) * E)
                        nc.tensor.matmul(
                            psh[k][:, 0:E], xt[:, xsl], wh_sb[:, wsl],
                            start=(j == 0), stop=False,
                        )
                        nc.tensor.matmul(
                            psh[k][:, E:E2], xt[:, xsl], wl8_sb[:, wsl],
                            start=False, stop=False,
                        )
                        nc.tensor.matmul(
                            psh[k][:, E2:E3], lt[:, xsl], w8_sb[:, wsl],
                            start=False, stop=(j == J - 1),
                        )
                for k in range(nch):
                    tt = tt0 + k
                    hb = bigt.tile([P, E], F32)
                    nc.scalar.activation(
                        hb[:], psh[k][:, E:E2], ACTF.Copy, scale=2.0 ** -17
                    )
                    t1 = bigt.tile([P, E], F32)
                    nc.vector.scalar_tensor_tensor(
                        t1[:], psh[k][:, E2:E3], LO_FOLD, hb[:],
                        op0=ALU.mult, op1=ALU.add,
                    )
                    lg = bigt.tile([P, E], F32)
                    nc.vector.tensor_add(lg[:], psh[k][:, 0:E], t1[:])
                    _emit_epilogue(tc, small, bigt, lg[:], oi_ap, ow_ap, tt)


def emit_probe_pe16(tc, x_ap, wc_sb, repeat=1):
    """Ablation probe: ONE fp16 quad loaded, then the full f16h matmul
    schedule re-reads the same tiles (no per-chunk DMA, no epilogue)."""
    nc = tc.nc
    T = x_ap.shape[1] // J
    n_tiles = T // P
    E2 = 2 * E
    with (
        tc.tile_pool(name="xpool", bufs=1) as xpool,
        tc.tile_pool(name="psum", bufs=2, space="PSUM") as psum_pool,
    ):
        xt = xpool.tile([P, 4 * P * J], F16)
        nc.sync.dma_start(xt[:], x_ap[:, 0 : 4 * P * J])
        for _rep in range(repeat):
            for _tt0 in range(0, n_tiles, 4):
                psh = [
                    psum_pool.tile([P, 512], F32, name="psh", tag=f"psh{k}")
                    for k in range(4)
                ]
                for j in range(J):
                    for k in range(4):
                        xsl = slice((k * J + j) * P, (k * J + j + 1) * P)
                        nc.tensor.matmul(
                            psh[k][:, 0:E2], xt[:, xsl],
                            wc_sb[:, j * E2 : (j + 1) * E2],
                            start=(j == 0), stop=(j == J - 1),
                        )


def emit_probe_pe_lo(tc, lo_ap, w8_sb, repeat=1):
    """Ablation probe: ONLY the fp8 N=160 lo matmuls of f16l."""
    nc = tc.nc
    T = lo_ap.shape[1] // J
    n_tiles = T // P
    E2 = 2 * E
    E3 = 3 * E
    with (
        tc.tile_pool(name="lxpool", bufs=1) as lxpool,
        tc.tile_pool(name="psum", bufs=2, space="PSUM") as psum_pool,
    ):
        lt = lxpool.tile([P, 4 * P * J], F8E3)
        nc.scalar.dma_start(lt[:], lo_ap[:, 0 : 4 * P * J])
        for _rep in range(repeat):
            for _tt0 in range(0, n_tiles, 4):
                psh = [
                    psum_pool.tile([P, 512], F32, name="psh", tag=f"psh{k}")
                    for k in range(4)
                ]
                for j in range(J):
                    for k in range(4):
                        xsl = slice((k * J + j) * P, (k * J + j + 1) * P)
                        nc.tensor.matmul(
                            psh[k][:, E2:E3], lt[:, xsl],
                            w8_sb[:, j * E : (j + 1) * E],
                            start=(j == 0), stop=(j == J - 1),
                        )


def emit_probe_dma(tc, x_ap, repeat=1, rings=1):
    """Ablation probe: ONLY the per-pair x DMAs of hilo3g (no consumers).
    The repeat-slope of this NEFF is the pure steady-state DMA rate.
    rings=2 splits each transfer across the sync and scalar HWDGE rings."""
    nc = tc.nc
    T = x_ap.shape[1] // J
    n_tiles = T // P
    with tc.tile_pool(name="xpool", bufs=2) as xpool:
        for _rep in range(repeat):
            for tt0 in range(0, n_tiles, 2):
                npair = 2 if tt0 + 1 < n_tiles else 1
                xt = xpool.tile([P, npair * P * J], F32)
                src = x_ap[:, tt0 * P * J : (tt0 + npair) * P * J]
                if rings == 2:
                    nc.sync.dma_start(xt[0:64, :], src[0:64, :])
                    nc.scalar.dma_start(xt[64:P, :], src[64:P, :])
                else:
                    nc.sync.dma_start(xt[:], src)


def emit_probe_pe_f16l(tc, x_ap, lo_ap, wc_sb, w8_sb, repeat=1):
    """Ablation probe: one quad loaded, then the full f16l matmul schedule
    (fp16 N=320 + fp8e3 N=160 per tile per k-tile) re-reads it."""
    nc = tc.nc
    T = x_ap.shape[1] // J
    n_tiles = T // P
    E2 = 2 * E
    E3 = 3 * E
    with (
        tc.tile_pool(name="xpool", bufs=1) as xpool,
        tc.tile_pool(name="psum", bufs=2, space="PSUM") as psum_pool,
    ):
        xt = xpool.tile([P, 4 * P * J], F16)
        nc.sync.dma_start(xt[:], x_ap[:, 0 : 4 * P * J])
        lt = xpool.tile([P, 4 * P * J], F8E3)
        nc.scalar.dma_start(lt[:], lo_ap[:, 0 : 4 * P * J])
        for _rep in range(repeat):
            for _tt0 in range(0, n_tiles, 4):
                psh = [
                    psum_pool.tile([P, 512], F32, name="psh", tag=f"psh{k}")
                    for k in range(4)
                ]
                for j in range(J):
                    for k in range(4):
                        xsl = slice((k * J + j) * P, (k * J + j + 1) * P)
                        nc.tensor.matmul(
                            psh[k][:, 0:E2], xt[:, xsl],
                            wc_sb[:, j * E2 : (j + 1) * E2],
                            start=(j == 0), stop=False,
                        )
                        nc.tensor.matmul(
                            psh[k][:, E2:E3], lt[:, xsl],
                            w8_sb[:, j * E : (j + 1) * E],
                            start=False, stop=(j == J - 1),
                        )


def emit_probe_pe(tc, x_ap, wc_sb, repeat=1):
    """Ablation probe: ONE pair loaded + split, then the full hilo3g
    matmul schedule re-reads the same hi/lo tiles (no per-pair DMA, no
    epilogue). The repeat-slope is the pure steady-state PE rate with
    identical instruction shapes to the real kernel."""
    nc = tc.nc
    T = x_ap.shape[1] // J
    n_tiles = T // P
    E2 = 2 * E
    with (
        tc.tile_pool(name="xpool", bufs=1) as xpool,
        tc.tile_pool(name="hpool", bufs=1) as hpool,
        tc.tile_pool(name="lpool", bufs=1) as lpool,
        tc.tile_pool(name="psum", bufs=3, space="PSUM") as psum_pool,
    ):
        xt = xpool.tile([P, 2 * P * J], F32)
        nc.sync.dma_start(xt[:], x_ap[:, 0 : 2 * P * J])
        hi = hpool.tile([P, 2 * P * J], BF16)
        nc.scalar.copy(hi[:], xt[:])
        lo = lpool.tile([P, 2 * P * J], BF16)
        nc.vector.scalar_tensor_tensor(
            lo[:], xt[:], 1.0, hi[:], op0=ALU.mult, op1=ALU.subtract
        )
        for _rep in range(repeat):
            for tt0 in range(0, n_tiles, 2):
                psh = [
                    psum_pool.tile([P, 512], F32, name="psh", tag=f"psh{k}")
                    for k in range(2)
                ]
                for j in range(J):
                    for k in range(2):
                        xsl = slice((k * J + j) * P, (k * J + j + 1) * P)
                        nc.tensor.matmul(
                            psh[k][:, 0:E2], hi[:, xsl],
                            wc_sb[:, j * E2 : (j + 1) * E2],
                            start=(j == 0), stop=False,
                        )
                        nc.tensor.matmul(
                            psh[k][:, 0:E], lo[:, xsl],
                            wc_sb[:, j * E2 : j * E2 + E],
                            start=False, stop=(j == J - 1),
                        )


def emit_gate_hilo(tc, x_ap, whi_ap, wlo_ap, oi_ap, ow_ap, terms=3):
    """Split-precision gate: x and W decomposed as bf16 hi + lo; logits =
    hi@Whi + hi@Wlo + lo@Whi (+ lo@Wlo with terms=4) accumulated in fp32
    PSUM (error ~2^-18). bf16 matmuls run ~4x faster than fp32 on the PE.
    W's split is precomputed on host; x's is done on-chip (ACT casts hi,
    DVE computes lo = x - hi)."""
    nc = tc.nc
    T = x_ap.shape[0]
    assert T % P == 0
    n_tiles = T // P

    with (
        tc.tile_pool(name="wpool", bufs=1) as wpool,
        tc.tile_pool(name="xpool", bufs=3) as xpool,
        tc.tile_pool(name="hpool", bufs=3) as hpool,
        tc.tile_pool(name="lpool", bufs=3) as lpool,
        tc.tile_pool(name="psum", bufs=4, space="PSUM") as psum_pool,
        tc.tile_pool(name="small", bufs=6) as small,
        tc.tile_pool(name="bigt", bufs=3) as bigt,
    ):
        whi_sb = wpool.tile([P, J * E], BF16)
        nc.sync.dma_start(whi_sb[:], whi_ap)
        wlo_sb = wpool.tile([P, J * E], BF16)
        nc.sync.dma_start(wlo_sb[:], wlo_ap)

        # process token-tiles in pairs: the two accumulation chains alternate
        # on the PE so each LDWEIGHTS can run in the background weight buffer
        # while the other chain's matmul streams
        for tt0 in range(0, n_tiles, 2):
            pair = [tt0, tt0 + 1] if tt0 + 1 < n_tiles else [tt0]
            his, los, pss = [], [], []
            for tt in pair:
                xt = xpool.tile([P, P * J], F32)
                src = x_ap[tt * P : (tt + 1) * P, :].rearrange(
                    "t (p j) -> p t j", p=P
                )
                dst = xt[:].rearrange("p (t j) -> p t j", j=J)
                # split the tile's 16K descriptors across both HWDGE rings
                # (two independent descriptor generators; measured ~15%
                # whole-kernel win over a single ring)
                half = P // 2
                nc.sync.dma_start(dst[:, :half, :], src[:, :half, :])
                nc.scalar.dma_start(dst[:, half:, :], src[:, half:, :])
                hi = hpool.tile([P, P * J], BF16)
                nc.scalar.copy(hi[:], xt[:])
                lo = lpool.tile([P, P * J], BF16)
                nc.vector.scalar_tensor_tensor(
                    lo[:], xt[:], 1.0, hi[:], op0=ALU.mult, op1=ALU.subtract
                )
                his.append(hi[:].rearrange("p (t j) -> p t j", j=J))
                los.append(lo[:].rearrange("p (t j) -> p t j", j=J))
                ps_k = psum_pool.tile([P, E], F32, name="ps", tag=f"ps{len(pss)}")
                pss.append(ps_k)

            for j in range(J):
                wsl = slice(j * E, (j + 1) * E)
                ops = [(his, whi_sb), (his, wlo_sb), (los, whi_sb)]
                if terms == 4:
                    ops.append((los, wlo_sb))
                for oi, (xs, wsb) in enumerate(ops):
                    last = j == J - 1 and oi == len(ops) - 1
                    for k in range(len(pair)):
                        nc.tensor.matmul(
                            pss[k][:], xs[k][:, :, j], wsb[:, wsl],
                            start=(j == 0 and oi == 0), stop=last,
                        )

            for k, tt in enumerate(pair):
                _emit_epilogue(tc, small, bigt, pss[k][:], oi_ap, ow_ap, tt)


def emit_gate_hilo_wide(tc, x_ap, wc_ap, oi_ap, ow_ap):
    """EXPERIMENTAL - DOES NOT COMPILE (walrus birverifier asserts on the
    N=320 matmul; root cause unidentified). Do not select mode "hilo4w".

    Like emit_gate_hilo(terms=4) but with Whi|Wlo concatenated into one
    N=320 moving operand, halving the matmul (and stationary-reload) count:
    two accumulation chains hi@[Whi|Wlo] and lo@[Whi|Wlo] into [128,320]
    PSUM tiles, folded into logits with three DVE adds."""
    nc = tc.nc
    T = x_ap.shape[0]
    assert T % P == 0
    n_tiles = T // P
    E2 = 2 * E

    with (
        tc.tile_pool(name="wpool", bufs=1) as wpool,
        tc.tile_pool(name="xpool", bufs=3) as xpool,
        tc.tile_pool(name="hpool", bufs=3) as hpool,
        tc.tile_pool(name="lpool", bufs=3) as lpool,
        tc.tile_pool(name="psum", bufs=3, space="PSUM") as psum_pool,
        tc.tile_pool(name="small", bufs=6) as small,
        tc.tile_pool(name="bigt", bufs=4) as bigt,
    ):
        wc_sb = wpool.tile([P, J * E2], BF16)
        nc.sync.dma_start(wc_sb[:], wc_ap)

        for tt in range(n_tiles):
            xt = xpool.tile([P, P * J], F32)
            src = x_ap[tt * P : (tt + 1) * P, :].rearrange("t (p j) -> p t j", p=P)
            nc.sync.dma_start(xt[:].rearrange("p (t j) -> p t j", j=J), src)
            hi = hpool.tile([P, P * J], BF16)
            nc.scalar.copy(hi[:], xt[:])
            lo = lpool.tile([P, P * J], BF16)
            nc.vector.scalar_tensor_tensor(
                lo[:], xt[:], 1.0, hi[:], op0=ALU.mult, op1=ALU.subtract
            )
            hi3 = hi[:].rearrange("p (t j) -> p t j", j=J)
            lo3 = lo[:].rearrange("p (t j) -> p t j", j=J)

            ps_h = psum_pool.tile([P, 512], F32, name="ps_h", tag="psh")[:, :E2]
            ps_l = psum_pool.tile([P, 512], F32, name="ps_l", tag="psl")[:, :E2]
            for src3, pst in ((hi3, ps_h), (lo3, ps_l)):
                for j in range(J):
                    wsl = slice(j * E2, (j + 1) * E2)
                    nc.tensor.matmul(
                        pst[:], src3[:, :, j], wc_sb[:, wsl],
                        start=(j == 0), stop=(j == J - 1),
                    )

            # logits = hi@Whi + hi@Wlo + lo@Whi + lo@Wlo
            ha = bigt.tile([P, E], F32)
            nc.vector.tensor_add(ha[:], ps_h[:, 0:E], ps_h[:, E:E2])
            la = bigt.tile([P, E], F32)
            nc.vector.tensor_add(la[:], ps_l[:, 0:E], ps_l[:, E:E2])
            lg = bigt.tile([P, E], F32)
            nc.vector.tensor_add(lg[:], ha[:], la[:])

            _emit_epilogue(tc, small, bigt, lg[:], oi_ap, ow_ap, tt)


def _emit_epilogue(tc, small, bigt, ps, oi_ap, ow_ap, tt):
    """ps: [P, E] AP of raw logits (PSUM or SBUF)."""
    nc = tc.nc
    ps3 = ps.rearrange("p (g i) -> p g i", i=EG)
    gmax = small.tile([P, G], F32)
    nc.vector.tensor_reduce(gmax[:], ps3, axis=AX.X, op=ALU.max)
    gsort = small.tile([P, 8], F32)
    nc.vector.max(gsort[:], gmax[:])
    gpen = small.tile([P, G], F32)
    nc.vector.tensor_scalar(
        gpen[:], gmax[:], gsort[:, TOPK_GROUP - 1 : TOPK_GROUP], NEG_BIG,
        op0=ALU.is_lt, op1=ALU.mult,
    )
    masked = bigt.tile([P, E], F32)
    nc.vector.scalar_tensor_tensor(
        masked[:].rearrange("p (g i) -> p g i", i=EG),
        ps3, 1.0,
        gpen[:, :, None].to_broadcast((P, G, EG)),
        op0=ALU.mult, op1=ALU.add,
    )
    v8 = small.tile([P, 8], F32)
    nc.vector.max(v8[:], masked[:])
    i8 = small.tile([P, 8], U32)
    nc.vector.max_index(i8[:], v8[:], masked[:])
    nrmax = small.tile([P, 1], F32)
    nc.vector.tensor_scalar_mul(nrmax[:], v8[:, 0:1], -1.0)
    exps = bigt.tile([P, E], F32)
    ssum = small.tile([P, 1], F32)
    nc.scalar.activation(
        exps[:], ps, ACTF.Exp, bias=nrmax[:], scale=1.0, accum_out=ssum[:]
    )
    rcp = small.tile([P, 1], F32)
    nc.vector.reciprocal(rcp[:], ssum[:])
    scl = small.tile([P, 1], F32)
    nc.vector.tensor_scalar_mul(scl[:], rcp[:], ROUTED_SCALING)
    e6 = small.tile([P, TOP_K], F32)
    nc.scalar.activation(e6[:], v8[:, 0:TOP_K], ACTF.Exp, bias=nrmax[:], scale=1.0)
    w6 = small.tile([P, TOP_K], F32)
    nc.vector.tensor_scalar_mul(w6[:], e6[:], scl[:])
    # outputs go out on the scalar HWDGE ring so the sync ring stays
    # dedicated to x prefetches
    nc.scalar.dma_start(oi_ap[tt * P : (tt + 1) * P, :], i8[:, 0:TOP_K])
    nc.scalar.dma_start(ow_ap[tt * P : (tt + 1) * P, :], w6[:])


def build_gate_kernel(T: int = T_CORE, repeat: int = 1, mode: str = "fp32"):
    nc = bacc.Bacc("TRN2", target_bir_lowering=False, debug=False, num_devices=N_CORES)
    oi_d = nc.dram_tensor("oi", [T, TOP_K], U32, kind="ExternalOutput")
    ow_d = nc.dram_tensor("ow", [T, TOP_K], F32, kind="ExternalOutput")
    if repeat == 0:
        # near-empty NEFF: same I/O signature, one tiny memset+store.
        # Used as a pure dispatch/RTT reference for timing.
        if mode in ("f32r",):
            nc.dram_tensor("x", [T, H], F32R, kind="ExternalInput")
            nc.dram_tensor("w", [P, J * E_PAD], F32R, kind="ExternalInput")
        elif mode == "hilo3h":
            nc.dram_tensor("x", [P, 2 * T * J], BF16, kind="ExternalInput")
            nc.dram_tensor("wc", [P, J * 2 * E], BF16, kind="ExternalInput")
        elif mode in ("hilo3w", "hilo3g"):
            nc.dram_tensor("x", [P, T * J], F32, kind="ExternalInput")
            nc.dram_tensor("wc", [P, J * 2 * E], BF16, kind="ExternalInput")
        else:
            nc.dram_tensor("x", [P, T * J], F32, kind="ExternalInput")
            nc.dram_tensor("whi", [P, J * E], BF16, kind="ExternalInput")
            nc.dram_tensor("wlo", [P, J * E], BF16, kind="ExternalInput")
        with TileContext(nc) as tc:
            with tc.tile_pool(name="zpool", bufs=1) as zp:
                z = zp.tile([P, TOP_K], U32)
                tc.nc.vector.memset(z[:], 0)
                tc.nc.sync.dma_start(oi_d.ap()[0:P, :], z[:])
                zw = zp.tile([P, TOP_K], F32)
                tc.nc.vector.memset(zw[:], 0)
                tc.nc.sync.dma_start(ow_d.ap()[0:P, :], zw[:])
        nc.compile()
        return nc
    if mode == "hilo4w":
        x_d = nc.dram_tensor("x", [T, H], F32, kind="ExternalInput")
        wc_d = nc.dram_tensor("wc", [P, J * 2 * E], BF16, kind="ExternalInput")
        with TileContext(nc) as tc:
            for _ in range(repeat):
                emit_gate_hilo_wide(tc, x_d.ap(), wc_d.ap(), oi_d.ap(), ow_d.ap())
    elif mode == "f32r":
        x_d = nc.dram_tensor("x", [P, T * J], F32R, kind="ExternalInput")
        w_d = nc.dram_tensor("w", [P, J * E_PAD], F32R, kind="ExternalInput")
        with TileContext(nc) as tc:
            for _ in range(repeat):
                emit_gate_f32r(tc, x_d.ap(), w_d.ap(), oi_d.ap(), ow_d.ap())
    elif mode == "hilo3f":
        x_d = nc.dram_tensor("x", [P, T * J], F32, kind="ExternalInput")
        whi_d = nc.dram_tensor("whi", [P, J * E], BF16, kind="ExternalInput")
        wlo_d = nc.dram_tensor("wlo", [P, J * E], BF16, kind="ExternalInput")
        with TileContext(nc) as tc:
            for _ in range(repeat):
                emit_gate_hilo3f(
                    tc, x_d.ap(), whi_d.ap(), wlo_d.ap(), oi_d.ap(), ow_d.ap()
                )
    elif mode == "hilo3w":
        x_d = nc.dram_tensor("x", [P, T * J], F32, kind="ExternalInput")
        wc_d = nc.dram_tensor("wc", [P, J * 2 * E], BF16, kind="ExternalInput")
        with TileContext(nc) as tc:
            for _ in range(repeat):
                emit_gate_hilo3w(
                    tc, x_d.ap(), wc_d.ap(), oi_d.ap(), ow_d.ap()
                )
    elif mode == "hilo3g":
        x_d = nc.dram_tensor("x", [P, T * J], F32, kind="ExternalInput")
        wc_d = nc.dram_tensor("wc", [P, J * 2 * E], BF16, kind="ExternalInput")
        with TileContext(nc) as tc:
            with tc.tile_pool(name="wpool", bufs=1) as wpool:
                wc_sb = wpool.tile([P, J * 2 * E], BF16)
                tc.nc.sync.dma_start(wc_sb[:], wc_d.ap())
                emit_gate_hilo3g(
                    tc, x_d.ap(), wc_sb, oi_d.ap(), ow_d.ap(), repeat=repeat
                )
    elif mode == "hilo3h":
        x_d = nc.dram_tensor("x", [P, 2 * T * J], BF16, kind="ExternalInput")
        wc_d = nc.dram_tensor("wc", [P, J * 2 * E], BF16, kind="ExternalInput")
        with TileContext(nc) as tc:
            with tc.tile_pool(name="wpool", bufs=1) as wpool:
                wc_sb = wpool.tile([P, J * 2 * E], BF16)
                tc.nc.sync.dma_start(wc_sb[:], wc_d.ap())
                for _ in range(repeat):
                    emit_gate_hilo3h(
                        tc, x_d.ap(), wc_sb, oi_d.ap(), ow_d.ap()
                    )
    elif mode == "f16h":
        x_d = nc.dram_tensor("x", [P, T * J], F16, kind="ExternalInput")
        wc_d = nc.dram_tensor("wc", [P, J * 2 * E], F16, kind="ExternalInput")
        with TileContext(nc) as tc:
            with tc.tile_pool(name="wpool", bufs=1) as wpool:
                wc_sb = wpool.tile([P, J * 2 * E], F16)
                tc.nc.sync.dma_start(wc_sb[:], wc_d.ap())
                emit_gate_f16(
                    tc, x_d.ap(), wc_sb, oi_d.ap(), ow_d.ap(), repeat=repeat
                )
    elif mode == "f16l":
        x_d = nc.dram_tensor("x", [P, T * J], F16, kind="ExternalInput")
        lo_d = nc.dram_tensor("xlo", [P, T * J], F8E3, kind="ExternalInput")
        wc_d = nc.dram_tensor("wc", [P, J * 2 * E], F16, kind="ExternalInput")
        w8_d = nc.dram_tensor("w8", [P, J * E], F8E3, kind="ExternalInput")
        with TileContext(nc) as tc:
            with tc.tile_pool(name="wpool", bufs=1) as wpool:
                wc_sb = wpool.tile([P, J * 2 * E], F16)
                tc.nc.sync.dma_start(wc_sb[:], wc_d.ap())
                w8_sb = wpool.tile([P, J * E], F8E3)
                tc.nc.sync.dma_start(w8_sb[:], w8_d.ap())
                emit_gate_f16l(
                    tc, x_d.ap(), lo_d.ap(), wc_sb, w8_sb,
                    oi_d.ap(), ow_d.ap(), repeat=repeat,
                )
    elif mode == "probe_pe16":
        x_d = nc.dram_tensor("x", [P, T * J], F16, kind="ExternalInput")
        wc_d = nc.dram_tensor("wc", [P, J * 2 * E], F16, kind="ExternalInput")
        with TileContext(nc) as tc:
            with tc.tile_pool(name="wpool", bufs=1) as wpool:
                wc_sb = wpool.tile([P, J * 2 * E], F16)
                tc.nc.sync.dma_start(wc_sb[:], wc_d.ap())
                z = wpool.tile([P, TOP_K], U32)
                tc.nc.vector.memset(z[:], 0)
                tc.nc.sync.dma_start(oi_d.ap()[0:P, :], z[:])
                zw = wpool.tile([P, TOP_K], F32)
                tc.nc.vector.memset(zw[:], 0)
                tc.nc.sync.dma_start(ow_d.ap()[0:P, :], zw[:])
                emit_probe_pe16(tc, x_d.ap(), wc_sb, repeat=repeat)
    elif mode == "f16x":
        x_d = nc.dram_tensor("x", [P, T * J], F16, kind="ExternalInput")
        lo_d = nc.dram_tensor("xlo", [P, T * J], F8E3, kind="ExternalInput")
        wh_d = nc.dram_tensor("wh", [P, J * E], F16, kind="ExternalInput")
        wl8_d = nc.dram_tensor("wl8", [P, J * E], F8E3, kind="ExternalInput")
        w8_d = nc.dram_tensor("w8", [P, J * E], F8E3, kind="ExternalInput")
        with TileContext(nc) as tc:
            with tc.tile_pool(name="wpool", bufs=1) as wpool:
                wh_sb = wpool.tile([P, J * E], F16)
                tc.nc.sync.dma_start(wh_sb[:], wh_d.ap())
                wl8_sb = wpool.tile([P, J * E], F8E3)
                tc.nc.sync.dma_start(wl8_sb[:], wl8_d.ap())
                w8_sb = wpool.tile([P, J * E], F8E3)
                tc.nc.sync.dma_start(w8_sb[:], w8_d.ap())
                emit_gate_f16x(
                    tc, x_d.ap(), lo_d.ap(), wh_sb, wl8_sb, w8_sb,
                    oi_d.ap(), ow_d.ap(), repeat=repeat,
                )
    elif mode == "probe_pe_lo":
        lo_d = nc.dram_tensor("xlo", [P, T * J], F8E3, kind="ExternalInput")
        w8_d = nc.dram_tensor("w8", [P, J * E], F8E3, kind="ExternalInput")
        with TileContext(nc) as tc:
            with tc.tile_pool(name="wpool", bufs=1) as wpool:
                w8_sb = wpool.tile([P, J * E], F8E3)
                tc.nc.sync.dma_start(w8_sb[:], w8_d.ap())
                z = wpool.tile([P, TOP_K], U32)
                tc.nc.vector.memset(z[:], 0)
                tc.nc.sync.dma_start(oi_d.ap()[0:P, :], z[:])
                zw = wpool.tile([P, TOP_K], F32)
                tc.nc.vector.memset(zw[:], 0)
                tc.nc.sync.dma_start(ow_d.ap()[0:P, :], zw[:])
                emit_probe_pe_lo(tc, lo_d.ap(), w8_sb, repeat=repeat)
    elif mode == "probe_pe_f16l":
        x_d = nc.dram_tensor("x", [P, T * J], F16, kind="ExternalInput")
        lo_d = nc.dram_tensor("xlo", [P, T * J], F8E3, kind="ExternalInput")
        wc_d = nc.dram_tensor("wc", [P, J * 2 * E], F16, kind="ExternalInput")
        w8_d = nc.dram_tensor("w8", [P, J * E], F8E3, kind="ExternalInput")
        with TileContext(nc) as tc:
            with tc.tile_pool(name="wpool", bufs=1) as wpool:
                wc_sb = wpool.tile([P, J * 2 * E], F16)
                tc.nc.sync.dma_start(wc_sb[:], wc_d.ap())
                w8_sb = wpool.tile([P, J * E], F8E3)
                tc.nc.sync.dma_start(w8_sb[:], w8_d.ap())
                z = wpool.tile([P, TOP_K], U32)
                tc.nc.vector.memset(z[:], 0)
                tc.nc.sync.dma_start(oi_d.ap()[0:P, :], z[:])
                zw = wpool.tile([P, TOP_K], F32)
                tc.nc.vector.memset(zw[:], 0)
                tc.nc.sync.dma_start(ow_d.ap()[0:P, :], zw[:])
                emit_probe_pe_f16l(
                    tc, x_d.ap(), lo_d.ap(), wc_sb, w8_sb, repeat=repeat
                )
    elif mode in ("probe_dma", "probe_dma2", "probe_pe"):
        x_d = nc.dram_tensor("x", [P, T * J], F32, kind="ExternalInput")
        wc_d = nc.dram_tensor("wc", [P, J * 2 * E], BF16, kind="ExternalInput")
        with TileContext(nc) as tc:
            with tc.tile_pool(name="wpool", bufs=1) as wpool:
                wc_sb = wpool.tile([P, J * 2 * E], BF16)
                tc.nc.sync.dma_start(wc_sb[:], wc_d.ap())
                z = wpool.tile([P, TOP_K], U32)
                tc.nc.vector.memset(z[:], 0)
                tc.nc.sync.dma_start(oi_d.ap()[0:P, :], z[:])
                zw = wpool.tile([P, TOP_K], F32)
                tc.nc.vector.memset(zw[:], 0)
                tc.nc.sync.dma_start(ow_d.ap()[0:P, :], zw[:])
                if mode == "probe_dma":
                    emit_probe_dma(tc, x_d.ap(), repeat=repeat)
                elif mode == "probe_dma2":
                    emit_probe_dma(tc, x_d.ap(), repeat=repeat, rings=2)
                else:
                    emit_probe_pe(tc, x_d.ap(), wc_sb, repeat=repeat)
    elif mode in ("hilo", "hilo4"):
        x_d = nc.dram_tensor("x", [T, H], F32, kind="ExternalInput")
        whi_d = nc.dram_tensor("whi", [P, J * E], BF16, kind="ExternalInput")
        wlo_d = nc.dram_tensor("wlo", [P, J * E], BF16, kind="ExternalInput")
        with TileContext(nc) as tc:
            for _ in range(repeat):
                emit_gate_hilo(
                    tc, x_d.ap(), whi_d.ap(), wlo_d.ap(), oi_d.ap(), ow_d.ap(),
                    terms=4 if mode == "hilo4" else 3,
                )
    else:
        x_d = nc.dram_tensor("x", [T, H], F32, kind="ExternalInput")
        w_d = nc.dram_tensor("w", [P, J * E], F32, kind="ExternalInput")
        with TileContext(nc) as tc:
            for _ in range(repeat):
                emit_gate(tc, x_d.ap(), w_d.ap(), oi_d.ap(), ow_d.ap())
    nc.compile()
    return nc


def prep_weight(weight: np.ndarray) -> np.ndarray:
    """[160, 5120] -> [128, 40*160] with w[p, j*E + e] = W[e, p*40 + j]."""
    wt = np.asarray(weight, dtype=np.float32).T  # [H, E]
    return np.ascontiguousarray(wt.reshape(P, J, E)).reshape(P, J * E)


def prep_weight_f32r(weight: np.ndarray) -> np.ndarray:
    """[160, 5120] -> [128, 40*256], w[p, j*E_PAD + e] = W[e, p*40 + j]
    (zero for e >= 160)."""
    wt = np.asarray(weight, dtype=np.float32).T  # [H, E]
    wp = np.zeros((H, E_PAD), np.float32)
    wp[:, :E] = wt
    return np.ascontiguousarray(wp.reshape(P, J, E_PAD)).reshape(P, J * E_PAD)


def prep_weight_f16(weight: np.ndarray) -> np.ndarray:
    """[160, 5120] -> [P, J*2E] fp16: per j-block [Whi | Wlo * 2^11].

    Whi is fp16(W) with denormals flushed to zero host-side (so a PE that
    flushes fp16 denormals sees exactly the value Wlo was computed
    against); Wlo is scaled by 2^11 into fp16 normal range and un-scaled
    in the kernel epilogue. W split error ~2^-22."""
    w = np.asarray(weight, dtype=np.float32)
    whi = w.astype(np.float16)
    whi_f = np.where(np.abs(whi.astype(np.float32)) < 6.104e-5, 0.0, whi.astype(np.float32))
    whi = whi_f.astype(np.float16)
    wlo = ((w - whi.astype(np.float32)) * 2048.0).astype(np.float16)

    def perm(a):
        return np.ascontiguousarray(a.astype(np.float16).T.reshape(P, J, E))

    return np.ascontiguousarray(
        np.concatenate([perm(whi), perm(wlo)], axis=2)
    ).reshape(P, J * 2 * E)


def prep_weight_f8(weight: np.ndarray) -> np.ndarray:
    """[160, 5120] -> [P, J*E] fp8e3m4 of W * 2^6, for the lo term."""
    import ml_dtypes

    w = np.asarray(weight, dtype=np.float32) * 64.0
    w8 = w.astype(ml_dtypes.float8_e3m4)
    return np.ascontiguousarray(w8.T.reshape(P, J, E)).reshape(P, J * E)


def prep_weight_hilo(weight: np.ndarray):
    import ml_dtypes

    w = np.asarray(weight, dtype=np.float32)
    whi = w.astype(ml_dtypes.bfloat16)
    wlo = (w - whi.astype(np.float32)).astype(ml_dtypes.bfloat16)

    def perm(a):
        return np.ascontiguousarray(a.T.reshape(P, J, E)).reshape(P, J * E)

    return perm(whi), perm(wlo)


_NC_CACHE = {}


# "hilo3g" = 3-term bf16 split matmul on the fast-DMA [p, tile, j, t]
# layout (line-rate 20KB-contiguous x loads, contiguous per-j stationary
# slices), with Whi|Wlo fused into one N=320 moving operand (2 matmuls per
# k-tile), the weight tile resident across repeats, and one 5.24MB DMA per
# token-tile pair. Measured 94.1us vs hilo4's 278.6us baseline; 6/98304
# near-tie index swaps, rel err 4.8e-3 (gate is 2e-2). "hilo4" kept as the
# old fallback; "f32r" is faster on paper but its ~11-bit operand
# truncation puts rel err at 1.99e-2 — disqualified.
MODE = "f16l"


def make_in_maps(hidden_states, weight, mode=None):
    mode = mode or MODE
    hs = np.ascontiguousarray(
        np.asarray(hidden_states, dtype=np.float32).reshape(T_TOTAL, H)
    )
    shards = hs.reshape(N_CORES, T_CORE, H)
    if mode in ("f16h", "f16l", "f16x", "probe_pe16"):
        # fast-DMA layout, fp16: xp[p, ((tile*J)+j)*P + t] = x[tile*P+t, p*J+j]
        n_tiles = T_CORE // P
        xs = hs.reshape(N_CORES, n_tiles, P, P, J)  # [c, tile, t, p, j]
        wc = prep_weight_f16(weight)
        maps = []
        for c in range(N_CORES):
            xc = np.ascontiguousarray(xs[c].transpose(2, 0, 3, 1)).reshape(
                P, T_CORE * J
            )
            xh = xc.astype(np.float16)
            # flush fp16 denormals host-side so a PE that FTZs sees the
            # exact value the lo residual was computed against
            xh = np.where(
                np.abs(xh.astype(np.float32)) < 6.104e-5, 0, xh
            ).astype(np.float16)
            if mode == "f16x":
                import ml_dtypes

                lo = (xc - xh.astype(np.float32)) * 4096.0  # 2^12
                m = {"x": xh, "xlo": lo.astype(ml_dtypes.float8_e3m4)}
                w = np.asarray(weight, dtype=np.float32)
                whi = w.astype(np.float16)
                whi = np.where(
                    np.abs(whi.astype(np.float32)) < 6.104e-5, 0, whi
                ).astype(np.float16)
                wl8 = ((w - whi.astype(np.float32)) * 2.0 ** 17).astype(
                    ml_dtypes.float8_e3m4
                )

                def perm(a):
                    return np.ascontiguousarray(a.T.reshape(P, J, E)).reshape(
                        P, J * E
                    )

                m["wh"] = perm(whi)
                m["wl8"] = perm(wl8)
                m["w8"] = prep_weight_f8(weight)
            else:
                m = {"x": xh, "wc": wc}
                if mode == "f16l":
                    import ml_dtypes

                    lo = (xc - xh.astype(np.float32)) * 4096.0  # 2^12
                    m["xlo"] = lo.astype(ml_dtypes.float8_e3m4)
                    m["w8"] = prep_weight_f8(weight)
            maps.append(m)
        return maps
    if mode in ("probe_pe_f16l", "probe_pe_lo"):
        maps = make_in_maps(hidden_states, weight, "f16l")
        if mode == "probe_pe_lo":
            maps = [{"xlo": m["xlo"], "w8": m["w8"]} for m in maps]
        return maps
    if mode in ("f32r", "hilo3f", "hilo3w", "hilo3g", "hilo3h", "probe_dma", "probe_dma2", "probe_pe"):
        # x[tile*P + t, p*J + j] -> xp[p, ((tile*J)+j)*P + t]: every
        # token-tile DMA is one contiguous 20KB run per partition, and each
        # k-tile's stationary slice is contiguous in SBUF.
        n_tiles = T_CORE // P
        xs = shards.reshape(N_CORES, n_tiles, P, P, J)  # [c, tile, t, p, j]
        xps = [
            np.ascontiguousarray(xs[c].transpose(2, 0, 3, 1)).reshape(
                P, T_CORE * J
            )
            for c in range(N_CORES)
        ]
        if mode == "f32r":
            wf = prep_weight_f32r(weight)
            return [{"x": xps[c], "w": wf} for c in range(N_CORES)]
        whi, wlo = prep_weight_hilo(weight)
        if mode == "hilo3h":
            import ml_dtypes

            wc = np.ascontiguousarray(
                np.concatenate(
                    [whi.reshape(P, J, E), wlo.reshape(P, J, E)], axis=2
                ).reshape(P, J * 2 * E)
            )
            n_pairs = T_CORE // P // 2
            maps = []
            for c in range(N_CORES):
                hi = xps[c].astype(ml_dtypes.bfloat16)
                lo = (xps[c] - hi.astype(np.float32)).astype(ml_dtypes.bfloat16)
                h3 = hi.reshape(P, n_pairs, 2 * J * P)
                l3 = lo.reshape(P, n_pairs, 2 * J * P)
                xc = np.concatenate(
                    [h3[:, :, None, :], l3[:, :, None, :]], axis=2
                ).reshape(P, 2 * T_CORE * J)
                maps.append({"x": np.ascontiguousarray(xc), "wc": wc})
            return maps
        if mode in ("hilo3w", "hilo3g", "probe_dma", "probe_dma2", "probe_pe"):
            wc = np.concatenate(
                [whi.reshape(P, J, E), wlo.reshape(P, J, E)], axis=2
            ).reshape(P, J * 2 * E)
            return [
                {"x": xps[c], "wc": np.ascontiguousarray(wc)}
                for c in range(N_CORES)
            ]
        return [
            {"x": xps[c], "whi": whi, "wlo": wlo} for c in range(N_CORES)
        ]
    if mode == "hilo4w":
        whi, wlo = prep_weight_hilo(weight)
        wc = np.concatenate(
            [whi.reshape(P, J, E), wlo.reshape(P, J, E)], axis=2
        ).reshape(P, J * 2 * E)
        wc = np.ascontiguousarray(wc)
        return [{"x": shards[c], "wc": wc} for c in range(N_CORES)]
    if mode in ("hilo", "hilo4"):
        whi, wlo = prep_weight_hilo(weight)
        return [
            {"x": shards[c], "whi": whi, "wlo": wlo} for c in range(N_CORES)
        ]
    wr = prep_weight(weight)
    return [{"x": shards[c], "w": wr} for c in range(N_CORES)]


def run(hidden_states, weight, trace=False, mode=None):
    mode = mode or MODE
    in_maps = make_in_maps(hidden_states, weight, mode)
    if mode not in _NC_CACHE:
        _NC_CACHE[mode] = build_gate_kernel(mode=mode)
    nc = _NC_CACHE[mode]
    res = bass_utils.run_bass_kernel_spmd(
        nc, in_maps, core_ids=list(range(N_CORES)), trace=trace
    )
    idx = np.concatenate([r["oi"].astype(np.int32) for r in res.results], axis=0)
    wts = np.concatenate([r["ow"] for r in res.results], axis=0)
    return (idx, wts), res


def kernel(hidden_states, weight):
    (idx, wts), _ = run(hidden_states, weight)
    return idx, wts



# revision 29
# speedup vs baseline: 1.5502x; 1.5502x over previous
"""DeepSeek-V2 MoE gate (group-limited greedy top-k routing) on 8 trn2 NeuronCores.

Reference computation (per token t over E=160 experts in G=8 groups of 20):
    logits = x @ W^T                       [T, E]
    scores = softmax(logits)
    group_scores[g] = max over group g of scores
    keep top-3 groups; mask scores of other groups to 0
    topk_weight, topk_idx = top_k(masked scores, 6); topk_weight *= 16.0

Sharding: tokens (B*S = 16384) split evenly across the 8 cores; the small
[160, 5120] gate weight is replicated (pre-arranged host-side).

The kernel is DMA-bound: each core must read its 41.9MB x shard once, and
the SBUF fabric ceiling (~435 GB/s) puts the floor near 100us. Everything
else is arranged to stay under that roofline:

- The tensor engine contracts over the partition axis, so both matmul
  operands need hidden (H=5120) on partitions. Host-side prep lays the
  shard out as xp[p, tile, j, t] = x[tile*128 + t, p*40 + j], making every
  token-tile load one fully contiguous 20KB-per-partition run (line rate)
  and every per-k-tile stationary slice contiguous in SBUF.
- Precision comes from a 3-term bf16 split (x = hi + lo, W = Whi + Wlo;
  logits = hi@Whi + hi@Wlo + lo@Whi accumulated in fp32 PSUM, error
  ~2^-18). Single-pass float32r would be ~10% faster on paper but its
  ~11-bit operand truncation flips too many near-tie expert picks
  (rel err 1.99e-2 vs the 2e-2 gate).
- The winning mode "hilo3g" fuses Whi|Wlo into one N=320 moving operand
  (2 matmuls per k-tile instead of 3), interleaves the hi and lo term
  streams into a single PSUM accumulation group per bank (psh[0:160] =
  hi@Whi + lo@Whi, psh[160:320] = hi@Wlo, folded with one copy+add),
  keeps the weight tile resident, fuses each token-tile pair's x load
  into a single 5.24MB DMA, rotates tile buffers seamlessly across
  repeat boundaries, and sends outputs on the scalar HWDGE ring so the
  sync ring stays dedicated to x prefetches. Measured ~90us/core vs the
  278.6us baseline; 6/98304 near-tie index swaps, rel err 4.8e-3.

Selection runs on raw logits (softmax is monotonic; the top-3-group test by
max-score equals the test by max-logit), so only the final 6 weights and the
softmax denominator need exp().
"""

import numpy as np

import concourse.bacc as bacc
import concourse.mybir as mybir
from concourse import bass_utils
from concourse.tile import TileContext

# Problem constants (hardcoded per the harness contract).
B, S, H = 4, 4096, 5120
E = 160                 # experts
G = 8                   # groups
EG = E // G             # experts per group (20)
TOP_K = 6
TOPK_GROUP = 3
ROUTED_SCALING = 16.0
N_CORES = 8
T_TOTAL = B * S         # 16384
T_CORE = T_TOTAL // N_CORES  # 2048
P = 128                 # SBUF partitions
J = H // P              # hidden values per partition (40) = number of k-tiles
NEG_BIG = -1.0e30

F32 = mybir.dt.float32
F32R = mybir.dt.float32r  # fp32 the PE streams at bf16 rate (moving dim
                          # >=256) but with ~11-bit operand truncation
BF16 = mybir.dt.bfloat16
F16 = mybir.dt.float16
F8E3 = mybir.dt.float8e3  # e3m4: 5 mantissa bits, range [2^-6, 15.5]
U32 = mybir.dt.uint32
ALU = mybir.AluOpType
ACTF = mybir.ActivationFunctionType
AX = mybir.AxisListType


def emit_gate(tc, x_ap, w_ap, oi_ap, ow_ap):
    """Emit the gate kernel body into TileContext `tc`.

    x_ap:  [T, H] f32 DRAM (T % 128 == 0)
    w_ap:  [P, J*E] f32 DRAM (pre-permuted weight, see module docstring)
    oi_ap: [T, TOP_K] u32 DRAM out (expert indices)
    ow_ap: [T, TOP_K] f32 DRAM out (routing weights)
    """
    nc = tc.nc
    T = x_ap.shape[0]
    assert T % P == 0
    n_tiles = T // P

    with (
        tc.tile_pool(name="wpool", bufs=1) as wpool,
        tc.tile_pool(name="xpool", bufs=3) as xpool,
        tc.tile_pool(name="psum", bufs=4, space="PSUM") as psum_pool,
        tc.tile_pool(name="small", bufs=6) as small,
        tc.tile_pool(name="bigt", bufs=3) as bigt,
    ):
        w_sb = wpool.tile([P, J * E], F32)
        nc.sync.dma_start(w_sb[:], w_ap)

        for tt in range(n_tiles):
            # x tile: [p, t*J + j] = x[t0 + t, p*J + j]
            xt = xpool.tile([P, P * J], F32)
            src = x_ap[tt * P : (tt + 1) * P, :].rearrange("t (p j) -> p t j", p=P)
            nc.sync.dma_start(xt[:].rearrange("p (t j) -> p t j", j=J), src)
            xt3 = xt[:].rearrange("p (t j) -> p t j", j=J)

            # logits[t, e] accumulated over the 40 k-tiles
            ps = psum_pool.tile([P, E], F32)
            for j in range(J):
                nc.tensor.matmul(
                    ps[:],
                    xt3[:, :, j],                  # stationary [128h, 128t]
                    w_sb[:, j * E : (j + 1) * E],  # moving     [128h, 160e]
                    start=(j == 0),
                    stop=(j == J - 1),
                )

            ps3 = ps[:].rearrange("p (g i) -> p g i", i=EG)

            # group max of logits -> top-3-group additive penalty mask
            gmax = small.tile([P, G], F32)
            nc.vector.tensor_reduce(gmax[:], ps3, axis=AX.X, op=ALU.max)
            gsort = small.tile([P, 8], F32)
            nc.vector.max(gsort[:], gmax[:])
            gpen = small.tile([P, G], F32)  # 0 for kept groups, NEG_BIG for dropped
            nc.vector.tensor_scalar(
                gpen[:], gmax[:], gsort[:, TOPK_GROUP - 1 : TOPK_GROUP], NEG_BIG,
                op0=ALU.is_lt, op1=ALU.mult,
            )

            # masked logits = logits + penalty(group)
            masked = bigt.tile([P, E], F32)
            nc.vector.scalar_tensor_tensor(
                masked[:].rearrange("p (g i) -> p g i", i=EG),
                ps3,
                1.0,
                gpen[:, :, None].to_broadcast((P, G, EG)),
                op0=ALU.mult,
                op1=ALU.add,
            )

            # top-8 masked logits (descending) + expert indices
            v8 = small.tile([P, 8], F32)
            nc.vector.max(v8[:], masked[:])
            i8 = small.tile([P, 8], U32)
            nc.vector.max_index(i8[:], v8[:], masked[:])

            # softmax pieces: global max logit is v8[:,0] (the best group holds it)
            nrmax = small.tile([P, 1], F32)
            nc.vector.tensor_scalar_mul(nrmax[:], v8[:, 0:1], -1.0)
            exps = bigt.tile([P, E], F32)
            ssum = small.tile([P, 1], F32)
            nc.scalar.activation(
                exps[:], ps[:], ACTF.Exp, bias=nrmax[:], scale=1.0, accum_out=ssum[:]
            )
            rcp = small.tile([P, 1], F32)
            nc.vector.reciprocal(rcp[:], ssum[:])
            scl = small.tile([P, 1], F32)
            nc.vector.tensor_scalar_mul(scl[:], rcp[:], ROUTED_SCALING)

            # weights = exp(v6 - rmax) * 16 / ssum
            e6 = small.tile([P, TOP_K], F32)
            nc.scalar.activation(e6[:], v8[:, 0:TOP_K], ACTF.Exp, bias=nrmax[:], scale=1.0)
            w6 = small.tile([P, TOP_K], F32)
            nc.vector.tensor_scalar_mul(w6[:], e6[:], scl[:])

            nc.sync.dma_start(oi_ap[tt * P : (tt + 1) * P, :], i8[:, 0:TOP_K])
            nc.sync.dma_start(ow_ap[tt * P : (tt + 1) * P, :], w6[:])


E_PAD = 256  # experts padded so the f32r moving operand is >=256 wide

# Fast-DMA activation layout, shared by the f32r and hilo3f modes:
# xp[p, ((tile*J) + j)*P + t] = x[tile*P + t, p*J + j]. Each token-tile's
# DMA is one fully contiguous 20KB run per partition (line rate), and the
# per-k-tile stationary slice xt[:, j*P:(j+1)*P] is contiguous in SBUF
# (for bf16 this lets the compiler's Fast Weight Load engage; a strided
# stationary AP defeats it and the kernel goes LDWEIGHTS-bound).


def emit_gate_f32r(tc, x_ap, w_ap, oi_ap, ow_ap):
    """Single-pass float32r gate.

    float32r is fp32 data the PE streams at bf16 rate (1 cycle/row) when the
    moving free dim is >=256 — below that it falls to 1/4 rate. The weight is
    therefore zero-padded from 160 to 256 experts; the epilogue only ever
    reads logits[:, :160] so the pad never enters selection.

    MEASURED: 116.9us (= the ~117us HBM roofline for the 41.9MB/core x
    read), but the f32r datapath truncates operands to ~11 mantissa bits:
    rel err 1.99e-2 vs the 2e-2 gate (hundreds of flipped near-tie 6th
    picks). Too risky to ship; kept for reference.
    """
    nc = tc.nc
    T = x_ap.shape[1] // (P * J) * P
    n_tiles = T // P

    with (
        tc.tile_pool(name="wpool", bufs=1) as wpool,
        tc.tile_pool(name="xpool", bufs=3) as xpool,
        tc.tile_pool(name="psum", bufs=4, space="PSUM") as psum_pool,
        tc.tile_pool(name="small", bufs=6) as small,
        tc.tile_pool(name="bigt", bufs=3) as bigt,
    ):
        w_sb = wpool.tile([P, J * E_PAD], F32R)
        nc.sync.dma_start(w_sb[:], w_ap)

        for tt0 in range(0, n_tiles, 2):
            pair = [tt0, tt0 + 1] if tt0 + 1 < n_tiles else [tt0]
            xts, pss = [], []
            for tt in pair:
                xt = xpool.tile([P, P * J], F32R)
                nc.sync.dma_start(
                    xt[:], x_ap[:, tt * P * J : (tt + 1) * P * J]
                )
                xts.append(xt[:])
                pss.append(psum_pool.tile([P, E_PAD], F32, name="ps", tag=f"ps{len(pss)}"))

            for j in range(J):
                for k in range(len(pair)):
                    nc.tensor.matmul(
                        pss[k][:],
                        xts[k][:, j * P : (j + 1) * P],
                        w_sb[:, j * E_PAD : (j + 1) * E_PAD],
                        start=(j == 0),
                        stop=(j == J - 1),
                    )

            for k, tt in enumerate(pair):
                _emit_epilogue(tc, small, bigt, pss[k][:, 0:E], oi_ap, ow_ap, tt)


def emit_gate_hilo3f(tc, x_ap, whi_ap, wlo_ap, oi_ap, ow_ap):
    """3-term bf16 split gate on the fast-DMA [p, tile, j, t] layout.

    logits = hi@Whi + hi@Wlo + lo@Whi, fp32 PSUM accumulation, error
    ~2^-18 (the dropped lo@Wlo term). The contiguous per-j stationary
    slice keeps LDWEIGHTS on the Fast-Weight-Load path (~53ns < the 67ns
    N=160 stream), so the PE runs at the 3x160x40 streaming floor
    (~128us/core) instead of the LDW-bound ~205us the strided layout
    gives. DMA is at line rate (~117us/core), fully overlapped.
    """
    nc = tc.nc
    T = x_ap.shape[1] // J
    n_tiles = T // P

    with (
        tc.tile_pool(name="wpool", bufs=1) as wpool,
        tc.tile_pool(name="xpool", bufs=3) as xpool,
        tc.tile_pool(name="hpool", bufs=3) as hpool,
        tc.tile_pool(name="lpool", bufs=3) as lpool,
        tc.tile_pool(name="psum", bufs=4, space="PSUM") as psum_pool,
        tc.tile_pool(name="small", bufs=6) as small,
        tc.tile_pool(name="bigt", bufs=3) as bigt,
    ):
        whi_sb = wpool.tile([P, J * E], BF16)
        nc.sync.dma_start(whi_sb[:], whi_ap)
        wlo_sb = wpool.tile([P, J * E], BF16)
        nc.sync.dma_start(wlo_sb[:], wlo_ap)

        for tt0 in range(0, n_tiles, 2):
            pair = [tt0, tt0 + 1] if tt0 + 1 < n_tiles else [tt0]
            his, los, pss = [], [], []
            for tt in pair:
                xt = xpool.tile([P, P * J], F32)
                nc.sync.dma_start(
                    xt[:], x_ap[:, tt * P * J : (tt + 1) * P * J]
                )
                hi = hpool.tile([P, P * J], BF16)
                nc.scalar.copy(hi[:], xt[:])
                lo = lpool.tile([P, P * J], BF16)
                nc.vector.scalar_tensor_tensor(
                    lo[:], xt[:], 1.0, hi[:], op0=ALU.mult, op1=ALU.subtract
                )
                his.append(hi[:])
                los.append(lo[:])
                pss.append(
                    psum_pool.tile([P, E], F32, name="ps", tag=f"ps{len(pss)}")
                )

            for j in range(J):
                xsl = slice(j * P, (j + 1) * P)
                wsl = slice(j * E, (j + 1) * E)
                ops = [(his, whi_sb), (his, wlo_sb), (los, whi_sb)]
                for oi, (xs, wsb) in enumerate(ops):
                    last = j == J - 1 and oi == len(ops) - 1
                    for k in range(len(pair)):
                        nc.tensor.matmul(
                            pss[k][:], xs[k][:, xsl], wsb[:, wsl],
                            start=(j == 0 and oi == 0), stop=last,
                        )

            for k, tt in enumerate(pair):
                _emit_epilogue(tc, small, bigt, pss[k][:], oi_ap, ow_ap, tt)


def emit_gate_hilo3w(tc, x_ap, wc_ap, oi_ap, ow_ap):
    """Like hilo3f but with Whi|Wlo concatenated per j into one N=320
    moving operand: per k-tile, 2 matmuls (hi@[Whi|Wlo], lo@Whi) instead
    of 3, cutting LDWEIGHTS/instruction count by a third at identical
    streamed-row count. logits = ps_h[:,0:160] + ps_h[:,160:320] + ps_l,
    folded with two DVE adds. wc_ap: [P, J*2E] bf16,
    wc[p, j*2E + e] = Whi[e] for e<160 else Wlo[e-160]."""
    nc = tc.nc
    T = x_ap.shape[1] // J
    n_tiles = T // P
    E2 = 2 * E

    with (
        tc.tile_pool(name="wpool", bufs=1) as wpool,
        tc.tile_pool(name="xpool", bufs=3) as xpool,
        tc.tile_pool(name="hpool", bufs=3) as hpool,
        tc.tile_pool(name="lpool", bufs=3) as lpool,
        tc.tile_pool(name="psum", bufs=2, space="PSUM") as psum_pool,
        tc.tile_pool(name="small", bufs=6) as small,
        tc.tile_pool(name="bigt", bufs=4) as bigt,
    ):
        wc_sb = wpool.tile([P, J * E2], BF16)
        nc.sync.dma_start(wc_sb[:], wc_ap)

        for tt0 in range(0, n_tiles, 2):
            pair = [tt0, tt0 + 1] if tt0 + 1 < n_tiles else [tt0]
            his, los, psh, psl = [], [], [], []
            for tt in pair:
                xt = xpool.tile([P, P * J], F32)
                nc.sync.dma_start(
                    xt[:], x_ap[:, tt * P * J : (tt + 1) * P * J]
                )
                hi = hpool.tile([P, P * J], BF16)
                nc.scalar.copy(hi[:], xt[:])
                lo = lpool.tile([P, P * J], BF16)
                nc.vector.scalar_tensor_tensor(
                    lo[:], xt[:], 1.0, hi[:], op0=ALU.mult, op1=ALU.subtract
                )
                his.append(hi[:])
                los.append(lo[:])
                # full-bank tiles so the two accumulation groups can never
                # share a PSUM bank (a group's start clears its whole bank)
                psh.append(
                    psum_pool.tile([P, 512], F32, name="psh", tag=f"psh{len(psh)}")
                )
                psl.append(
                    psum_pool.tile([P, 512], F32, name="psl", tag=f"psl{len(psl)}")
                )

            for j in range(J):
                xsl = slice(j * P, (j + 1) * P)
                for k in range(len(pair)):
                    nc.tensor.matmul(
                        psh[k][:, 0:E2], his[k][:, xsl],
                        wc_sb[:, j * E2 : (j + 1) * E2],
                        start=(j == 0), stop=(j == J - 1),
                    )
                    nc.tensor.matmul(
                        psl[k][:, 0:E], los[k][:, xsl],
                        wc_sb[:, j * E2 : j * E2 + E],
                        start=(j == 0), stop=(j == J - 1),
                    )

            for k, tt in enumerate(pair):
                # DVE/ACT may read at most one PSUM input per instruction
                hb = bigt.tile([P, E], F32)
                nc.scalar.copy(hb[:], psh[k][:, E:E2])
                ha = bigt.tile([P, E], F32)
                nc.vector.tensor_add(ha[:], psh[k][:, 0:E], hb[:])
                lg = bigt.tile([P, E], F32)
                nc.vector.tensor_add(lg[:], ha[:], psl[k][:, 0:E])
                _emit_epilogue(tc, small, bigt, lg[:], oi_ap, ow_ap, tt)


def emit_gate_hilo3g(tc, x_ap, wc_sb, oi_ap, ow_ap, repeat=1):
    """hilo3w with a resident weight tile (loaded once per NEFF, shared
    across repeats) and one fused 5.24MB DMA per token-tile pair.

    The repeat loop runs INSIDE the open tile pools so buffer rotation
    flows seamlessly across repeat boundaries (repeat r+1's first DMA
    prefetches during repeat r's tail) — the repeat-slope then measures
    pure steady-state pipeline rate, which is also the real back-to-back
    invocation throughput.

    wc_sb: [P, J*2E] bf16 SBUF AP, already loaded.
    """
    nc = tc.nc
    T = x_ap.shape[1] // J
    n_tiles = T // P
    E2 = 2 * E

    with (
        tc.tile_pool(name="xpool", bufs=2) as xpool,
        tc.tile_pool(name="hpool", bufs=2) as hpool,
        tc.tile_pool(name="lpool", bufs=2) as lpool,
        tc.tile_pool(name="psum", bufs=3, space="PSUM") as psum_pool,
        tc.tile_pool(name="small", bufs=6) as small,
        tc.tile_pool(name="bigt", bufs=4) as bigt,
    ):
        for _rep in range(repeat):
            for tt0 in range(0, n_tiles, 2):
                npair = 2 if tt0 + 1 < n_tiles else 1
                xt = xpool.tile([P, npair * P * J], F32)
                nc.sync.dma_start(
                    xt[:], x_ap[:, tt0 * P * J : (tt0 + npair) * P * J]
                )
                hi = hpool.tile([P, npair * P * J], BF16)
                nc.scalar.copy(hi[:], xt[:])
                lo = lpool.tile([P, npair * P * J], BF16)
                nc.vector.scalar_tensor_tensor(
                    lo[:], xt[:], 1.0, hi[:], op0=ALU.mult, op1=ALU.subtract
                )
                psh = [
                    psum_pool.tile([P, 512], F32, name="psh", tag=f"psh{k}")
                    for k in range(npair)
                ]

                # hi@[Whi|Wlo] (N=320) and lo@Whi (N=160) interleave in ONE
                # accumulation group per bank: psh[0:160] accumulates
                # hi@Whi + lo@Whi, psh[160:320] accumulates hi@Wlo. start
                # clears the whole bank on the first matmul only.
                for j in range(J):
                    for k in range(npair):
                        xsl = slice((k * J + j) * P, (k * J + j + 1) * P)
                        nc.tensor.matmul(
                            psh[k][:, 0:E2], hi[:, xsl],
                            wc_sb[:, j * E2 : (j + 1) * E2],
                            start=(j == 0), stop=False,
                        )
                        nc.tensor.matmul(
                            psh[k][:, 0:E], lo[:, xsl],
                            wc_sb[:, j * E2 : j * E2 + E],
                            start=False, stop=(j == J - 1),
                        )

                for k in range(npair):
                    tt = tt0 + k
                    hb = bigt.tile([P, E], F32)
                    nc.scalar.copy(hb[:], psh[k][:, E:E2])
                    lg = bigt.tile([P, E], F32)
                    nc.vector.tensor_add(lg[:], psh[k][:, 0:E], hb[:])
                    _emit_epilogue(tc, small, bigt, lg[:], oi_ap, ow_ap, tt)


def emit_gate_hilo3h(tc, x_ap, wc_sb, oi_ap, ow_ap):
    """hilo3g with the bf16 hi/lo split done host-side: x_ap is
    [P, n_pairs * 4*P*J] bf16 laid out per token-tile pair as
    [hi(tile0) hi(tile1) lo(tile0) lo(tile1)], so each pair is one
    5.24MB contiguous DMA and the ACT cast / DVE subtract disappear
    from the device entirely (same total DMA bytes as f32 x).
    """
    nc = tc.nc
    TJ4 = 4 * P * J
    n_pairs = x_ap.shape[1] // TJ4
    E2 = 2 * E

    with (
        tc.tile_pool(name="xpool", bufs=3) as xpool,
        tc.tile_pool(name="psum", bufs=2, space="PSUM") as psum_pool,
        tc.tile_pool(name="small", bufs=6) as small,
        tc.tile_pool(name="bigt", bufs=4) as bigt,
    ):
        for q in range(n_pairs):
            xc = xpool.tile([P, TJ4], BF16)
            nc.sync.dma_start(xc[:], x_ap[:, q * TJ4 : (q + 1) * TJ4])
            psh = [
                psum_pool.tile([P, 512], F32, name="psh", tag=f"psh{k}")
                for k in range(2)
            ]
            psl = [
                psum_pool.tile([P, 512], F32, name="psl", tag=f"psl{k}")
                for k in range(2)
            ]

            for j in range(J):
                for k in range(2):
                    hsl = slice((k * J + j) * P, (k * J + j + 1) * P)
                    lsl = slice(
                        (2 * J + k * J + j) * P, (2 * J + k * J + j + 1) * P
                    )
                    nc.tensor.matmul(
                        psh[k][:, 0:E2], xc[:, hsl],
                        wc_sb[:, j * E2 : (j + 1) * E2],
                        start=(j == 0), stop=(j == J - 1),
                    )
                    nc.tensor.matmul(
                        psl[k][:, 0:E], xc[:, lsl],
                        wc_sb[:, j * E2 : j * E2 + E],
                        start=(j == 0), stop=(j == J - 1),
                    )

            for k in range(2):
                tt = 2 * q + k
                hb = bigt.tile([P, E], F32)
                nc.scalar.copy(hb[:], psh[k][:, E:E2])
                ha = bigt.tile([P, E], F32)
                nc.vector.tensor_add(ha[:], psh[k][:, 0:E], hb[:])
                lg = bigt.tile([P, E], F32)
                nc.vector.tensor_add(lg[:], ha[:], psl[k][:, 0:E])
                _emit_epilogue(tc, small, bigt, lg[:], oi_ap, ow_ap, tt)


def emit_gate_f16(tc, x_ap, wc_sb, oi_ap, ow_ap, repeat=1, chunk=4):
    """fp16 host-split gate: x arrives as fp16 (half the DMA bytes of f32),
    W as [Whi_f16 | Wlo_f16 * 2^11] fused into one N=320 moving operand
    (Wlo is pre-scaled into fp16 normal range host-side; the epilogue's
    ACT copy un-scales it). One matmul per (tile, k-tile): 320 moving
    cols vs the bf16 3-term's 480 — and no on-chip hi/lo split at all.

    logits = xh @ Whi + 2^-11 * (xh @ (Wlo*2^11)); error ~2^-11 from the
    fp16 rounding of x only (W split exact to ~2^-22).

    x_ap: [P, T*J] f16 in the fast-DMA layout
    xp[p, ((tile*J)+j)*P + t] = fp16(x[tile*P + t, p*J + j]).
    `chunk` token-tiles are fetched per DMA (chunk=4: 5.24MB transfers)
    and processed as `chunk` interleaved PSUM accumulation chains.
    """
    nc = tc.nc
    T = x_ap.shape[1] // J
    n_tiles = T // P
    E2 = 2 * E

    with (
        tc.tile_pool(name="xpool", bufs=2) as xpool,
        tc.tile_pool(name="psum", bufs=2, space="PSUM") as psum_pool,
        tc.tile_pool(name="small", bufs=6) as small,
        tc.tile_pool(name="bigt", bufs=4) as bigt,
    ):
        for _rep in range(repeat):
            for tt0 in range(0, n_tiles, chunk):
                nch = min(chunk, n_tiles - tt0)
                xt = xpool.tile([P, nch * P * J], F16)
                nc.sync.dma_start(
                    xt[:], x_ap[:, tt0 * P * J : (tt0 + nch) * P * J]
                )
                psh = [
                    psum_pool.tile([P, 512], F32, name="psh", tag=f"psh{k}")
                    for k in range(nch)
                ]
                for j in range(J):
                    for k in range(nch):
                        xsl = slice((k * J + j) * P, (k * J + j + 1) * P)
                        nc.tensor.matmul(
                            psh[k][:, 0:E2], xt[:, xsl],
                            wc_sb[:, j * E2 : (j + 1) * E2],
                            start=(j == 0), stop=(j == J - 1),
                        )
                for k in range(nch):
                    tt = tt0 + k
                    # un-scale the Wlo half (2^-11) while folding: ACT copy
                    # with scale, then one DVE add (<=1 PSUM operand each)
                    hb = bigt.tile([P, E], F32)
                    nc.scalar.activation(
                        hb[:], psh[k][:, E:E2], ACTF.Copy, scale=2.0 ** -11
                    )
                    lg = bigt.tile([P, E], F32)
                    nc.vector.tensor_add(lg[:], psh[k][:, 0:E], hb[:])
                    _emit_epilogue(tc, small, bigt, lg[:], oi_ap, ow_ap, tt)


LO_FOLD = 2.0 ** -18  # lo stored as e3m4(lo*2^12), W8 as e3m4(W*2^6)


def emit_gate_f16l(tc, x_ap, lo_ap, wc_sb, w8_sb, oi_ap, ow_ap, repeat=1, chunk=4):
    """f16h plus an fp8e3m4 lo-correction term: x = fp16(x) + lo, with
    lo shipped as e3m4(lo * 2^12) (1 byte) and W for the lo term as
    e3m4(W * 2^6). x error drops from 2^-11 (f16h, fails the gate) to
    ~2^-16. The lo matmuls share the hi accumulation group, landing in
    cols 320:480 of the same PSUM bank; the epilogue folds all three
    column ranges with the 2^-18 un-scale."""
    nc = tc.nc
    T = x_ap.shape[1] // J
    n_tiles = T // P
    E2 = 2 * E
    E3 = 3 * E

    with (
        tc.tile_pool(name="xpool", bufs=2) as xpool,
        tc.tile_pool(name="lpool", bufs=2) as lpool,
        tc.tile_pool(name="psum", bufs=2, space="PSUM") as psum_pool,
        tc.tile_pool(name="small", bufs=6) as small,
        tc.tile_pool(name="bigt", bufs=4) as bigt,
    ):
        for _rep in range(repeat):
            for tt0 in range(0, n_tiles, chunk):
                nch = min(chunk, n_tiles - tt0)
                xt = xpool.tile([P, nch * P * J], F16)
                nc.sync.dma_start(
                    xt[:], x_ap[:, tt0 * P * J : (tt0 + nch) * P * J]
                )
                lt = lpool.tile([P, nch * P * J], F8E3)
                nc.scalar.dma_start(
                    lt[:], lo_ap[:, tt0 * P * J : (tt0 + nch) * P * J]
                )
                psh = [
                    psum_pool.tile([P, 512], F32, name="psh", tag=f"psh{k}")
                    for k in range(nch)
                ]
                for j in range(J):
                    for k in range(nch):
                        xsl = slice((k * J + j) * P, (k * J + j + 1) * P)
                        nc.tensor.matmul(
                            psh[k][:, 0:E2], xt[:, xsl],
                            wc_sb[:, j * E2 : (j + 1) * E2],
                            start=(j == 0), stop=False,
                        )
                        nc.tensor.matmul(
                            psh[k][:, E2:E3], lt[:, xsl],
                            w8_sb[:, j * E : (j + 1) * E],
                            start=False, stop=(j == J - 1),
                        )
                for k in range(nch):
                    tt = tt0 + k
                    hb = bigt.tile([P, E], F32)
                    nc.scalar.activation(
                        hb[:], psh[k][:, E:E2], ACTF.Copy, scale=2.0 ** -11
                    )
                    t1 = bigt.tile([P, E], F32)
                    nc.vector.scalar_tensor_tensor(
                        t1[:], psh[k][:, E2:E3], LO_FOLD, hb[:],
                        op0=ALU.mult, op1=ALU.add,
                    )
                    lg = bigt.tile([P, E], F32)
                    nc.vector.tensor_add(lg[:], psh[k][:, 0:E], t1[:])
                    _emit_epilogue(tc, small, bigt, lg[:], oi_ap, ow_ap, tt)


def emit_gate_f16x(tc, x_ap, lo_ap, wh_sb, wl8_sb, w8_sb, oi_ap, ow_ap,
                   repeat=1, chunk=4):
    """Scheme X: like f16l but the Wlo correction stream rides fp8e3m4 on
    the SAME xh stationary (mixed-dtype matmul), so the fp16 moving stream
    shrinks to N=160:
        psh[0:160]   += xh  @ Whi_f16            (fp16 moving)
        psh[160:320] += xh  @ e3m4(Wlo*2^17)     (fp8 moving, same lhsT)
        psh[320:480] += lo8 @ e3m4(W*2^6)        (fp8 moving)
    Wins if fp8 moving columns stream faster than fp16 ones."""
    nc = tc.nc
    T = x_ap.shape[1] // J
    n_tiles = T // P
    E2 = 2 * E
    E3 = 3 * E

    with (
        tc.tile_pool(name="xpool", bufs=2) as xpool,
        tc.tile_pool(name="lpool", bufs=2) as lpool,
        tc.tile_pool(name="psum", bufs=2, space="PSUM") as psum_pool,
        tc.tile_pool(name="small", bufs=6) as small,
        tc.tile_pool(name="bigt", bufs=4) as bigt,
    ):
        for _rep in range(repeat):
            for tt0 in range(0, n_tiles, chunk):
                nch = min(chunk, n_tiles - tt0)
                xt = xpool.tile([P, nch * P * J], F16)
                nc.sync.dma_start(
                    xt[:], x_ap[:, tt0 * P * J : (tt0 + nch) * P * J]
                )
                lt = lpool.tile([P, nch * P * J], F8E3)
                nc.scalar.dma_start(
                    lt[:], lo_ap[:, tt0 * P * J : (tt0 + nch) * P * J]
                )
                psh = [
                    psum_pool.tile([P, 512], F32, name="psh", tag=f"psh{k}")
                    for k in range(nch)
                ]
                for j in range(J):
                    for k in range(nch):
                        xsl = slice((k * J + j) * P, (k * J + j + 1) * P)
                        wsl = slice(j * E, (j + 1) * E)
                        nc.tensor.matmul(
                            psh[k][:, 0:E], xt[:, xsl], wh_sb[:, wsl],
                            start=(j == 0), stop=False,
                        )
                        nc.tensor.matmul(
                            psh[k][:, E:E2], xt[:, xsl], wl8_sb[:, wsl],
                            start=False, stop=False,
                        )
                        nc.tensor.matmul(
                            psh[k][:, E2:E3], lt[:, xsl], w8_sb[:, wsl],
                            start=False, stop=(j == J - 1),
                        )
                for k in range(nch):
                    tt = tt0 + k
                    hb = bigt.tile([P, E], F32)
                    nc.scalar.activation(
                        hb[:], psh[k][:, E:E2], ACTF.Copy, scale=2.0 ** -17
                    )
                    t1 = bigt.tile([P, E], F32)
                    nc.vector.scalar_tensor_tensor(
                        t1[:], psh[k][:, E2:E3], LO_FOLD, hb[:],
                        op0=ALU.mult, op1=ALU.add,
                    )
                    lg = bigt.tile([P, E], F32)
                    nc.vector.tensor_add(lg[:], psh[k][:, 0:E], t1[:])
                    _emit_epilogue(tc, small, bigt, lg[:], oi_ap, ow_ap, tt)


def emit_gate_f8w(tc, x_ap, lo_ap, wabc_sb, w8_sb, oi_ap, ow_ap,
                  repeat=1, chunk=2):
    """All-fp8 W-streams: per (tile, k-tile) just TWO matmuls —
        psh[0:480]  += xh  @ [A|B|C]   (one fp8e3m4 N=480 stream;
                                        A=e3m4(W*2^6), B=e3m4(r*2^11),
                                        C=e3m4(r'*2^16): 15 bits of W)
        psl[0:160]  += lo8 @ W8        (fp8 N=160)
    vs f16l's 800 moving bytes / 2 LDWs and f16x's 3 LDWs: 640 moving
    bytes, one fp16 LDW + one fp8 LDW. PSUM: psh needs a full bank, so
    psl lives in its own bank -> chunk=2 tiles to stay within 8 banks
    double-buffered."""
    nc = tc.nc
    T = x_ap.shape[1] // J
    n_tiles = T // P
    E3 = 3 * E

    with (
        tc.tile_pool(name="xpool", bufs=2) as xpool,
        tc.tile_pool(name="lpool", bufs=2) as lpool,
        tc.tile_pool(name="psum", bufs=2, space="PSUM") as psum_pool,
        tc.tile_pool(name="small", bufs=6) as small,
        tc.tile_pool(name="bigt", bufs=5) as bigt,
    ):
        for _rep in range(repeat):
            for tt0 in range(0, n_tiles, chunk):
                nch = min(chunk, n_tiles - tt0)
                xt = xpool.tile([P, nch * P * J], F16)
                nc.sync.dma_start(
                    xt[:], x_ap[:, tt0 * P * J : (tt0 + nch) * P * J]
                )
                lt = lpool.tile([P, nch * P * J], F8E3)
                nc.scalar.dma_start(
                    lt[:], lo_ap[:, tt0 * P * J : (tt0 + nch) * P * J]
                )
                psh = [
                    psum_pool.tile([P, 512], F32, name="psh", tag=f"psh{k}")
                    for k in range(nch)
                ]
                psl = [
                    psum_pool.tile([P, 512], F32, name="psl", tag=f"psl{k}")
                    for k in range(nch)
                ]
                for j in range(J):
                    for k in range(nch):
                        xsl = slice((k * J + j) * P, (k * J + j + 1) * P)
                        nc.tensor.matmul(
                            psh[k][:, 0:E3], xt[:, xsl],
                            wabc_sb[:, j * E3 : (j + 1) * E3],
                            start=(j == 0), stop=(j == J - 1),
                        )
                        nc.tensor.matmul(
                            psl[k][:, 0:E], lt[:, xsl],
                            w8_sb[:, j * E : (j + 1) * E],
                            start=(j == 0), stop=(j == J - 1),
                        )
                for k in range(nch):
                    tt = tt0 + k
                    s1 = bigt.tile([P, E], F32)
                    nc.scalar.activation(
                        s1[:], psh[k][:, 0:E], ACTF.Copy, scale=2.0 ** -6
                    )
                    s2 = bigt.tile([P, E], F32)
                    nc.vector.scalar_tensor_tensor(
                        s2[:], psh[k][:, E : 2 * E], 2.0 ** -11, s1[:],
                        op0=ALU.mult, op1=ALU.add,
                    )
                    s3 = bigt.tile([P, E], F32)
                    nc.vector.scalar_tensor_tensor(
                        s3[:], psh[k][:, 2 * E : E3], 2.0 ** -16, s2[:],
                        op0=ALU.mult, op1=ALU.add,
                    )
                    lg = bigt.tile([P, E], F32)
                    nc.vector.scalar_tensor_tensor(
                        lg[:], psl[k][:, 0:E], LO_FOLD, s3[:],
                        op0=ALU.mult, op1=ALU.add,
                    )
                    _emit_epilogue(tc, small, bigt, lg[:], oi_ap, ow_ap, tt)


def emit_probe_pe16(tc, x_ap, wc_sb, repeat=1):
    """Ablation probe: ONE fp16 quad loaded, then the full f16h matmul
    schedule re-reads the same tiles (no per-chunk DMA, no epilogue)."""
    nc = tc.nc
    T = x_ap.shape[1] // J
    n_tiles = T // P
    E2 = 2 * E
    with (
        tc.tile_pool(name="xpool", bufs=1) as xpool,
        tc.tile_pool(name="psum", bufs=2, space="PSUM") as psum_pool,
    ):
        xt = xpool.tile([P, 4 * P * J], F16)
        nc.sync.dma_start(xt[:], x_ap[:, 0 : 4 * P * J])
        for _rep in range(repeat):
            for _tt0 in range(0, n_tiles, 4):
                psh = [
                    psum_pool.tile([P, 512], F32, name="psh", tag=f"psh{k}")
                    for k in range(4)
                ]
                for j in range(J):
                    for k in range(4):
                        xsl = slice((k * J + j) * P, (k * J + j + 1) * P)
                        nc.tensor.matmul(
                            psh[k][:, 0:E2], xt[:, xsl],
                            wc_sb[:, j * E2 : (j + 1) * E2],
                            start=(j == 0), stop=(j == J - 1),
                        )


def emit_probe_pe_lo(tc, lo_ap, w8_sb, repeat=1):
    """Ablation probe: ONLY the fp8 N=160 lo matmuls of f16l."""
    nc = tc.nc
    T = lo_ap.shape[1] // J
    n_tiles = T // P
    E2 = 2 * E
    E3 = 3 * E
    with (
        tc.tile_pool(name="lxpool", bufs=1) as lxpool,
        tc.tile_pool(name="psum", bufs=2, space="PSUM") as psum_pool,
    ):
        lt = lxpool.tile([P, 4 * P * J], F8E3)
        nc.scalar.dma_start(lt[:], lo_ap[:, 0 : 4 * P * J])
        for _rep in range(repeat):
            for _tt0 in range(0, n_tiles, 4):
                psh = [
                    psum_pool.tile([P, 512], F32, name="psh", tag=f"psh{k}")
                    for k in range(4)
                ]
                for j in range(J):
                    for k in range(4):
                        xsl = slice((k * J + j) * P, (k * J + j + 1) * P)
                        nc.tensor.matmul(
                            psh[k][:, E2:E3], lt[:, xsl],
                            w8_sb[:, j * E : (j + 1) * E],
                            start=(j == 0), stop=(j == J - 1),
                        )


def emit_probe_dma(tc, x_ap, repeat=1, rings=1):
    """Ablation probe: ONLY the per-pair x DMAs of hilo3g (no consumers).
    The repeat-slope of this NEFF is the pure steady-state DMA rate.
    rings=2 splits each transfer across the sync and scalar HWDGE rings."""
    nc = tc.nc
    T = x_ap.shape[1] // J
    n_tiles = T // P
    with tc.tile_pool(name="xpool", bufs=2) as xpool:
        for _rep in range(repeat):
            for tt0 in range(0, n_tiles, 2):
                npair = 2 if tt0 + 1 < n_tiles else 1
                xt = xpool.tile([P, npair * P * J], F32)
                src = x_ap[:, tt0 * P * J : (tt0 + npair) * P * J]
                if rings == 2:
                    nc.sync.dma_start(xt[0:64, :], src[0:64, :])
                    nc.scalar.dma_start(xt[64:P, :], src[64:P, :])
                else:
                    nc.sync.dma_start(xt[:], src)


def emit_probe_pe_f16l(tc, x_ap, lo_ap, wc_sb, w8_sb, repeat=1):
    """Ablation probe: one quad loaded, then the full f16l matmul schedule
    (fp16 N=320 + fp8e3 N=160 per tile per k-tile) re-reads it."""
    nc = tc.nc
    T = x_ap.shape[1] // J
    n_tiles = T // P
    E2 = 2 * E
    E3 = 3 * E
    with (
        tc.tile_pool(name="xpool", bufs=1) as xpool,
        tc.tile_pool(name="psum", bufs=2, space="PSUM") as psum_pool,
    ):
        xt = xpool.tile([P, 4 * P * J], F16)
        nc.sync.dma_start(xt[:], x_ap[:, 0 : 4 * P * J])
        lt = xpool.tile([P, 4 * P * J], F8E3)
        nc.scalar.dma_start(lt[:], lo_ap[:, 0 : 4 * P * J])
        for _rep in range(repeat):
            for _tt0 in range(0, n_tiles, 4):
                psh = [
                    psum_pool.tile([P, 512], F32, name="psh", tag=f"psh{k}")
                    for k in range(4)
                ]
                for j in range(J):
                    for k in range(4):
                        xsl = slice((k * J + j) * P, (k * J + j + 1) * P)
                        nc.tensor.matmul(
                            psh[k][:, 0:E2], xt[:, xsl],
                            wc_sb[:, j * E2 : (j + 1) * E2],
                            start=(j == 0), stop=False,
                        )
                        nc.tensor.matmul(
                            psh[k][:, E2:E3], lt[:, xsl],
                            w8_sb[:, j * E : (j + 1) * E],
                            start=False, stop=(j == J - 1),
                        )


def emit_probe_pe(tc, x_ap, wc_sb, repeat=1):
    """Ablation probe: ONE pair loaded + split, then the full hilo3g
    matmul schedule re-reads the same hi/lo tiles (no per-pair DMA, no
    epilogue). The repeat-slope is the pure steady-state PE rate with
    identical instruction shapes to the real kernel."""
    nc = tc.nc
    T = x_ap.shape[1] // J
    n_tiles = T // P
    E2 = 2 * E
    with (
        tc.tile_pool(name="xpool", bufs=1) as xpool,
        tc.tile_pool(name="hpool", bufs=1) as hpool,
        tc.tile_pool(name="lpool", bufs=1) as lpool,
        tc.tile_pool(name="psum", bufs=3, space="PSUM") as psum_pool,
    ):
        xt = xpool.tile([P, 2 * P * J], F32)
        nc.sync.dma_start(xt[:], x_ap[:, 0 : 2 * P * J])
        hi = hpool.tile([P, 2 * P * J], BF16)
        nc.scalar.copy(hi[:], xt[:])
        lo = lpool.tile([P, 2 * P * J], BF16)
        nc.vector.scalar_tensor_tensor(
            lo[:], xt[:], 1.0, hi[:], op0=ALU.mult, op1=ALU.subtract
        )
        for _rep in range(repeat):
            for tt0 in range(0, n_tiles, 2):
                psh = [
                    psum_pool.tile([P, 512], F32, name="psh", tag=f"psh{k}")
                    for k in range(2)
                ]
                for j in range(J):
                    for k in range(2):
                        xsl = slice((k * J + j) * P, (k * J + j + 1) * P)
                        nc.tensor.matmul(
                            psh[k][:, 0:E2], hi[:, xsl],
                            wc_sb[:, j * E2 : (j + 1) * E2],
                            start=(j == 0), stop=False,
                        )
                        nc.tensor.matmul(
                            psh[k][:, 0:E], lo[:, xsl],
                            wc_sb[:, j * E2 : j * E2 + E],
                            start=False, stop=(j == J - 1),
                        )


def emit_gate_hilo(tc, x_ap, whi_ap, wlo_ap, oi_ap, ow_ap, terms=3):
    """Split-precision gate: x and W decomposed as bf16 hi + lo; logits =
    hi@Whi + hi@Wlo + lo@Whi (+ lo@Wlo with terms=4) accumulated in fp32
    PSUM (error ~2^-18). bf16 matmuls run ~4x faster than fp32 on the PE.
    W's split is precomputed on host; x's is done on-chip (ACT casts hi,
    DVE computes lo = x - hi)."""
    nc = tc.nc
    T = x_ap.shape[0]
    assert T % P == 0
    n_tiles = T // P

    with (
        tc.tile_pool(name="wpool", bufs=1) as wpool,
        tc.tile_pool(name="xpool", bufs=3) as xpool,
        tc.tile_pool(name="hpool", bufs=3) as hpool,
        tc.tile_pool(name="lpool", bufs=3) as lpool,
        tc.tile_pool(name="psum", bufs=4, space="PSUM") as psum_pool,
        tc.tile_pool(name="small", bufs=6) as small,
        tc.tile_pool(name="bigt", bufs=3) as bigt,
    ):
        whi_sb = wpool.tile([P, J * E], BF16)
        nc.sync.dma_start(whi_sb[:], whi_ap)
        wlo_sb = wpool.tile([P, J * E], BF16)
        nc.sync.dma_start(wlo_sb[:], wlo_ap)

        # process token-tiles in pairs: the two accumulation chains alternate
        # on the PE so each LDWEIGHTS can run in the background weight buffer
        # while the other chain's matmul streams
        for tt0 in range(0, n_tiles, 2):
            pair = [tt0, tt0 + 1] if tt0 + 1 < n_tiles else [tt0]
            his, los, pss = [], [], []
            for tt in pair:
                xt = xpool.tile([P, P * J], F32)
                src = x_ap[tt * P : (tt + 1) * P, :].rearrange(
                    "t (p j) -> p t j", p=P
                )
                dst = xt[:].rearrange("p (t j) -> p t j", j=J)
                # split the tile's 16K descriptors across both HWDGE rings
                # (two independent descriptor generators; measured ~15%
                # whole-kernel win over a single ring)
                half = P // 2
                nc.sync.dma_start(dst[:, :half, :], src[:, :half, :])
                nc.scalar.dma_start(dst[:, half:, :], src[:, half:, :])
                hi = hpool.tile([P, P * J], BF16)
                nc.scalar.copy(hi[:], xt[:])
                lo = lpool.tile([P, P * J], BF16)
                nc.vector.scalar_tensor_tensor(
                    lo[:], xt[:], 1.0, hi[:], op0=ALU.mult, op1=ALU.subtract
                )
                his.append(hi[:].rearrange("p (t j) -> p t j", j=J))
                los.append(lo[:].rearrange("p (t j) -> p t j", j=J))
                ps_k = psum_pool.tile([P, E], F32, name="ps", tag=f"ps{len(pss)}")
                pss.append(ps_k)

            for j in range(J):
                wsl = slice(j * E, (j + 1) * E)
                ops = [(his, whi_sb), (his, wlo_sb), (los, whi_sb)]
                if terms == 4:
                    ops.append((los, wlo_sb))
                for oi, (xs, wsb) in enumerate(ops):
                    last = j == J - 1 and oi == len(ops) - 1
                    for k in range(len(pair)):
                        nc.tensor.matmul(
                            pss[k][:], xs[k][:, :, j], wsb[:, wsl],
                            start=(j == 0 and oi == 0), stop=last,
                        )

            for k, tt in enumerate(pair):
                _emit_epilogue(tc, small, bigt, pss[k][:], oi_ap, ow_ap, tt)


def emit_gate_hilo_wide(tc, x_ap, wc_ap, oi_ap, ow_ap):
    """EXPERIMENTAL - DOES NOT COMPILE (walrus birverifier asserts on the
    N=320 matmul; root cause unidentified). Do not select mode "hilo4w".

    Like emit_gate_hilo(terms=4) but with Whi|Wlo concatenated into one
    N=320 moving operand, halving the matmul (and stationary-reload) count:
    two accumulation chains hi@[Whi|Wlo] and lo@[Whi|Wlo] into [128,320]
    PSUM tiles, folded into logits with three DVE adds."""
    nc = tc.nc
    T = x_ap.shape[0]
    assert T % P == 0
    n_tiles = T // P
    E2 = 2 * E

    with (
        tc.tile_pool(name="wpool", bufs=1) as wpool,
        tc.tile_pool(name="xpool", bufs=3) as xpool,
        tc.tile_pool(name="hpool", bufs=3) as hpool,
        tc.tile_pool(name="lpool", bufs=3) as lpool,
        tc.tile_pool(name="psum", bufs=3, space="PSUM") as psum_pool,
        tc.tile_pool(name="small", bufs=6) as small,
        tc.tile_pool(name="bigt", bufs=4) as bigt,
    ):
        wc_sb = wpool.tile([P, J * E2], BF16)
        nc.sync.dma_start(wc_sb[:], wc_ap)

        for tt in range(n_tiles):
            xt = xpool.tile([P, P * J], F32)
            src = x_ap[tt * P : (tt + 1) * P, :].rearrange("t (p j) -> p t j", p=P)
            nc.sync.dma_start(xt[:].rearrange("p (t j) -> p t j", j=J), src)
            hi = hpool.tile([P, P * J], BF16)
            nc.scalar.copy(hi[:], xt[:])
            lo = lpool.tile([P, P * J], BF16)
            nc.vector.scalar_tensor_tensor(
                lo[:], xt[:], 1.0, hi[:], op0=ALU.mult, op1=ALU.subtract
            )
            hi3 = hi[:].rearrange("p (t j) -> p t j", j=J)
            lo3 = lo[:].rearrange("p (t j) -> p t j", j=J)

            ps_h = psum_pool.tile([P, 512], F32, name="ps_h", tag="psh")[:, :E2]
            ps_l = psum_pool.tile([P, 512], F32, name="ps_l", tag="psl")[:, :E2]
            for src3, pst in ((hi3, ps_h), (lo3, ps_l)):
                for j in range(J):
                    wsl = slice(j * E2, (j + 1) * E2)
                    nc.tensor.matmul(
                        pst[:], src3[:, :, j], wc_sb[:, wsl],
                        start=(j == 0), stop=(j == J - 1),
                    )

            # logits = hi@Whi + hi@Wlo + lo@Whi + lo@Wlo
            ha = bigt.tile([P, E], F32)
            nc.vector.tensor_add(ha[:], ps_h[:, 0:E], ps_h[:, E:E2])
            la = bigt.tile([P, E], F32)
            nc.vector.tensor_add(la[:], ps_l[:, 0:E], ps_l[:, E:E2])
            lg = bigt.tile([P, E], F32)
            nc.vector.tensor_add(lg[:], ha[:], la[:])

            _emit_epilogue(tc, small, bigt, lg[:], oi_ap, ow_ap, tt)


def _emit_epilogue(tc, small, bigt, ps, oi_ap, ow_ap, tt):
    """ps: [P, E] AP of raw logits (PSUM or SBUF)."""
    nc = tc.nc
    ps3 = ps.rearrange("p (g i) -> p g i", i=EG)
    gmax = small.tile([P, G], F32)
    nc.vector.tensor_reduce(gmax[:], ps3, axis=AX.X, op=ALU.max)
    gsort = small.tile([P, 8], F32)
    nc.vector.max(gsort[:], gmax[:])
    gpen = small.tile([P, G], F32)
    nc.vector.tensor_scalar(
        gpen[:], gmax[:], gsort[:, TOPK_GROUP - 1 : TOPK_GROUP], NEG_BIG,
        op0=ALU.is_lt, op1=ALU.mult,
    )
    masked = bigt.tile([P, E], F32)
    nc.vector.scalar_tensor_tensor(
        masked[:].rearrange("p (g i) -> p g i", i=EG),
        ps3, 1.0,
        gpen[:, :, None].to_broadcast((P, G, EG)),
        op0=ALU.mult, op1=ALU.add,
    )
    v8 = small.tile([P, 8], F32)
    nc.vector.max(v8[:], masked[:])
    i8 = small.tile([P, 8], U32)
    nc.vector.max_index(i8[:], v8[:], masked[:])
    nrmax = small.tile([P, 1], F32)
    nc.vector.tensor_scalar_mul(nrmax[:], v8[:, 0:1], -1.0)
    exps = bigt.tile([P, E], F32)
    ssum = small.tile([P, 1], F32)
    nc.scalar.activation(
        exps[:], ps, ACTF.Exp, bias=nrmax[:], scale=1.0, accum_out=ssum[:]
    )
    rcp = small.tile([P, 1], F32)
    nc.vector.reciprocal(rcp[:], ssum[:])
    scl = small.tile([P, 1], F32)
    nc.vector.tensor_scalar_mul(scl[:], rcp[:], ROUTED_SCALING)
    e6 = small.tile([P, TOP_K], F32)
    nc.scalar.activation(e6[:], v8[:, 0:TOP_K], ACTF.Exp, bias=nrmax[:], scale=1.0)
    w6 = small.tile([P, TOP_K], F32)
    nc.vector.tensor_scalar_mul(w6[:], e6[:], scl[:])
    # outputs go out on the scalar HWDGE ring so the sync ring stays
    # dedicated to x prefetches
    nc.scalar.dma_start(oi_ap[tt * P : (tt + 1) * P, :], i8[:, 0:TOP_K])
    nc.scalar.dma_start(ow_ap[tt * P : (tt + 1) * P, :], w6[:])


def build_gate_kernel(T: int = T_CORE, repeat: int = 1, mode: str = "fp32"):
    nc = bacc.Bacc("TRN2", target_bir_lowering=False, debug=False, num_devices=N_CORES)
    oi_d = nc.dram_tensor("oi", [T, TOP_K], U32, kind="ExternalOutput")
    ow_d = nc.dram_tensor("ow", [T, TOP_K], F32, kind="ExternalOutput")
    if repeat == 0:
        # near-empty NEFF: same I/O signature, one tiny memset+store.
        # Used as a pure dispatch/RTT reference for timing.
        if mode in ("f32r",):
            nc.dram_tensor("x", [T, H], F32R, kind="ExternalInput")
            nc.dram_tensor("w", [P, J * E_PAD], F32R, kind="ExternalInput")
        elif mode == "hilo3h":
            nc.dram_tensor("x", [P, 2 * T * J], BF16, kind="ExternalInput")
            nc.dram_tensor("wc", [P, J * 2 * E], BF16, kind="ExternalInput")
        elif mode in ("hilo3w", "hilo3g"):
            nc.dram_tensor("x", [P, T * J], F32, kind="ExternalInput")
            nc.dram_tensor("wc", [P, J * 2 * E], BF16, kind="ExternalInput")
        else:
            nc.dram_tensor("x", [P, T * J], F32, kind="ExternalInput")
            nc.dram_tensor("whi", [P, J * E], BF16, kind="ExternalInput")
            nc.dram_tensor("wlo", [P, J * E], BF16, kind="ExternalInput")
        with TileContext(nc) as tc:
            with tc.tile_pool(name="zpool", bufs=1) as zp:
                z = zp.tile([P, TOP_K], U32)
                tc.nc.vector.memset(z[:], 0)
                tc.nc.sync.dma_start(oi_d.ap()[0:P, :], z[:])
                zw = zp.tile([P, TOP_K], F32)
                tc.nc.vector.memset(zw[:], 0)
                tc.nc.sync.dma_start(ow_d.ap()[0:P, :], zw[:])
        nc.compile()
        return nc
    if mode == "hilo4w":
        x_d = nc.dram_tensor("x", [T, H], F32, kind="ExternalInput")
        wc_d = nc.dram_tensor("wc", [P, J * 2 * E], BF16, kind="ExternalInput")
        with TileContext(nc) as tc:
            for _ in range(repeat):
                emit_gate_hilo_wide(tc, x_d.ap(), wc_d.ap(), oi_d.ap(), ow_d.ap())
    elif mode == "f32r":
        x_d = nc.dram_tensor("x", [P, T * J], F32R, kind="ExternalInput")
        w_d = nc.dram_tensor("w", [P, J * E_PAD], F32R, kind="ExternalInput")
        with TileContext(nc) as tc:
            for _ in range(repeat):
                emit_gate_f32r(tc, x_d.ap(), w_d.ap(), oi_d.ap(), ow_d.ap())
    elif mode == "hilo3f":
        x_d = nc.dram_tensor("x", [P, T * J], F32, kind="ExternalInput")
        whi_d = nc.dram_tensor("whi", [P, J * E], BF16, kind="ExternalInput")
        wlo_d = nc.dram_tensor("wlo", [P, J * E], BF16, kind="ExternalInput")
        with TileContext(nc) as tc:
            for _ in range(repeat):
                emit_gate_hilo3f(
                    tc, x_d.ap(), whi_d.ap(), wlo_d.ap(), oi_d.ap(), ow_d.ap()
                )
    elif mode == "hilo3w":
        x_d = nc.dram_tensor("x", [P, T * J], F32, kind="ExternalInput")
        wc_d = nc.dram_tensor("wc", [P, J * 2 * E], BF16, kind="ExternalInput")
        with TileContext(nc) as tc:
            for _ in range(repeat):
                emit_gate_hilo3w(
                    tc, x_d.ap(), wc_d.ap(), oi_d.ap(), ow_d.ap()
                )
    elif mode == "hilo3g":
        x_d = nc.dram_tensor("x", [P, T * J], F32, kind="ExternalInput")
        wc_d = nc.dram_tensor("wc", [P, J * 2 * E], BF16, kind="ExternalInput")
        with TileContext(nc) as tc:
            with tc.tile_pool(name="wpool", bufs=1) as wpool:
                wc_sb = wpool.tile([P, J * 2 * E], BF16)
                tc.nc.sync.dma_start(wc_sb[:], wc_d.ap())
                emit_gate_hilo3g(
                    tc, x_d.ap(), wc_sb, oi_d.ap(), ow_d.ap(), repeat=repeat
                )
    elif mode == "hilo3h":
        x_d = nc.dram_tensor("x", [P, 2 * T * J], BF16, kind="ExternalInput")
        wc_d = nc.dram_tensor("wc", [P, J * 2 * E], BF16, kind="ExternalInput")
        with TileContext(nc) as tc:
            with tc.tile_pool(name="wpool", bufs=1) as wpool:
                wc_sb = wpool.tile([P, J * 2 * E], BF16)
                tc.nc.sync.dma_start(wc_sb[:], wc_d.ap())
                for _ in range(repeat):
                    emit_gate_hilo3h(
                        tc, x_d.ap(), wc_sb, oi_d.ap(), ow_d.ap()
                    )
    elif mode == "f16h":
        x_d = nc.dram_tensor("x", [P, T * J], F16, kind="ExternalInput")
        wc_d = nc.dram_tensor("wc", [P, J * 2 * E], F16, kind="ExternalInput")
        with TileContext(nc) as tc:
            with tc.tile_pool(name="wpool", bufs=1) as wpool:
                wc_sb = wpool.tile([P, J * 2 * E], F16)
                tc.nc.sync.dma_start(wc_sb[:], wc_d.ap())
                emit_gate_f16(
                    tc, x_d.ap(), wc_sb, oi_d.ap(), ow_d.ap(), repeat=repeat
                )
    elif mode == "f16l":
        x_d = nc.dram_tensor("x", [P, T * J], F16, kind="ExternalInput")
        lo_d = nc.dram_tensor("xlo", [P, T * J], F8E3, kind="ExternalInput")
        wc_d = nc.dram_tensor("wc", [P, J * 2 * E], F16, kind="ExternalInput")
        w8_d = nc.dram_tensor("w8", [P, J * E], F8E3, kind="ExternalInput")
        with TileContext(nc) as tc:
            with tc.tile_pool(name="wpool", bufs=1) as wpool:
                wc_sb = wpool.tile([P, J * 2 * E], F16)
                tc.nc.sync.dma_start(wc_sb[:], wc_d.ap())
                w8_sb = wpool.tile([P, J * E], F8E3)
                tc.nc.sync.dma_start(w8_sb[:], w8_d.ap())
                emit_gate_f16l(
                    tc, x_d.ap(), lo_d.ap(), wc_sb, w8_sb,
                    oi_d.ap(), ow_d.ap(), repeat=repeat,
                )
    elif mode == "probe_pe16":
        x_d = nc.dram_tensor("x", [P, T * J], F16, kind="ExternalInput")
        wc_d = nc.dram_tensor("wc", [P, J * 2 * E], F16, kind="ExternalInput")
        with TileContext(nc) as tc:
            with tc.tile_pool(name="wpool", bufs=1) as wpool:
                wc_sb = wpool.tile([P, J * 2 * E], F16)
                tc.nc.sync.dma_start(wc_sb[:], wc_d.ap())
                z = wpool.tile([P, TOP_K], U32)
                tc.nc.vector.memset(z[:], 0)
                tc.nc.sync.dma_start(oi_d.ap()[0:P, :], z[:])
                zw = wpool.tile([P, TOP_K], F32)
                tc.nc.vector.memset(zw[:], 0)
                tc.nc.sync.dma_start(ow_d.ap()[0:P, :], zw[:])
                emit_probe_pe16(tc, x_d.ap(), wc_sb, repeat=repeat)
    elif mode == "f8w":
        x_d = nc.dram_tensor("x", [P, T * J], F16, kind="ExternalInput")
        lo_d = nc.dram_tensor("xlo", [P, T * J], F8E3, kind="ExternalInput")
        wabc_d = nc.dram_tensor("wabc", [P, J * 3 * E], F8E3, kind="ExternalInput")
        w8_d = nc.dram_tensor("w8", [P, J * E], F8E3, kind="ExternalInput")
        with TileContext(nc) as tc:
            with tc.tile_pool(name="wpool", bufs=1) as wpool:
                wabc_sb = wpool.tile([P, J * 3 * E], F8E3)
                tc.nc.sync.dma_start(wabc_sb[:], wabc_d.ap())
                w8_sb = wpool.tile([P, J * E], F8E3)
                tc.nc.sync.dma_start(w8_sb[:], w8_d.ap())
                emit_gate_f8w(
                    tc, x_d.ap(), lo_d.ap(), wabc_sb, w8_sb,
                    oi_d.ap(), ow_d.ap(), repeat=repeat,
                )
    elif mode == "f16x":
        x_d = nc.dram_tensor("x", [P, T * J], F16, kind="ExternalInput")
        lo_d = nc.dram_tensor("xlo", [P, T * J], F8E3, kind="ExternalInput")
        wh_d = nc.dram_tensor("wh", [P, J * E], F16, kind="ExternalInput")
        wl8_d = nc.dram_tensor("wl8", [P, J * E], F8E3, kind="ExternalInput")
        w8_d = nc.dram_tensor("w8", [P, J * E], F8E3, kind="ExternalInput")
        with TileContext(nc) as tc:
            with tc.tile_pool(name="wpool", bufs=1) as wpool:
                wh_sb = wpool.tile([P, J * E], F16)
                tc.nc.sync.dma_start(wh_sb[:], wh_d.ap())
                wl8_sb = wpool.tile([P, J * E], F8E3)
                tc.nc.sync.dma_start(wl8_sb[:], wl8_d.ap())
                w8_sb = wpool.tile([P, J * E], F8E3)
                tc.nc.sync.dma_start(w8_sb[:], w8_d.ap())
                emit_gate_f16x(
                    tc, x_d.ap(), lo_d.ap(), wh_sb, wl8_sb, w8_sb,
                    oi_d.ap(), ow_d.ap(), repeat=repeat,
                )
    elif mode == "probe_pe_lo":
        lo_d = nc.dram_tensor("xlo", [P, T * J], F8E3, kind="ExternalInput")
        w8_d = nc.dram_tensor("w8", [P, J * E], F8E3, kind="ExternalInput")
        with TileContext(nc) as tc:
            with tc.tile_pool(name="wpool", bufs=1) as wpool:
                w8_sb = wpool.tile([P, J * E], F8E3)
                tc.nc.sync.dma_start(w8_sb[:], w8_d.ap())
                z = wpool.tile([P, TOP_K], U32)
                tc.nc.vector.memset(z[:], 0)
                tc.nc.sync.dma_start(oi_d.ap()[0:P, :], z[:])
                zw = wpool.tile([P, TOP_K], F32)
                tc.nc.vector.memset(zw[:], 0)
                tc.nc.sync.dma_start(ow_d.ap()[0:P, :], zw[:])
                emit_probe_pe_lo(tc, lo_d.ap(), w8_sb, repeat=repeat)
    elif mode == "probe_pe_f16l":
        x_d = nc.dram_tensor("x", [P, T * J], F16, kind="ExternalInput")
        lo_d = nc.dram_tensor("xlo", [P, T * J], F8E3, kind="ExternalInput")
        wc_d = nc.dram_tensor("wc", [P, J * 2 * E], F16, kind="ExternalInput")
        w8_d = nc.dram_tensor("w8", [P, J * E], F8E3, kind="ExternalInput")
        with TileContext(nc) as tc:
            with tc.tile_pool(name="wpool", bufs=1) as wpool:
                wc_sb = wpool.tile([P, J * 2 * E], F16)
                tc.nc.sync.dma_start(wc_sb[:], wc_d.ap())
                w8_sb = wpool.tile([P, J * E], F8E3)
                tc.nc.sync.dma_start(w8_sb[:], w8_d.ap())
                z = wpool.tile([P, TOP_K], U32)
                tc.nc.vector.memset(z[:], 0)
                tc.nc.sync.dma_start(oi_d.ap()[0:P, :], z[:])
                zw = wpool.tile([P, TOP_K], F32)
                tc.nc.vector.memset(zw[:], 0)
                tc.nc.sync.dma_start(ow_d.ap()[0:P, :], zw[:])
                emit_probe_pe_f16l(
                    tc, x_d.ap(), lo_d.ap(), wc_sb, w8_sb, repeat=repeat
                )
    elif mode in ("probe_dma", "probe_dma2", "probe_pe"):
        x_d = nc.dram_tensor("x", [P, T * J], F32, kind="ExternalInput")
        wc_d = nc.dram_tensor("wc", [P, J * 2 * E], BF16, kind="ExternalInput")
        with TileContext(nc) as tc:
            with tc.tile_pool(name="wpool", bufs=1) as wpool:
                wc_sb = wpool.tile([P, J * 2 * E], BF16)
                tc.nc.sync.dma_start(wc_sb[:], wc_d.ap())
                z = wpool.tile([P, TOP_K], U32)
                tc.nc.vector.memset(z[:], 0)
                tc.nc.sync.dma_start(oi_d.ap()[0:P, :], z[:])
                zw = wpool.tile([P, TOP_K], F32)
                tc.nc.vector.memset(zw[:], 0)
                tc.nc.sync.dma_start(ow_d.ap()[0:P, :], zw[:])
                if mode == "probe_dma":
                    emit_probe_dma(tc, x_d.ap(), repeat=repeat)
                elif mode == "probe_dma2":
                    emit_probe_dma(tc, x_d.ap(), repeat=repeat, rings=2)
                else:
                    emit_probe_pe(tc, x_d.ap(), wc_sb, repeat=repeat)
    elif mode in ("hilo", "hilo4"):
        x_d = nc.dram_tensor("x", [T, H], F32, kind="ExternalInput")
        whi_d = nc.dram_tensor("whi", [P, J * E], BF16, kind="ExternalInput")
        wlo_d = nc.dram_tensor("wlo", [P, J * E], BF16, kind="ExternalInput")
        with TileContext(nc) as tc:
            for _ in range(repeat):
                emit_gate_hilo(
                    tc, x_d.ap(), whi_d.ap(), wlo_d.ap(), oi_d.ap(), ow_d.ap(),
                    terms=4 if mode == "hilo4" else 3,
                )
    else:
        x_d = nc.dram_tensor("x", [T, H], F32, kind="ExternalInput")
        w_d = nc.dram_tensor("w", [P, J * E], F32, kind="ExternalInput")
        with TileContext(nc) as tc:
            for _ in range(repeat):
                emit_gate(tc, x_d.ap(), w_d.ap(), oi_d.ap(), ow_d.ap())
    nc.compile()
    return nc


def prep_weight(weight: np.ndarray) -> np.ndarray:
    """[160, 5120] -> [128, 40*160] with w[p, j*E + e] = W[e, p*40 + j]."""
    wt = np.asarray(weight, dtype=np.float32).T  # [H, E]
    return np.ascontiguousarray(wt.reshape(P, J, E)).reshape(P, J * E)


def prep_weight_f32r(weight: np.ndarray) -> np.ndarray:
    """[160, 5120] -> [128, 40*256], w[p, j*E_PAD + e] = W[e, p*40 + j]
    (zero for e >= 160)."""
    wt = np.asarray(weight, dtype=np.float32).T  # [H, E]
    wp = np.zeros((H, E_PAD), np.float32)
    wp[:, :E] = wt
    return np.ascontiguousarray(wp.reshape(P, J, E_PAD)).reshape(P, J * E_PAD)


def prep_weight_f16(weight: np.ndarray) -> np.ndarray:
    """[160, 5120] -> [P, J*2E] fp16: per j-block [Whi | Wlo * 2^11].

    Whi is fp16(W) with denormals flushed to zero host-side (so a PE that
    flushes fp16 denormals sees exactly the value Wlo was computed
    against); Wlo is scaled by 2^11 into fp16 normal range and un-scaled
    in the kernel epilogue. W split error ~2^-22."""
    w = np.asarray(weight, dtype=np.float32)
    whi = w.astype(np.float16)
    whi_f = np.where(np.abs(whi.astype(np.float32)) < 6.104e-5, 0.0, whi.astype(np.float32))
    whi = whi_f.astype(np.float16)
    wlo = ((w - whi.astype(np.float32)) * 2048.0).astype(np.float16)

    def perm(a):
        return np.ascontiguousarray(a.astype(np.float16).T.reshape(P, J, E))

    return np.ascontiguousarray(
        np.concatenate([perm(whi), perm(wlo)], axis=2)
    ).reshape(P, J * 2 * E)


def prep_weight_f8abc(weight: np.ndarray) -> np.ndarray:
    """[160, 5120] -> [P, J*3E] fp8e3m4: per j-block [A | B | C] with
    A = e3m4(W*2^6), B = e3m4((W - A/2^6)*2^11), C = e3m4(residual*2^16).
    Three 5-bit terms -> ~15 bits of W."""
    import ml_dtypes

    w = np.asarray(weight, dtype=np.float32)
    A = (w * 64.0).astype(ml_dtypes.float8_e3m4)
    rB = w - A.astype(np.float32) / 64.0
    B = (rB * 2.0 ** 11).astype(ml_dtypes.float8_e3m4)
    rC = rB - B.astype(np.float32) * 2.0 ** -11
    C = (rC * 2.0 ** 16).astype(ml_dtypes.float8_e3m4)

    def perm(a):
        return a.T.reshape(P, J, E)

    return np.ascontiguousarray(
        np.concatenate([perm(A), perm(B), perm(C)], axis=2)
    ).reshape(P, J * 3 * E)


def prep_weight_f8(weight: np.ndarray) -> np.ndarray:
    """[160, 5120] -> [P, J*E] fp8e3m4 of W * 2^6, for the lo term."""
    import ml_dtypes

    w = np.asarray(weight, dtype=np.float32) * 64.0
    w8 = w.astype(ml_dtypes.float8_e3m4)
    return np.ascontiguousarray(w8.T.reshape(P, J, E)).reshape(P, J * E)


def prep_weight_hilo(weight: np.ndarray):
    import ml_dtypes

    w = np.asarray(weight, dtype=np.float32)
    whi = w.astype(ml_dtypes.bfloat16)
    wlo = (w - whi.astype(np.float32)).astype(ml_dtypes.bfloat16)

    def perm(a):
        return np.ascontiguousarray(a.T.reshape(P, J, E)).reshape(P, J * E)

    return perm(whi), perm(wlo)


_NC_CACHE = {}


# "hilo3g" = 3-term bf16 split matmul on the fast-DMA [p, tile, j, t]
# layout (line-rate 20KB-contiguous x loads, contiguous per-j stationary
# slices), with Whi|Wlo fused into one N=320 moving operand (2 matmuls per
# k-tile), the weight tile resident across repeats, and one 5.24MB DMA per
# token-tile pair. Measured 94.1us vs hilo4's 278.6us baseline; 6/98304
# near-tie index swaps, rel err 4.8e-3 (gate is 2e-2). "hilo4" kept as the
# old fallback; "f32r" is faster on paper but its ~11-bit operand
# truncation puts rel err at 1.99e-2 — disqualified.
MODE = "f16l"


def make_in_maps(hidden_states, weight, mode=None):
    mode = mode or MODE
    hs = np.ascontiguousarray(
        np.asarray(hidden_states, dtype=np.float32).reshape(T_TOTAL, H)
    )
    shards = hs.reshape(N_CORES, T_CORE, H)
    if mode in ("f16h", "f16l", "f16x", "f8w", "probe_pe16"):
        # fast-DMA layout, fp16: xp[p, ((tile*J)+j)*P + t] = x[tile*P+t, p*J+j]
        n_tiles = T_CORE // P
        xs = hs.reshape(N_CORES, n_tiles, P, P, J)  # [c, tile, t, p, j]
        wc = prep_weight_f16(weight)
        maps = []
        for c in range(N_CORES):
            xc = np.ascontiguousarray(xs[c].transpose(2, 0, 3, 1)).reshape(
                P, T_CORE * J
            )
            xh = xc.astype(np.float16)
            # flush fp16 denormals host-side so a PE that FTZs sees the
            # exact value the lo residual was computed against
            xh = np.where(
                np.abs(xh.astype(np.float32)) < 6.104e-5, 0, xh
            ).astype(np.float16)
            if mode == "f8w":
                import ml_dtypes

                lo = (xc - xh.astype(np.float32)) * 4096.0  # 2^12
                m = {
                    "x": xh,
                    "xlo": lo.astype(ml_dtypes.float8_e3m4),
                    "wabc": prep_weight_f8abc(weight),
                    "w8": prep_weight_f8(weight),
                }
            elif mode == "f16x":
                import ml_dtypes

                lo = (xc - xh.astype(np.float32)) * 4096.0  # 2^12
                m = {"x": xh, "xlo": lo.astype(ml_dtypes.float8_e3m4)}
                w = np.asarray(weight, dtype=np.float32)
                whi = w.astype(np.float16)
                whi = np.where(
                    np.abs(whi.astype(np.float32)) < 6.104e-5, 0, whi
                ).astype(np.float16)
                wl8 = ((w - whi.astype(np.float32)) * 2.0 ** 17).astype(
                    ml_dtypes.float8_e3m4
                )

                def perm(a):
                    return np.ascontiguousarray(a.T.reshape(P, J, E)).reshape(
                        P, J * E
                    )

                m["wh"] = perm(whi)
                m["wl8"] = perm(wl8)
                m["w8"] = prep_weight_f8(weight)
            else:
                m = {"x": xh, "wc": wc}
                if mode == "f16l":
                    import ml_dtypes

                    lo = (xc - xh.astype(np.float32)) * 4096.0  # 2^12
                    m["xlo"] = lo.astype(ml_dtypes.float8_e3m4)
                    m["w8"] = prep_weight_f8(weight)
            maps.append(m)
        return maps
    if mode in ("probe_pe_f16l", "probe_pe_lo"):
        maps = make_in_maps(hidden_states, weight, "f16l")
        if mode == "probe_pe_lo":
            maps = [{"xlo": m["xlo"], "w8": m["w8"]} for m in maps]
        return maps
    if mode in ("f32r", "hilo3f", "hilo3w", "hilo3g", "hilo3h", "probe_dma", "probe_dma2", "probe_pe"):
        # x[tile*P + t, p*J + j] -> xp[p, ((tile*J)+j)*P + t]: every
        # token-tile DMA is one contiguous 20KB run per partition, and each
        # k-tile's stationary slice is contiguous in SBUF.
        n_tiles = T_CORE // P
        xs = shards.reshape(N_CORES, n_tiles, P, P, J)  # [c, tile, t, p, j]
        xps = [
            np.ascontiguousarray(xs[c].transpose(2, 0, 3, 1)).reshape(
                P, T_CORE * J
            )
            for c in range(N_CORES)
        ]
        if mode == "f32r":
            wf = prep_weight_f32r(weight)
            return [{"x": xps[c], "w": wf} for c in range(N_CORES)]
        whi, wlo = prep_weight_hilo(weight)
        if mode == "hilo3h":
            import ml_dtypes

            wc = np.ascontiguousarray(
                np.concatenate(
                    [whi.reshape(P, J, E), wlo.reshape(P, J, E)], axis=2
                ).reshape(P, J * 2 * E)
            )
            n_pairs = T_CORE // P // 2
            maps = []
            for c in range(N_CORES):
                hi = xps[c].astype(ml_dtypes.bfloat16)
                lo = (xps[c] - hi.astype(np.float32)).astype(ml_dtypes.bfloat16)
                h3 = hi.reshape(P, n_pairs, 2 * J * P)
                l3 = lo.reshape(P, n_pairs, 2 * J * P)
                xc = np.concatenate(
                    [h3[:, :, None, :], l3[:, :, None, :]], axis=2
                ).reshape(P, 2 * T_CORE * J)
                maps.append({"x": np.ascontiguousarray(xc), "wc": wc})
            return maps
        if mode in ("hilo3w", "hilo3g", "probe_dma", "probe_dma2", "probe_pe"):
            wc = np.concatenate(
                [whi.reshape(P, J, E), wlo.reshape(P, J, E)], axis=2
            ).reshape(P, J * 2 * E)
            return [
                {"x": xps[c], "wc": np.ascontiguousarray(wc)}
                for c in range(N_CORES)
            ]
        return [
            {"x": xps[c], "whi": whi, "wlo": wlo} for c in range(N_CORES)
        ]
    if mode == "hilo4w":
        whi, wlo = prep_weight_hilo(weight)
        wc = np.concatenate(
            [whi.reshape(P, J, E), wlo.reshape(P, J, E)], axis=2
        ).reshape(P, J * 2 * E)
        wc = np.ascontiguousarray(wc)
        return [{"x": shards[c], "wc": wc} for c in range(N_CORES)]
    if mode in ("hilo", "hilo4"):
        whi, wlo = prep_weight_hilo(weight)
        return [
            {"x": shards[c], "whi": whi, "wlo": wlo} for c in range(N_CORES)
        ]
    wr = prep_weight(weight)
    return [{"x": shards[c], "w": wr} for c in range(N_CORES)]


def run(hidden_states, weight, trace=False, mode=None):
    mode = mode or MODE
    in_maps = make_in_maps(hidden_states, weight, mode)
    if mode not in _NC_CACHE:
        _NC_CACHE[mode] = build_gate_kernel(mode=mode)
    nc = _NC_CACHE[mode]
    res = bass_utils.run_bass_kernel_spmd(
        nc, in_maps, core_ids=list(range(N_CORES)), trace=trace
    )
    idx = np.concatenate([r["oi"].astype(np.int32) for r in res.results], axis=0)
    wts = np.concatenate([r["ow"] for r in res.results], axis=0)
    return (idx, wts), res


def kernel(hidden_states, weight):
    (idx, wts), _ = run(hidden_states, weight)
    return idx, wts



# revision 37
# speedup vs baseline: 1.8740x; 1.2089x over previous
"""DeepSeek-V2 MoE gate (group-limited greedy top-k routing) on 8 trn2 NeuronCores.

Reference computation (per token t over E=160 experts in G=8 groups of 20):
    logits = x @ W^T                       [T, E]
    scores = softmax(logits)
    group_scores[g] = max over group g of scores
    keep top-3 groups; mask scores of other groups to 0
    topk_weight, topk_idx = top_k(masked scores, 6); topk_weight *= 16.0

Sharding: tokens (B*S = 16384) split evenly across the 8 cores; the small
[160, 5120] gate weight is replicated (pre-arranged host-side).

The shipped modes ("f16l" / "f16x") replace the old on-chip bf16 3-term
split ("hilo3g", graded 60.7us) with a host-prequantized fp16+fp8 form
chosen to minimize the device's two real bottlenecks at once — moving-
operand bytes on the tensor engine and DMA bytes:

    x = xh(fp16) + 2^-12 * lo8(fp8e3m4)           [host split, 3B/elem]
    f16l: logits = xh@[Whi|Wlo*2^11](fp16, N=320) + lo8@W8(fp8) * 2^-18
    f16x: logits = xh@Whi(fp16, N=160) + xh@Wl8(fp8) * 2^-17
                   + lo8@W8(fp8) * 2^-18

- Measured on HW: moving-operand streaming is byte-rate limited
  (~32b/lane/cycle: bf16/fp16 ~2 cols/cyc, fp8e3m4 N=160 ~4 cols/cyc =
  16.7ns/MM), so the fp8 correction streams cost 1/2-1/4 of the old
  bf16 columns. Mixed-dtype matmul (fp16 stationary x fp8 moving) works
  and is exact. A fused all-fp8 N=480 W stream ("f8w") is 1.4x SLOWER -
  wide fp8 moving operands fall off the fast path. An int16 fixed-point
  variant is rejected by the walrus BIR verifier (float dtypes only).
- Precision: x error ~2^-16 -> f16l 0/98304 flipped picks (rel 6e-8),
  f16x 4 flips (rel 4.6e-3) vs the 2e-2 gate. fp16-x alone (124 flips,
  2.29e-2) fails; numpy simulation of the quantization reproduces HW
  flip counts exactly, so schemes were screened host-side.
- Both fp16 halves of W ride with denormals flushed/pre-scaled into
  normal range (fp16 min normal 6.1e-5; Wlo is ~2^-11*W) so PE FTZ
  behavior can't bite; all fp8 tensors are pre-scaled likewise.
- DMA drops 41.9MB -> 31.4MB/core, split over both HWDGE rings (xh on
  sync, lo8 on scalar), well under the matmul time.
- The ACT cast (~68us busy) and DVE subtract (~43us) of the on-chip
  split are gone entirely; ACT/DVE only run the epilogue now.

Selection runs on raw logits (softmax is monotonic; the top-3-group test by
max-score equals the test by max-logit), so only the final 6 weights and the
softmax denominator need exp().
"""

import numpy as np

import concourse.bacc as bacc
import concourse.mybir as mybir
from concourse import bass_utils
from concourse.tile import TileContext

# Problem constants (hardcoded per the harness contract).
B, S, H = 4, 4096, 5120
E = 160                 # experts
G = 8                   # groups
EG = E // G             # experts per group (20)
TOP_K = 6
TOPK_GROUP = 3
ROUTED_SCALING = 16.0
N_CORES = 8
T_TOTAL = B * S         # 16384
T_CORE = T_TOTAL // N_CORES  # 2048
P = 128                 # SBUF partitions
J = H // P              # hidden values per partition (40) = number of k-tiles
NEG_BIG = -1.0e30

F32 = mybir.dt.float32
F32R = mybir.dt.float32r  # fp32 the PE streams at bf16 rate (moving dim
                          # >=256) but with ~11-bit operand truncation
BF16 = mybir.dt.bfloat16
F16 = mybir.dt.float16
F8E3 = mybir.dt.float8e3  # e3m4: 5 mantissa bits, range [2^-6, 15.5]
U32 = mybir.dt.uint32
ALU = mybir.AluOpType
ACTF = mybir.ActivationFunctionType
AX = mybir.AxisListType


def emit_gate(tc, x_ap, w_ap, oi_ap, ow_ap):
    """Emit the gate kernel body into TileContext `tc`.

    x_ap:  [T, H] f32 DRAM (T % 128 == 0)
    w_ap:  [P, J*E] f32 DRAM (pre-permuted weight, see module docstring)
    oi_ap: [T, TOP_K] u32 DRAM out (expert indices)
    ow_ap: [T, TOP_K] f32 DRAM out (routing weights)
    """
    nc = tc.nc
    T = x_ap.shape[0]
    assert T % P == 0
    n_tiles = T // P

    with (
        tc.tile_pool(name="wpool", bufs=1) as wpool,
        tc.tile_pool(name="xpool", bufs=3) as xpool,
        tc.tile_pool(name="psum", bufs=4, space="PSUM") as psum_pool,
        tc.tile_pool(name="small", bufs=6) as small,
        tc.tile_pool(name="bigt", bufs=3) as bigt,
    ):
        w_sb = wpool.tile([P, J * E], F32)
        nc.sync.dma_start(w_sb[:], w_ap)

        for tt in range(n_tiles):
            # x tile: [p, t*J + j] = x[t0 + t, p*J + j]
            xt = xpool.tile([P, P * J], F32)
            src = x_ap[tt * P : (tt + 1) * P, :].rearrange("t (p j) -> p t j", p=P)
            nc.sync.dma_start(xt[:].rearrange("p (t j) -> p t j", j=J), src)
            xt3 = xt[:].rearrange("p (t j) -> p t j", j=J)

            # logits[t, e] accumulated over the 40 k-tiles
            ps = psum_pool.tile([P, E], F32)
            for j in range(J):
                nc.tensor.matmul(
                    ps[:],
                    xt3[:, :, j],                  # stationary [128h, 128t]
                    w_sb[:, j * E : (j + 1) * E],  # moving     [128h, 160e]
                    start=(j == 0),
                    stop=(j == J - 1),
                )

            ps3 = ps[:].rearrange("p (g i) -> p g i", i=EG)

            # group max of logits -> top-3-group additive penalty mask
            gmax = small.tile([P, G], F32)
            nc.vector.tensor_reduce(gmax[:], ps3, axis=AX.X, op=ALU.max)
            gsort = small.tile([P, 8], F32)
            nc.vector.max(gsort[:], gmax[:])
            gpen = small.tile([P, G], F32)  # 0 for kept groups, NEG_BIG for dropped
            nc.vector.tensor_scalar(
                gpen[:], gmax[:], gsort[:, TOPK_GROUP - 1 : TOPK_GROUP], NEG_BIG,
                op0=ALU.is_lt, op1=ALU.mult,
            )

            # masked logits = logits + penalty(group)
            masked = bigt.tile([P, E], F32)
            nc.vector.scalar_tensor_tensor(
                masked[:].rearrange("p (g i) -> p g i", i=EG),
                ps3,
                1.0,
                gpen[:, :, None].to_broadcast((P, G, EG)),
                op0=ALU.mult,
                op1=ALU.add,
            )

            # top-8 masked logits (descending) + expert indices
            v8 = small.tile([P, 8], F32)
            nc.vector.max(v8[:], masked[:])
            i8 = small.tile([P, 8], U32)
            nc.vector.max_index(i8[:], v8[:], masked[:])

            # softmax pieces: global max logit is v8[:,0] (the best group holds it)
            nrmax = small.tile([P, 1], F32)
            nc.vector.tensor_scalar_mul(nrmax[:], v8[:, 0:1], -1.0)
            exps = bigt.tile([P, E], F32)
            ssum = small.tile([P, 1], F32)
            nc.scalar.activation(
                exps[:], ps[:], ACTF.Exp, bias=nrmax[:], scale=1.0, accum_out=ssum[:]
            )
            rcp = small.tile([P, 1], F32)
            nc.vector.reciprocal(rcp[:], ssum[:])
            scl = small.tile([P, 1], F32)
            nc.vector.tensor_scalar_mul(scl[:], rcp[:], ROUTED_SCALING)

            # weights = exp(v6 - rmax) * 16 / ssum
            e6 = small.tile([P, TOP_K], F32)
            nc.scalar.activation(e6[:], v8[:, 0:TOP_K], ACTF.Exp, bias=nrmax[:], scale=1.0)
            w6 = small.tile([P, TOP_K], F32)
            nc.vector.tensor_scalar_mul(w6[:], e6[:], scl[:])

            nc.sync.dma_start(oi_ap[tt * P : (tt + 1) * P, :], i8[:, 0:TOP_K])
            nc.sync.dma_start(ow_ap[tt * P : (tt + 1) * P, :], w6[:])


E_PAD = 256  # experts padded so the f32r moving operand is >=256 wide

# Fast-DMA activation layout, shared by the f32r and hilo3f modes:
# xp[p, ((tile*J) + j)*P + t] = x[tile*P + t, p*J + j]. Each token-tile's
# DMA is one fully contiguous 20KB run per partition (line rate), and the
# per-k-tile stationary slice xt[:, j*P:(j+1)*P] is contiguous in SBUF
# (for bf16 this lets the compiler's Fast Weight Load engage; a strided
# stationary AP defeats it and the kernel goes LDWEIGHTS-bound).


def emit_gate_f32r(tc, x_ap, w_ap, oi_ap, ow_ap):
    """Single-pass float32r gate.

    float32r is fp32 data the PE streams at bf16 rate (1 cycle/row) when the
    moving free dim is >=256 — below that it falls to 1/4 rate. The weight is
    therefore zero-padded from 160 to 256 experts; the epilogue only ever
    reads logits[:, :160] so the pad never enters selection.

    MEASURED: 116.9us (= the ~117us HBM roofline for the 41.9MB/core x
    read), but the f32r datapath truncates operands to ~11 mantissa bits:
    rel err 1.99e-2 vs the 2e-2 gate (hundreds of flipped near-tie 6th
    picks). Too risky to ship; kept for reference.
    """
    nc = tc.nc
    T = x_ap.shape[1] // (P * J) * P
    n_tiles = T // P

    with (
        tc.tile_pool(name="wpool", bufs=1) as wpool,
        tc.tile_pool(name="xpool", bufs=3) as xpool,
        tc.tile_pool(name="psum", bufs=4, space="PSUM") as psum_pool,
        tc.tile_pool(name="small", bufs=6) as small,
        tc.tile_pool(name="bigt", bufs=3) as bigt,
    ):
        w_sb = wpool.tile([P, J * E_PAD], F32R)
        nc.sync.dma_start(w_sb[:], w_ap)

        for tt0 in range(0, n_tiles, 2):
            pair = [tt0, tt0 + 1] if tt0 + 1 < n_tiles else [tt0]
            xts, pss = [], []
            for tt in pair:
                xt = xpool.tile([P, P * J], F32R)
                nc.sync.dma_start(
                    xt[:], x_ap[:, tt * P * J : (tt + 1) * P * J]
                )
                xts.append(xt[:])
                pss.append(psum_pool.tile([P, E_PAD], F32, name="ps", tag=f"ps{len(pss)}"))

            for j in range(J):
                for k in range(len(pair)):
                    nc.tensor.matmul(
                        pss[k][:],
                        xts[k][:, j * P : (j + 1) * P],
                        w_sb[:, j * E_PAD : (j + 1) * E_PAD],
                        start=(j == 0),
                        stop=(j == J - 1),
                    )

            for k, tt in enumerate(pair):
                _emit_epilogue(tc, small, bigt, pss[k][:, 0:E], oi_ap, ow_ap, tt)


def emit_gate_hilo3f(tc, x_ap, whi_ap, wlo_ap, oi_ap, ow_ap):
    """3-term bf16 split gate on the fast-DMA [p, tile, j, t] layout.

    logits = hi@Whi + hi@Wlo + lo@Whi, fp32 PSUM accumulation, error
    ~2^-18 (the dropped lo@Wlo term). The contiguous per-j stationary
    slice keeps LDWEIGHTS on the Fast-Weight-Load path (~53ns < the 67ns
    N=160 stream), so the PE runs at the 3x160x40 streaming floor
    (~128us/core) instead of the LDW-bound ~205us the strided layout
    gives. DMA is at line rate (~117us/core), fully overlapped.
    """
    nc = tc.nc
    T = x_ap.shape[1] // J
    n_tiles = T // P

    with (
        tc.tile_pool(name="wpool", bufs=1) as wpool,
        tc.tile_pool(name="xpool", bufs=3) as xpool,
        tc.tile_pool(name="hpool", bufs=3) as hpool,
        tc.tile_pool(name="lpool", bufs=3) as lpool,
        tc.tile_pool(name="psum", bufs=4, space="PSUM") as psum_pool,
        tc.tile_pool(name="small", bufs=6) as small,
        tc.tile_pool(name="bigt", bufs=3) as bigt,
    ):
        whi_sb = wpool.tile([P, J * E], BF16)
        nc.sync.dma_start(whi_sb[:], whi_ap)
        wlo_sb = wpool.tile([P, J * E], BF16)
        nc.sync.dma_start(wlo_sb[:], wlo_ap)

        for tt0 in range(0, n_tiles, 2):
            pair = [tt0, tt0 + 1] if tt0 + 1 < n_tiles else [tt0]
            his, los, pss = [], [], []
            for tt in pair:
                xt = xpool.tile([P, P * J], F32)
                nc.sync.dma_start(
                    xt[:], x_ap[:, tt * P * J : (tt + 1) * P * J]
                )
                hi = hpool.tile([P, P * J], BF16)
                nc.scalar.copy(hi[:], xt[:])
                lo = lpool.tile([P, P * J], BF16)
                nc.vector.scalar_tensor_tensor(
                    lo[:], xt[:], 1.0, hi[:], op0=ALU.mult, op1=ALU.subtract
                )
                his.append(hi[:])
                los.append(lo[:])
                pss.append(
                    psum_pool.tile([P, E], F32, name="ps", tag=f"ps{len(pss)}")
                )

            for j in range(J):
                xsl = slice(j * P, (j + 1) * P)
                wsl = slice(j * E, (j + 1) * E)
                ops = [(his, whi_sb), (his, wlo_sb), (los, whi_sb)]
                for oi, (xs, wsb) in enumerate(ops):
                    last = j == J - 1 and oi == len(ops) - 1
                    for k in range(len(pair)):
                        nc.tensor.matmul(
                            pss[k][:], xs[k][:, xsl], wsb[:, wsl],
                            start=(j == 0 and oi == 0), stop=last,
                        )

            for k, tt in enumerate(pair):
                _emit_epilogue(tc, small, bigt, pss[k][:], oi_ap, ow_ap, tt)


def emit_gate_hilo3w(tc, x_ap, wc_ap, oi_ap, ow_ap):
    """Like hilo3f but with Whi|Wlo concatenated per j into one N=320
    moving operand: per k-tile, 2 matmuls (hi@[Whi|Wlo], lo@Whi) instead
    of 3, cutting LDWEIGHTS/instruction count by a third at identical
    streamed-row count. logits = ps_h[:,0:160] + ps_h[:,160:320] + ps_l,
    folded with two DVE adds. wc_ap: [P, J*2E] bf16,
    wc[p, j*2E + e] = Whi[e] for e<160 else Wlo[e-160]."""
    nc = tc.nc
    T = x_ap.shape[1] // J
    n_tiles = T // P
    E2 = 2 * E

    with (
        tc.tile_pool(name="wpool", bufs=1) as wpool,
        tc.tile_pool(name="xpool", bufs=3) as xpool,
        tc.tile_pool(name="hpool", bufs=3) as hpool,
        tc.tile_pool(name="lpool", bufs=3) as lpool,
        tc.tile_pool(name="psum", bufs=2, space="PSUM") as psum_pool,
        tc.tile_pool(name="small", bufs=6) as small,
        tc.tile_pool(name="bigt", bufs=4) as bigt,
    ):
        wc_sb = wpool.tile([P, J * E2], BF16)
        nc.sync.dma_start(wc_sb[:], wc_ap)

        for tt0 in range(0, n_tiles, 2):
            pair = [tt0, tt0 + 1] if tt0 + 1 < n_tiles else [tt0]
            his, los, psh, psl = [], [], [], []
            for tt in pair:
                xt = xpool.tile([P, P * J], F32)
                nc.sync.dma_start(
                    xt[:], x_ap[:, tt * P * J : (tt + 1) * P * J]
                )
                hi = hpool.tile([P, P * J], BF16)
                nc.scalar.copy(hi[:], xt[:])
                lo = lpool.tile([P, P * J], BF16)
                nc.vector.scalar_tensor_tensor(
                    lo[:], xt[:], 1.0, hi[:], op0=ALU.mult, op1=ALU.subtract
                )
                his.append(hi[:])
                los.append(lo[:])
                # full-bank tiles so the two accumulation groups can never
                # share a PSUM bank (a group's start clears its whole bank)
                psh.append(
                    psum_pool.tile([P, 512], F32, name="psh", tag=f"psh{len(psh)}")
                )
                psl.append(
                    psum_pool.tile([P, 512], F32, name="psl", tag=f"psl{len(psl)}")
                )

            for j in range(J):
                xsl = slice(j * P, (j + 1) * P)
                for k in range(len(pair)):
                    nc.tensor.matmul(
                        psh[k][:, 0:E2], his[k][:, xsl],
                        wc_sb[:, j * E2 : (j + 1) * E2],
                        start=(j == 0), stop=(j == J - 1),
                    )
                    nc.tensor.matmul(
                        psl[k][:, 0:E], los[k][:, xsl],
                        wc_sb[:, j * E2 : j * E2 + E],
                        start=(j == 0), stop=(j == J - 1),
                    )

            for k, tt in enumerate(pair):
                # DVE/ACT may read at most one PSUM input per instruction
                hb = bigt.tile([P, E], F32)
                nc.scalar.copy(hb[:], psh[k][:, E:E2])
                ha = bigt.tile([P, E], F32)
                nc.vector.tensor_add(ha[:], psh[k][:, 0:E], hb[:])
                lg = bigt.tile([P, E], F32)
                nc.vector.tensor_add(lg[:], ha[:], psl[k][:, 0:E])
                _emit_epilogue(tc, small, bigt, lg[:], oi_ap, ow_ap, tt)


def emit_gate_hilo3g(tc, x_ap, wc_sb, oi_ap, ow_ap, repeat=1):
    """hilo3w with a resident weight tile (loaded once per NEFF, shared
    across repeats) and one fused 5.24MB DMA per token-tile pair.

    The repeat loop runs INSIDE the open tile pools so buffer rotation
    flows seamlessly across repeat boundaries (repeat r+1's first DMA
    prefetches during repeat r's tail) — the repeat-slope then measures
    pure steady-state pipeline rate, which is also the real back-to-back
    invocation throughput.

    wc_sb: [P, J*2E] bf16 SBUF AP, already loaded.
    """
    nc = tc.nc
    T = x_ap.shape[1] // J
    n_tiles = T // P
    E2 = 2 * E

    with (
        tc.tile_pool(name="xpool", bufs=2) as xpool,
        tc.tile_pool(name="hpool", bufs=2) as hpool,
        tc.tile_pool(name="lpool", bufs=2) as lpool,
        tc.tile_pool(name="psum", bufs=3, space="PSUM") as psum_pool,
        tc.tile_pool(name="small", bufs=6) as small,
        tc.tile_pool(name="bigt", bufs=4) as bigt,
    ):
        for _rep in range(repeat):
            for tt0 in range(0, n_tiles, 2):
                npair = 2 if tt0 + 1 < n_tiles else 1
                xt = xpool.tile([P, npair * P * J], F32)
                nc.sync.dma_start(
                    xt[:], x_ap[:, tt0 * P * J : (tt0 + npair) * P * J]
                )
                hi = hpool.tile([P, npair * P * J], BF16)
                nc.scalar.copy(hi[:], xt[:])
                lo = lpool.tile([P, npair * P * J], BF16)
                nc.vector.scalar_tensor_tensor(
                    lo[:], xt[:], 1.0, hi[:], op0=ALU.mult, op1=ALU.subtract
                )
                psh = [
                    psum_pool.tile([P, 512], F32, name="psh", tag=f"psh{k}")
                    for k in range(npair)
                ]

                # hi@[Whi|Wlo] (N=320) and lo@Whi (N=160) interleave in ONE
                # accumulation group per bank: psh[0:160] accumulates
                # hi@Whi + lo@Whi, psh[160:320] accumulates hi@Wlo. start
                # clears the whole bank on the first matmul only.
                for j in range(J):
                    for k in range(npair):
                        xsl = slice((k * J + j) * P, (k * J + j + 1) * P)
                        nc.tensor.matmul(
                            psh[k][:, 0:E2], hi[:, xsl],
                            wc_sb[:, j * E2 : (j + 1) * E2],
                            start=(j == 0), stop=False,
                        )
                        nc.tensor.matmul(
                            psh[k][:, 0:E], lo[:, xsl],
                            wc_sb[:, j * E2 : j * E2 + E],
                            start=False, stop=(j == J - 1),
                        )

                for k in range(npair):
                    tt = tt0 + k
                    hb = bigt.tile([P, E], F32)
                    nc.scalar.copy(hb[:], psh[k][:, E:E2])
                    lg = bigt.tile([P, E], F32)
                    nc.vector.tensor_add(lg[:], psh[k][:, 0:E], hb[:])
                    _emit_epilogue(tc, small, bigt, lg[:], oi_ap, ow_ap, tt)


def emit_gate_hilo3h(tc, x_ap, wc_sb, oi_ap, ow_ap):
    """hilo3g with the bf16 hi/lo split done host-side: x_ap is
    [P, n_pairs * 4*P*J] bf16 laid out per token-tile pair as
    [hi(tile0) hi(tile1) lo(tile0) lo(tile1)], so each pair is one
    5.24MB contiguous DMA and the ACT cast / DVE subtract disappear
    from the device entirely (same total DMA bytes as f32 x).
    """
    nc = tc.nc
    TJ4 = 4 * P * J
    n_pairs = x_ap.shape[1] // TJ4
    E2 = 2 * E

    with (
        tc.tile_pool(name="xpool", bufs=3) as xpool,
        tc.tile_pool(name="psum", bufs=2, space="PSUM") as psum_pool,
        tc.tile_pool(name="small", bufs=6) as small,
        tc.tile_pool(name="bigt", bufs=4) as bigt,
    ):
        for q in range(n_pairs):
            xc = xpool.tile([P, TJ4], BF16)
            nc.sync.dma_start(xc[:], x_ap[:, q * TJ4 : (q + 1) * TJ4])
            psh = [
                psum_pool.tile([P, 512], F32, name="psh", tag=f"psh{k}")
                for k in range(2)
            ]
            psl = [
                psum_pool.tile([P, 512], F32, name="psl", tag=f"psl{k}")
                for k in range(2)
            ]

            for j in range(J):
                for k in range(2):
                    hsl = slice((k * J + j) * P, (k * J + j + 1) * P)
                    lsl = slice(
                        (2 * J + k * J + j) * P, (2 * J + k * J + j + 1) * P
                    )
                    nc.tensor.matmul(
                        psh[k][:, 0:E2], xc[:, hsl],
                        wc_sb[:, j * E2 : (j + 1) * E2],
                        start=(j == 0), stop=(j == J - 1),
                    )
                    nc.tensor.matmul(
                        psl[k][:, 0:E], xc[:, lsl],
                        wc_sb[:, j * E2 : j * E2 + E],
                        start=(j == 0), stop=(j == J - 1),
                    )

            for k in range(2):
                tt = 2 * q + k
                hb = bigt.tile([P, E], F32)
                nc.scalar.copy(hb[:], psh[k][:, E:E2])
                ha = bigt.tile([P, E], F32)
                nc.vector.tensor_add(ha[:], psh[k][:, 0:E], hb[:])
                lg = bigt.tile([P, E], F32)
                nc.vector.tensor_add(lg[:], ha[:], psl[k][:, 0:E])
                _emit_epilogue(tc, small, bigt, lg[:], oi_ap, ow_ap, tt)


def emit_gate_f16(tc, x_ap, wc_sb, oi_ap, ow_ap, repeat=1, chunk=4):
    """fp16 host-split gate: x arrives as fp16 (half the DMA bytes of f32),
    W as [Whi_f16 | Wlo_f16 * 2^11] fused into one N=320 moving operand
    (Wlo is pre-scaled into fp16 normal range host-side; the epilogue's
    ACT copy un-scales it). One matmul per (tile, k-tile): 320 moving
    cols vs the bf16 3-term's 480 — and no on-chip hi/lo split at all.

    logits = xh @ Whi + 2^-11 * (xh @ (Wlo*2^11)); error ~2^-11 from the
    fp16 rounding of x only (W split exact to ~2^-22).

    x_ap: [P, T*J] f16 in the fast-DMA layout
    xp[p, ((tile*J)+j)*P + t] = fp16(x[tile*P + t, p*J + j]).
    `chunk` token-tiles are fetched per DMA (chunk=4: 5.24MB transfers)
    and processed as `chunk` interleaved PSUM accumulation chains.
    """
    nc = tc.nc
    T = x_ap.shape[1] // J
    n_tiles = T // P
    E2 = 2 * E

    with (
        tc.tile_pool(name="xpool", bufs=2) as xpool,
        tc.tile_pool(name="psum", bufs=2, space="PSUM") as psum_pool,
        tc.tile_pool(name="small", bufs=6) as small,
        tc.tile_pool(name="bigt", bufs=4) as bigt,
    ):
        for _rep in range(repeat):
            for tt0 in range(0, n_tiles, chunk):
                nch = min(chunk, n_tiles - tt0)
                xt = xpool.tile([P, nch * P * J], F16)
                nc.sync.dma_start(
                    xt[:], x_ap[:, tt0 * P * J : (tt0 + nch) * P * J]
                )
                psh = [
                    psum_pool.tile([P, 512], F32, name="psh", tag=f"psh{k}")
                    for k in range(nch)
                ]
                for j in range(J):
                    for k in range(nch):
                        xsl = slice((k * J + j) * P, (k * J + j + 1) * P)
                        nc.tensor.matmul(
                            psh[k][:, 0:E2], xt[:, xsl],
                            wc_sb[:, j * E2 : (j + 1) * E2],
                            start=(j == 0), stop=(j == J - 1),
                        )
                for k in range(nch):
                    tt = tt0 + k
                    # un-scale the Wlo half (2^-11) while folding: ACT copy
                    # with scale, then one DVE add (<=1 PSUM operand each)
                    hb = bigt.tile([P, E], F32)
                    nc.scalar.activation(
                        hb[:], psh[k][:, E:E2], ACTF.Copy, scale=2.0 ** -11
                    )
                    lg = bigt.tile([P, E], F32)
                    nc.vector.tensor_add(lg[:], psh[k][:, 0:E], hb[:])
                    _emit_epilogue(tc, small, bigt, lg[:], oi_ap, ow_ap, tt)


LO_FOLD = 2.0 ** -18  # lo stored as e3m4(lo*2^12), W8 as e3m4(W*2^6)


def emit_gate_f16l(tc, x_ap, lo_ap, wc_sb, w8_sb, oi_ap, ow_ap, repeat=1, chunk=4):
    """f16h plus an fp8e3m4 lo-correction term: x = fp16(x) + lo, with
    lo shipped as e3m4(lo * 2^12) (1 byte) and W for the lo term as
    e3m4(W * 2^6). x error drops from 2^-11 (f16h, fails the gate) to
    ~2^-16. The lo matmuls share the hi accumulation group, landing in
    cols 320:480 of the same PSUM bank; the epilogue folds all three
    column ranges with the 2^-18 un-scale."""
    nc = tc.nc
    T = x_ap.shape[1] // J
    n_tiles = T // P
    E2 = 2 * E
    E3 = 3 * E

    with (
        tc.tile_pool(name="xpool", bufs=2) as xpool,
        tc.tile_pool(name="lpool", bufs=2) as lpool,
        tc.tile_pool(name="psum", bufs=2, space="PSUM") as psum_pool,
        tc.tile_pool(name="small", bufs=6) as small,
        tc.tile_pool(name="bigt", bufs=4) as bigt,
    ):
        for _rep in range(repeat):
            for tt0 in range(0, n_tiles, chunk):
                nch = min(chunk, n_tiles - tt0)
                xt = xpool.tile([P, nch * P * J], F16)
                nc.sync.dma_start(
                    xt[:], x_ap[:, tt0 * P * J : (tt0 + nch) * P * J]
                )
                lt = lpool.tile([P, nch * P * J], F8E3)
                nc.scalar.dma_start(
                    lt[:], lo_ap[:, tt0 * P * J : (tt0 + nch) * P * J]
                )
                psh = [
                    psum_pool.tile([P, 512], F32, name="psh", tag=f"psh{k}")
                    for k in range(nch)
                ]
                for j in range(J):
                    for k in range(nch):
                        xsl = slice((k * J + j) * P, (k * J + j + 1) * P)
                        nc.tensor.matmul(
                            psh[k][:, 0:E2], xt[:, xsl],
                            wc_sb[:, j * E2 : (j + 1) * E2],
                            start=(j == 0), stop=False,
                        )
                        nc.tensor.matmul(
                            psh[k][:, E2:E3], lt[:, xsl],
                            w8_sb[:, j * E : (j + 1) * E],
                            start=False, stop=(j == J - 1),
                        )
                for k in range(nch):
                    tt = tt0 + k
                    hb = bigt.tile([P, E], F32)
                    nc.scalar.activation(
                        hb[:], psh[k][:, E:E2], ACTF.Copy, scale=2.0 ** -11
                    )
                    t1 = bigt.tile([P, E], F32)
                    nc.vector.scalar_tensor_tensor(
                        t1[:], psh[k][:, E2:E3], LO_FOLD, hb[:],
                        op0=ALU.mult, op1=ALU.add,
                    )
                    lg = bigt.tile([P, E], F32)
                    nc.vector.tensor_add(lg[:], psh[k][:, 0:E], t1[:])
                    _emit_epilogue(tc, small, bigt, lg[:], oi_ap, ow_ap, tt)


def emit_gate_f16x(tc, x_ap, lo_ap, wh_sb, wl8_sb, w8_sb, oi_ap, ow_ap,
                   repeat=1, chunk=4, gp=False):
    """Scheme X: like f16l but the Wlo correction stream rides fp8e3m4 on
    the SAME xh stationary (mixed-dtype matmul), so the fp16 moving stream
    shrinks to N=160:
        psh[0:160]   += xh  @ Whi_f16            (fp16 moving)
        psh[160:320] += xh  @ e3m4(Wlo*2^17)     (fp8 moving, same lhsT)
        psh[320:480] += lo8 @ e3m4(W*2^6)        (fp8 moving)
    Wins if fp8 moving columns stream faster than fp16 ones."""
    nc = tc.nc
    T = x_ap.shape[1] // J
    n_tiles = T // P
    E2 = 2 * E
    E3 = 3 * E

    with (
        tc.tile_pool(name="xpool", bufs=2) as xpool,
        tc.tile_pool(name="lpool", bufs=2) as lpool,
        tc.tile_pool(name="psum", bufs=2, space="PSUM") as psum_pool,
        tc.tile_pool(name="small", bufs=6) as small,
        tc.tile_pool(name="bigt", bufs=4) as bigt,
    ):
        for _rep in range(repeat):
            for tt0 in range(0, n_tiles, chunk):
                nch = min(chunk, n_tiles - tt0)
                xt = xpool.tile([P, nch * P * J], F16)
                nc.sync.dma_start(
                    xt[:], x_ap[:, tt0 * P * J : (tt0 + nch) * P * J]
                )
                lt = lpool.tile([P, nch * P * J], F8E3)
                nc.scalar.dma_start(
                    lt[:], lo_ap[:, tt0 * P * J : (tt0 + nch) * P * J]
                )
                psh = [
                    psum_pool.tile([P, 512], F32, name="psh", tag=f"psh{k}")
                    for k in range(nch)
                ]
                for j in range(J):
                    for k in range(nch):
                        xsl = slice((k * J + j) * P, (k * J + j + 1) * P)
                        wsl = slice(j * E, (j + 1) * E)
                        nc.tensor.matmul(
                            psh[k][:, 0:E], xt[:, xsl], wh_sb[:, wsl],
                            start=(j == 0), stop=False,
                        )
                        nc.tensor.matmul(
                            psh[k][:, E:E2], xt[:, xsl], wl8_sb[:, wsl],
                            start=False, stop=False,
                        )
                        nc.tensor.matmul(
                            psh[k][:, E2:E3], lt[:, xsl], w8_sb[:, wsl],
                            start=False, stop=(j == J - 1),
                        )
                for k in range(nch):
                    tt = tt0 + k
                    hb = bigt.tile([P, E], F32)
                    nc.scalar.activation(
                        hb[:], psh[k][:, E:E2], ACTF.Copy, scale=2.0 ** -17
                    )
                    t1 = bigt.tile([P, E], F32)
                    nc.vector.scalar_tensor_tensor(
                        t1[:], psh[k][:, E2:E3], LO_FOLD, hb[:],
                        op0=ALU.mult, op1=ALU.add,
                    )
                    lg = bigt.tile([P, E], F32)
                    nc.vector.tensor_add(lg[:], psh[k][:, 0:E], t1[:])
                    _emit_epilogue(
                        tc, small, bigt, lg[:], oi_ap, ow_ap, tt, gp=gp
                    )


def emit_gate_f8w(tc, x_ap, lo_ap, wabc_sb, w8_sb, oi_ap, ow_ap,
                  repeat=1, chunk=2):
    """All-fp8 W-streams: per (tile, k-tile) just TWO matmuls —
        psh[0:480]  += xh  @ [A|B|C]   (one fp8e3m4 N=480 stream;
                                        A=e3m4(W*2^6), B=e3m4(r*2^11),
                                        C=e3m4(r'*2^16): 15 bits of W)
        psl[0:160]  += lo8 @ W8        (fp8 N=160)
    vs f16l's 800 moving bytes / 2 LDWs and f16x's 3 LDWs: 640 moving
    bytes, one fp16 LDW + one fp8 LDW. PSUM: psh needs a full bank, so
    psl lives in its own bank -> chunk=2 tiles to stay within 8 banks
    double-buffered."""
    nc = tc.nc
    T = x_ap.shape[1] // J
    n_tiles = T // P
    E3 = 3 * E

    with (
        tc.tile_pool(name="xpool", bufs=2) as xpool,
        tc.tile_pool(name="lpool", bufs=2) as lpool,
        tc.tile_pool(name="psum", bufs=2, space="PSUM") as psum_pool,
        tc.tile_pool(name="small", bufs=6) as small,
        tc.tile_pool(name="bigt", bufs=5) as bigt,
    ):
        for _rep in range(repeat):
            for tt0 in range(0, n_tiles, chunk):
                nch = min(chunk, n_tiles - tt0)
                xt = xpool.tile([P, nch * P * J], F16)
                nc.sync.dma_start(
                    xt[:], x_ap[:, tt0 * P * J : (tt0 + nch) * P * J]
                )
                lt = lpool.tile([P, nch * P * J], F8E3)
                nc.scalar.dma_start(
                    lt[:], lo_ap[:, tt0 * P * J : (tt0 + nch) * P * J]
                )
                psh = [
                    psum_pool.tile([P, 512], F32, name="psh", tag=f"psh{k}")
                    for k in range(nch)
                ]
                psl = [
                    psum_pool.tile([P, 512], F32, name="psl", tag=f"psl{k}")
                    for k in range(nch)
                ]
                for j in range(J):
                    for k in range(nch):
                        xsl = slice((k * J + j) * P, (k * J + j + 1) * P)
                        nc.tensor.matmul(
                            psh[k][:, 0:E3], xt[:, xsl],
                            wabc_sb[:, j * E3 : (j + 1) * E3],
                            start=(j == 0), stop=(j == J - 1),
                        )
                        nc.tensor.matmul(
                            psl[k][:, 0:E], lt[:, xsl],
                            w8_sb[:, j * E : (j + 1) * E],
                            start=(j == 0), stop=(j == J - 1),
                        )
                for k in range(nch):
                    tt = tt0 + k
                    s1 = bigt.tile([P, E], F32)
                    nc.scalar.activation(
                        s1[:], psh[k][:, 0:E], ACTF.Copy, scale=2.0 ** -6
                    )
                    s2 = bigt.tile([P, E], F32)
                    nc.vector.scalar_tensor_tensor(
                        s2[:], psh[k][:, E : 2 * E], 2.0 ** -11, s1[:],
                        op0=ALU.mult, op1=ALU.add,
                    )
                    s3 = bigt.tile([P, E], F32)
                    nc.vector.scalar_tensor_tensor(
                        s3[:], psh[k][:, 2 * E : E3], 2.0 ** -16, s2[:],
                        op0=ALU.mult, op1=ALU.add,
                    )
                    lg = bigt.tile([P, E], F32)
                    nc.vector.scalar_tensor_tensor(
                        lg[:], psl[k][:, 0:E], LO_FOLD, s3[:],
                        op0=ALU.mult, op1=ALU.add,
                    )
                    _emit_epilogue(tc, small, bigt, lg[:], oi_ap, ow_ap, tt)


def emit_probe_pe16(tc, x_ap, wc_sb, repeat=1):
    """Ablation probe: ONE fp16 quad loaded, then the full f16h matmul
    schedule re-reads the same tiles (no per-chunk DMA, no epilogue)."""
    nc = tc.nc
    T = x_ap.shape[1] // J
    n_tiles = T // P
    E2 = 2 * E
    with (
        tc.tile_pool(name="xpool", bufs=1) as xpool,
        tc.tile_pool(name="psum", bufs=2, space="PSUM") as psum_pool,
    ):
        xt = xpool.tile([P, 4 * P * J], F16)
        nc.sync.dma_start(xt[:], x_ap[:, 0 : 4 * P * J])
        for _rep in range(repeat):
            for _tt0 in range(0, n_tiles, 4):
                psh = [
                    psum_pool.tile([P, 512], F32, name="psh", tag=f"psh{k}")
                    for k in range(4)
                ]
                for j in range(J):
                    for k in range(4):
                        xsl = slice((k * J + j) * P, (k * J + j + 1) * P)
                        nc.tensor.matmul(
                            psh[k][:, 0:E2], xt[:, xsl],
                            wc_sb[:, j * E2 : (j + 1) * E2],
                            start=(j == 0), stop=(j == J - 1),
                        )


def emit_probe_pe_lo(tc, lo_ap, w8_sb, repeat=1):
    """Ablation probe: ONLY the fp8 N=160 lo matmuls of f16l."""
    nc = tc.nc
    T = lo_ap.shape[1] // J
    n_tiles = T // P
    E2 = 2 * E
    E3 = 3 * E
    with (
        tc.tile_pool(name="lxpool", bufs=1) as lxpool,
        tc.tile_pool(name="psum", bufs=2, space="PSUM") as psum_pool,
    ):
        lt = lxpool.tile([P, 4 * P * J], F8E3)
        nc.scalar.dma_start(lt[:], lo_ap[:, 0 : 4 * P * J])
        for _rep in range(repeat):
            for _tt0 in range(0, n_tiles, 4):
                psh = [
                    psum_pool.tile([P, 512], F32, name="psh", tag=f"psh{k}")
                    for k in range(4)
                ]
                for j in range(J):
                    for k in range(4):
                        xsl = slice((k * J + j) * P, (k * J + j + 1) * P)
                        nc.tensor.matmul(
                            psh[k][:, E2:E3], lt[:, xsl],
                            w8_sb[:, j * E : (j + 1) * E],
                            start=(j == 0), stop=(j == J - 1),
                        )


def emit_probe_dma(tc, x_ap, repeat=1, rings=1):
    """Ablation probe: ONLY the per-pair x DMAs of hilo3g (no consumers).
    The repeat-slope of this NEFF is the pure steady-state DMA rate.
    rings=2 splits each transfer across the sync and scalar HWDGE rings."""
    nc = tc.nc
    T = x_ap.shape[1] // J
    n_tiles = T // P
    with tc.tile_pool(name="xpool", bufs=2) as xpool:
        for _rep in range(repeat):
            for tt0 in range(0, n_tiles, 2):
                npair = 2 if tt0 + 1 < n_tiles else 1
                xt = xpool.tile([P, npair * P * J], F32)
                src = x_ap[:, tt0 * P * J : (tt0 + npair) * P * J]
                if rings == 2:
                    nc.sync.dma_start(xt[0:64, :], src[0:64, :])
                    nc.scalar.dma_start(xt[64:P, :], src[64:P, :])
                else:
                    nc.sync.dma_start(xt[:], src)


def emit_probe_pe_f16l(tc, x_ap, lo_ap, wc_sb, w8_sb, repeat=1):
    """Ablation probe: one quad loaded, then the full f16l matmul schedule
    (fp16 N=320 + fp8e3 N=160 per tile per k-tile) re-reads it."""
    nc = tc.nc
    T = x_ap.shape[1] // J
    n_tiles = T // P
    E2 = 2 * E
    E3 = 3 * E
    with (
        tc.tile_pool(name="xpool", bufs=1) as xpool,
        tc.tile_pool(name="psum", bufs=2, space="PSUM") as psum_pool,
    ):
        xt = xpool.tile([P, 4 * P * J], F16)
        nc.sync.dma_start(xt[:], x_ap[:, 0 : 4 * P * J])
        lt = xpool.tile([P, 4 * P * J], F8E3)
        nc.scalar.dma_start(lt[:], lo_ap[:, 0 : 4 * P * J])
        for _rep in range(repeat):
            for _tt0 in range(0, n_tiles, 4):
                psh = [
                    psum_pool.tile([P, 512], F32, name="psh", tag=f"psh{k}")
                    for k in range(4)
                ]
                for j in range(J):
                    for k in range(4):
                        xsl = slice((k * J + j) * P, (k * J + j + 1) * P)
                        nc.tensor.matmul(
                            psh[k][:, 0:E2], xt[:, xsl],
                            wc_sb[:, j * E2 : (j + 1) * E2],
                            start=(j == 0), stop=False,
                        )
                        nc.tensor.matmul(
                            psh[k][:, E2:E3], lt[:, xsl],
                            w8_sb[:, j * E : (j + 1) * E],
                            start=False, stop=(j == J - 1),
                        )


def emit_probe_pe(tc, x_ap, wc_sb, repeat=1):
    """Ablation probe: ONE pair loaded + split, then the full hilo3g
    matmul schedule re-reads the same hi/lo tiles (no per-pair DMA, no
    epilogue). The repeat-slope is the pure steady-state PE rate with
    identical instruction shapes to the real kernel."""
    nc = tc.nc
    T = x_ap.shape[1] // J
    n_tiles = T // P
    E2 = 2 * E
    with (
        tc.tile_pool(name="xpool", bufs=1) as xpool,
        tc.tile_pool(name="hpool", bufs=1) as hpool,
        tc.tile_pool(name="lpool", bufs=1) as lpool,
        tc.tile_pool(name="psum", bufs=3, space="PSUM") as psum_pool,
    ):
        xt = xpool.tile([P, 2 * P * J], F32)
        nc.sync.dma_start(xt[:], x_ap[:, 0 : 2 * P * J])
        hi = hpool.tile([P, 2 * P * J], BF16)
        nc.scalar.copy(hi[:], xt[:])
        lo = lpool.tile([P, 2 * P * J], BF16)
        nc.vector.scalar_tensor_tensor(
            lo[:], xt[:], 1.0, hi[:], op0=ALU.mult, op1=ALU.subtract
        )
        for _rep in range(repeat):
            for tt0 in range(0, n_tiles, 2):
                psh = [
                    psum_pool.tile([P, 512], F32, name="psh", tag=f"psh{k}")
                    for k in range(2)
                ]
                for j in range(J):
                    for k in range(2):
                        xsl = slice((k * J + j) * P, (k * J + j + 1) * P)
                        nc.tensor.matmul(
                            psh[k][:, 0:E2], hi[:, xsl],
                            wc_sb[:, j * E2 : (j + 1) * E2],
                            start=(j == 0), stop=False,
                        )
                        nc.tensor.matmul(
                            psh[k][:, 0:E], lo[:, xsl],
                            wc_sb[:, j * E2 : j * E2 + E],
                            start=False, stop=(j == J - 1),
                        )


def emit_gate_hilo(tc, x_ap, whi_ap, wlo_ap, oi_ap, ow_ap, terms=3):
    """Split-precision gate: x and W decomposed as bf16 hi + lo; logits =
    hi@Whi + hi@Wlo + lo@Whi (+ lo@Wlo with terms=4) accumulated in fp32
    PSUM (error ~2^-18). bf16 matmuls run ~4x faster than fp32 on the PE.
    W's split is precomputed on host; x's is done on-chip (ACT casts hi,
    DVE computes lo = x - hi)."""
    nc = tc.nc
    T = x_ap.shape[0]
    assert T % P == 0
    n_tiles = T // P

    with (
        tc.tile_pool(name="wpool", bufs=1) as wpool,
        tc.tile_pool(name="xpool", bufs=3) as xpool,
        tc.tile_pool(name="hpool", bufs=3) as hpool,
        tc.tile_pool(name="lpool", bufs=3) as lpool,
        tc.tile_pool(name="psum", bufs=4, space="PSUM") as psum_pool,
        tc.tile_pool(name="small", bufs=6) as small,
        tc.tile_pool(name="bigt", bufs=3) as bigt,
    ):
        whi_sb = wpool.tile([P, J * E], BF16)
        nc.sync.dma_start(whi_sb[:], whi_ap)
        wlo_sb = wpool.tile([P, J * E], BF16)
        nc.sync.dma_start(wlo_sb[:], wlo_ap)

        # process token-tiles in pairs: the two accumulation chains alternate
        # on the PE so each LDWEIGHTS can run in the background weight buffer
        # while the other chain's matmul streams
        for tt0 in range(0, n_tiles, 2):
            pair = [tt0, tt0 + 1] if tt0 + 1 < n_tiles else [tt0]
            his, los, pss = [], [], []
            for tt in pair:
                xt = xpool.tile([P, P * J], F32)
                src = x_ap[tt * P : (tt + 1) * P, :].rearrange(
                    "t (p j) -> p t j", p=P
                )
                dst = xt[:].rearrange("p (t j) -> p t j", j=J)
                # split the tile's 16K descriptors across both HWDGE rings
                # (two independent descriptor generators; measured ~15%
                # whole-kernel win over a single ring)
                half = P // 2
                nc.sync.dma_start(dst[:, :half, :], src[:, :half, :])
                nc.scalar.dma_start(dst[:, half:, :], src[:, half:, :])
                hi = hpool.tile([P, P * J], BF16)
                nc.scalar.copy(hi[:], xt[:])
                lo = lpool.tile([P, P * J], BF16)
                nc.vector.scalar_tensor_tensor(
                    lo[:], xt[:], 1.0, hi[:], op0=ALU.mult, op1=ALU.subtract
                )
                his.append(hi[:].rearrange("p (t j) -> p t j", j=J))
                los.append(lo[:].rearrange("p (t j) -> p t j", j=J))
                ps_k = psum_pool.tile([P, E], F32, name="ps", tag=f"ps{len(pss)}")
                pss.append(ps_k)

            for j in range(J):
                wsl = slice(j * E, (j + 1) * E)
                ops = [(his, whi_sb), (his, wlo_sb), (los, whi_sb)]
                if terms == 4:
                    ops.append((los, wlo_sb))
                for oi, (xs, wsb) in enumerate(ops):
                    last = j == J - 1 and oi == len(ops) - 1
                    for k in range(len(pair)):
                        nc.tensor.matmul(
                            pss[k][:], xs[k][:, :, j], wsb[:, wsl],
                            start=(j == 0 and oi == 0), stop=last,
                        )

            for k, tt in enumerate(pair):
                _emit_epilogue(tc, small, bigt, pss[k][:], oi_ap, ow_ap, tt)


def emit_gate_hilo_wide(tc, x_ap, wc_ap, oi_ap, ow_ap):
    """EXPERIMENTAL - DOES NOT COMPILE (walrus birverifier asserts on the
    N=320 matmul; root cause unidentified). Do not select mode "hilo4w".

    Like emit_gate_hilo(terms=4) but with Whi|Wlo concatenated into one
    N=320 moving operand, halving the matmul (and stationary-reload) count:
    two accumulation chains hi@[Whi|Wlo] and lo@[Whi|Wlo] into [128,320]
    PSUM tiles, folded into logits with three DVE adds."""
    nc = tc.nc
    T = x_ap.shape[0]
    assert T % P == 0
    n_tiles = T // P
    E2 = 2 * E

    with (
        tc.tile_pool(name="wpool", bufs=1) as wpool,
        tc.tile_pool(name="xpool", bufs=3) as xpool,
        tc.tile_pool(name="hpool", bufs=3) as hpool,
        tc.tile_pool(name="lpool", bufs=3) as lpool,
        tc.tile_pool(name="psum", bufs=3, space="PSUM") as psum_pool,
        tc.tile_pool(name="small", bufs=6) as small,
        tc.tile_pool(name="bigt", bufs=4) as bigt,
    ):
        wc_sb = wpool.tile([P, J * E2], BF16)
        nc.sync.dma_start(wc_sb[:], wc_ap)

        for tt in range(n_tiles):
            xt = xpool.tile([P, P * J], F32)
            src = x_ap[tt * P : (tt + 1) * P, :].rearrange("t (p j) -> p t j", p=P)
            nc.sync.dma_start(xt[:].rearrange("p (t j) -> p t j", j=J), src)
            hi = hpool.tile([P, P * J], BF16)
            nc.scalar.copy(hi[:], xt[:])
            lo = lpool.tile([P, P * J], BF16)
            nc.vector.scalar_tensor_tensor(
                lo[:], xt[:], 1.0, hi[:], op0=ALU.mult, op1=ALU.subtract
            )
            hi3 = hi[:].rearrange("p (t j) -> p t j", j=J)
            lo3 = lo[:].rearrange("p (t j) -> p t j", j=J)

            ps_h = psum_pool.tile([P, 512], F32, name="ps_h", tag="psh")[:, :E2]
            ps_l = psum_pool.tile([P, 512], F32, name="ps_l", tag="psl")[:, :E2]
            for src3, pst in ((hi3, ps_h), (lo3, ps_l)):
                for j in range(J):
                    wsl = slice(j * E2, (j + 1) * E2)
                    nc.tensor.matmul(
                        pst[:], src3[:, :, j], wc_sb[:, wsl],
                        start=(j == 0), stop=(j == J - 1),
                    )

            # logits = hi@Whi + hi@Wlo + lo@Whi + lo@Wlo
            ha = bigt.tile([P, E], F32)
            nc.vector.tensor_add(ha[:], ps_h[:, 0:E], ps_h[:, E:E2])
            la = bigt.tile([P, E], F32)
            nc.vector.tensor_add(la[:], ps_l[:, 0:E], ps_l[:, E:E2])
            lg = bigt.tile([P, E], F32)
            nc.vector.tensor_add(lg[:], ha[:], la[:])

            _emit_epilogue(tc, small, bigt, lg[:], oi_ap, ow_ap, tt)


def _emit_epilogue(tc, small, bigt, ps, oi_ap, ow_ap, tt, gp=False):
    """ps: [P, E] AP of raw logits (PSUM or SBUF). gp=True offloads the
    two big SBUF-input DVE ops (group-max reduce, mask-add) to the idle
    GpSimd engine — only legal when ps is in SBUF."""
    nc = tc.nc
    veng = nc.gpsimd if gp else nc.vector
    ps3 = ps.rearrange("p (g i) -> p g i", i=EG)
    gmax = small.tile([P, G], F32)
    nc.vector.tensor_reduce(gmax[:], ps3, axis=AX.X, op=ALU.max)
    gsort = small.tile([P, 8], F32)
    nc.vector.max(gsort[:], gmax[:])
    gpen = small.tile([P, G], F32)
    nc.vector.tensor_scalar(
        gpen[:], gmax[:], gsort[:, TOPK_GROUP - 1 : TOPK_GROUP], NEG_BIG,
        op0=ALU.is_lt, op1=ALU.mult,
    )
    masked = bigt.tile([P, E], F32)
    veng.scalar_tensor_tensor(
        masked[:].rearrange("p (g i) -> p g i", i=EG),
        ps3, 1.0,
        gpen[:, :, None].to_broadcast((P, G, EG)),
        op0=ALU.mult, op1=ALU.add,
    )
    v8 = small.tile([P, 8], F32)
    nc.vector.max(v8[:], masked[:])
    i8 = small.tile([P, 8], U32)
    nc.vector.max_index(i8[:], v8[:], masked[:])
    nrmax = small.tile([P, 1], F32)
    nc.vector.tensor_scalar_mul(nrmax[:], v8[:, 0:1], -1.0)
    exps = bigt.tile([P, E], F32)
    ssum = small.tile([P, 1], F32)
    nc.scalar.activation(
        exps[:], ps, ACTF.Exp, bias=nrmax[:], scale=1.0, accum_out=ssum[:]
    )
    rcp = small.tile([P, 1], F32)
    nc.vector.reciprocal(rcp[:], ssum[:])
    scl = small.tile([P, 1], F32)
    nc.vector.tensor_scalar_mul(scl[:], rcp[:], ROUTED_SCALING)
    e6 = small.tile([P, TOP_K], F32)
    nc.scalar.activation(e6[:], v8[:, 0:TOP_K], ACTF.Exp, bias=nrmax[:], scale=1.0)
    w6 = small.tile([P, TOP_K], F32)
    nc.vector.tensor_scalar_mul(w6[:], e6[:], scl[:])
    # outputs go out on the scalar HWDGE ring so the sync ring stays
    # dedicated to x prefetches
    nc.scalar.dma_start(oi_ap[tt * P : (tt + 1) * P, :], i8[:, 0:TOP_K])
    nc.scalar.dma_start(ow_ap[tt * P : (tt + 1) * P, :], w6[:])


def build_gate_kernel(T: int = T_CORE, repeat: int = 1, mode: str = "fp32"):
    nc = bacc.Bacc("TRN2", target_bir_lowering=False, debug=False, num_devices=N_CORES)
    oi_d = nc.dram_tensor("oi", [T, TOP_K], U32, kind="ExternalOutput")
    ow_d = nc.dram_tensor("ow", [T, TOP_K], F32, kind="ExternalOutput")
    if repeat == 0:
        # near-empty NEFF: same I/O signature, one tiny memset+store.
        # Used as a pure dispatch/RTT reference for timing.
        if mode in ("f32r",):
            nc.dram_tensor("x", [T, H], F32R, kind="ExternalInput")
            nc.dram_tensor("w", [P, J * E_PAD], F32R, kind="ExternalInput")
        elif mode == "hilo3h":
            nc.dram_tensor("x", [P, 2 * T * J], BF16, kind="ExternalInput")
            nc.dram_tensor("wc", [P, J * 2 * E], BF16, kind="ExternalInput")
        elif mode in ("hilo3w", "hilo3g"):
            nc.dram_tensor("x", [P, T * J], F32, kind="ExternalInput")
            nc.dram_tensor("wc", [P, J * 2 * E], BF16, kind="ExternalInput")
        else:
            nc.dram_tensor("x", [P, T * J], F32, kind="ExternalInput")
            nc.dram_tensor("whi", [P, J * E], BF16, kind="ExternalInput")
            nc.dram_tensor("wlo", [P, J * E], BF16, kind="ExternalInput")
        with TileContext(nc) as tc:
            with tc.tile_pool(name="zpool", bufs=1) as zp:
                z = zp.tile([P, TOP_K], U32)
                tc.nc.vector.memset(z[:], 0)
                tc.nc.sync.dma_start(oi_d.ap()[0:P, :], z[:])
                zw = zp.tile([P, TOP_K], F32)
                tc.nc.vector.memset(zw[:], 0)
                tc.nc.sync.dma_start(ow_d.ap()[0:P, :], zw[:])
        nc.compile()
        return nc
    if mode == "hilo4w":
        x_d = nc.dram_tensor("x", [T, H], F32, kind="ExternalInput")
        wc_d = nc.dram_tensor("wc", [P, J * 2 * E], BF16, kind="ExternalInput")
        with TileContext(nc) as tc:
            for _ in range(repeat):
                emit_gate_hilo_wide(tc, x_d.ap(), wc_d.ap(), oi_d.ap(), ow_d.ap())
    elif mode == "f32r":
        x_d = nc.dram_tensor("x", [P, T * J], F32R, kind="ExternalInput")
        w_d = nc.dram_tensor("w", [P, J * E_PAD], F32R, kind="ExternalInput")
        with TileContext(nc) as tc:
            for _ in range(repeat):
                emit_gate_f32r(tc, x_d.ap(), w_d.ap(), oi_d.ap(), ow_d.ap())
    elif mode == "hilo3f":
        x_d = nc.dram_tensor("x", [P, T * J], F32, kind="ExternalInput")
        whi_d = nc.dram_tensor("whi", [P, J * E], BF16, kind="ExternalInput")
        wlo_d = nc.dram_tensor("wlo", [P, J * E], BF16, kind="ExternalInput")
        with TileContext(nc) as tc:
            for _ in range(repeat):
                emit_gate_hilo3f(
                    tc, x_d.ap(), whi_d.ap(), wlo_d.ap(), oi_d.ap(), ow_d.ap()
                )
    elif mode == "hilo3w":
        x_d = nc.dram_tensor("x", [P, T * J], F32, kind="ExternalInput")
        wc_d = nc.dram_tensor("wc", [P, J * 2 * E], BF16, kind="ExternalInput")
        with TileContext(nc) as tc:
            for _ in range(repeat):
                emit_gate_hilo3w(
                    tc, x_d.ap(), wc_d.ap(), oi_d.ap(), ow_d.ap()
                )
    elif mode == "hilo3g":
        x_d = nc.dram_tensor("x", [P, T * J], F32, kind="ExternalInput")
        wc_d = nc.dram_tensor("wc", [P, J * 2 * E], BF16, kind="ExternalInput")
        with TileContext(nc) as tc:
            with tc.tile_pool(name="wpool", bufs=1) as wpool:
                wc_sb = wpool.tile([P, J * 2 * E], BF16)
                tc.nc.sync.dma_start(wc_sb[:], wc_d.ap())
                emit_gate_hilo3g(
                    tc, x_d.ap(), wc_sb, oi_d.ap(), ow_d.ap(), repeat=repeat
                )
    elif mode == "hilo3h":
        x_d = nc.dram_tensor("x", [P, 2 * T * J], BF16, kind="ExternalInput")
        wc_d = nc.dram_tensor("wc", [P, J * 2 * E], BF16, kind="ExternalInput")
        with TileContext(nc) as tc:
            with tc.tile_pool(name="wpool", bufs=1) as wpool:
                wc_sb = wpool.tile([P, J * 2 * E], BF16)
                tc.nc.sync.dma_start(wc_sb[:], wc_d.ap())
                for _ in range(repeat):
                    emit_gate_hilo3h(
                        tc, x_d.ap(), wc_sb, oi_d.ap(), ow_d.ap()
                    )
    elif mode == "f16h":
        x_d = nc.dram_tensor("x", [P, T * J], F16, kind="ExternalInput")
        wc_d = nc.dram_tensor("wc", [P, J * 2 * E], F16, kind="ExternalInput")
        with TileContext(nc) as tc:
            with tc.tile_pool(name="wpool", bufs=1) as wpool:
                wc_sb = wpool.tile([P, J * 2 * E], F16)
                tc.nc.sync.dma_start(wc_sb[:], wc_d.ap())
                emit_gate_f16(
                    tc, x_d.ap(), wc_sb, oi_d.ap(), ow_d.ap(), repeat=repeat
                )
    elif mode == "f16l":
        x_d = nc.dram_tensor("x", [P, T * J], F16, kind="ExternalInput")
        lo_d = nc.dram_tensor("xlo", [P, T * J], F8E3, kind="ExternalInput")
        wc_d = nc.dram_tensor("wc", [P, J * 2 * E], F16, kind="ExternalInput")
        w8_d = nc.dram_tensor("w8", [P, J * E], F8E3, kind="ExternalInput")
        with TileContext(nc) as tc:
            with tc.tile_pool(name="wpool", bufs=1) as wpool:
                wc_sb = wpool.tile([P, J * 2 * E], F16)
                tc.nc.sync.dma_start(wc_sb[:], wc_d.ap())
                w8_sb = wpool.tile([P, J * E], F8E3)
                tc.nc.sync.dma_start(w8_sb[:], w8_d.ap())
                emit_gate_f16l(
                    tc, x_d.ap(), lo_d.ap(), wc_sb, w8_sb,
                    oi_d.ap(), ow_d.ap(), repeat=repeat,
                )
    elif mode == "probe_pe16":
        x_d = nc.dram_tensor("x", [P, T * J], F16, kind="ExternalInput")
        wc_d = nc.dram_tensor("wc", [P, J * 2 * E], F16, kind="ExternalInput")
        with TileContext(nc) as tc:
            with tc.tile_pool(name="wpool", bufs=1) as wpool:
                wc_sb = wpool.tile([P, J * 2 * E], F16)
                tc.nc.sync.dma_start(wc_sb[:], wc_d.ap())
                z = wpool.tile([P, TOP_K], U32)
                tc.nc.vector.memset(z[:], 0)
                tc.nc.sync.dma_start(oi_d.ap()[0:P, :], z[:])
                zw = wpool.tile([P, TOP_K], F32)
                tc.nc.vector.memset(zw[:], 0)
                tc.nc.sync.dma_start(ow_d.ap()[0:P, :], zw[:])
                emit_probe_pe16(tc, x_d.ap(), wc_sb, repeat=repeat)
    elif mode == "f8w":
        x_d = nc.dram_tensor("x", [P, T * J], F16, kind="ExternalInput")
        lo_d = nc.dram_tensor("xlo", [P, T * J], F8E3, kind="ExternalInput")
        wabc_d = nc.dram_tensor("wabc", [P, J * 3 * E], F8E3, kind="ExternalInput")
        w8_d = nc.dram_tensor("w8", [P, J * E], F8E3, kind="ExternalInput")
        with TileContext(nc) as tc:
            with tc.tile_pool(name="wpool", bufs=1) as wpool:
                wabc_sb = wpool.tile([P, J * 3 * E], F8E3)
                tc.nc.sync.dma_start(wabc_sb[:], wabc_d.ap())
                w8_sb = wpool.tile([P, J * E], F8E3)
                tc.nc.sync.dma_start(w8_sb[:], w8_d.ap())
                emit_gate_f8w(
                    tc, x_d.ap(), lo_d.ap(), wabc_sb, w8_sb,
                    oi_d.ap(), ow_d.ap(), repeat=repeat,
                )
    elif mode in ("f16x", "f16xg"):
        x_d = nc.dram_tensor("x", [P, T * J], F16, kind="ExternalInput")
        lo_d = nc.dram_tensor("xlo", [P, T * J], F8E3, kind="ExternalInput")
        wh_d = nc.dram_tensor("wh", [P, J * E], F16, kind="ExternalInput")
        wl8_d = nc.dram_tensor("wl8", [P, J * E], F8E3, kind="ExternalInput")
        w8_d = nc.dram_tensor("w8", [P, J * E], F8E3, kind="ExternalInput")
        with TileContext(nc) as tc:
            with tc.tile_pool(name="wpool", bufs=1) as wpool:
                wh_sb = wpool.tile([P, J * E], F16)
                tc.nc.sync.dma_start(wh_sb[:], wh_d.ap())
                wl8_sb = wpool.tile([P, J * E], F8E3)
                tc.nc.sync.dma_start(wl8_sb[:], wl8_d.ap())
                w8_sb = wpool.tile([P, J * E], F8E3)
                tc.nc.sync.dma_start(w8_sb[:], w8_d.ap())
                emit_gate_f16x(
                    tc, x_d.ap(), lo_d.ap(), wh_sb, wl8_sb, w8_sb,
                    oi_d.ap(), ow_d.ap(), repeat=repeat,
                    gp=(mode == "f16xg"),
                )
    elif mode == "probe_pe_lo":
        lo_d = nc.dram_tensor("xlo", [P, T * J], F8E3, kind="ExternalInput")
        w8_d = nc.dram_tensor("w8", [P, J * E], F8E3, kind="ExternalInput")
        with TileContext(nc) as tc:
            with tc.tile_pool(name="wpool", bufs=1) as wpool:
                w8_sb = wpool.tile([P, J * E], F8E3)
                tc.nc.sync.dma_start(w8_sb[:], w8_d.ap())
                z = wpool.tile([P, TOP_K], U32)
                tc.nc.vector.memset(z[:], 0)
                tc.nc.sync.dma_start(oi_d.ap()[0:P, :], z[:])
                zw = wpool.tile([P, TOP_K], F32)
                tc.nc.vector.memset(zw[:], 0)
                tc.nc.sync.dma_start(ow_d.ap()[0:P, :], zw[:])
                emit_probe_pe_lo(tc, lo_d.ap(), w8_sb, repeat=repeat)
    elif mode == "probe_pe_f16l":
        x_d = nc.dram_tensor("x", [P, T * J], F16, kind="ExternalInput")
        lo_d = nc.dram_tensor("xlo", [P, T * J], F8E3, kind="ExternalInput")
        wc_d = nc.dram_tensor("wc", [P, J * 2 * E], F16, kind="ExternalInput")
        w8_d = nc.dram_tensor("w8", [P, J * E], F8E3, kind="ExternalInput")
        with TileContext(nc) as tc:
            with tc.tile_pool(name="wpool", bufs=1) as wpool:
                wc_sb = wpool.tile([P, J * 2 * E], F16)
                tc.nc.sync.dma_start(wc_sb[:], wc_d.ap())
                w8_sb = wpool.tile([P, J * E], F8E3)
                tc.nc.sync.dma_start(w8_sb[:], w8_d.ap())
                z = wpool.tile([P, TOP_K], U32)
                tc.nc.vector.memset(z[:], 0)
                tc.nc.sync.dma_start(oi_d.ap()[0:P, :], z[:])
                zw = wpool.tile([P, TOP_K], F32)
                tc.nc.vector.memset(zw[:], 0)
                tc.nc.sync.dma_start(ow_d.ap()[0:P, :], zw[:])
                emit_probe_pe_f16l(
                    tc, x_d.ap(), lo_d.ap(), wc_sb, w8_sb, repeat=repeat
                )
    elif mode in ("probe_dma", "probe_dma2", "probe_pe"):
        x_d = nc.dram_tensor("x", [P, T * J], F32, kind="ExternalInput")
        wc_d = nc.dram_tensor("wc", [P, J * 2 * E], BF16, kind="ExternalInput")
        with TileContext(nc) as tc:
            with tc.tile_pool(name="wpool", bufs=1) as wpool:
                wc_sb = wpool.tile([P, J * 2 * E], BF16)
                tc.nc.sync.dma_start(wc_sb[:], wc_d.ap())
                z = wpool.tile([P, TOP_K], U32)
                tc.nc.vector.memset(z[:], 0)
                tc.nc.sync.dma_start(oi_d.ap()[0:P, :], z[:])
                zw = wpool.tile([P, TOP_K], F32)
                tc.nc.vector.memset(zw[:], 0)
                tc.nc.sync.dma_start(ow_d.ap()[0:P, :], zw[:])
                if mode == "probe_dma":
                    emit_probe_dma(tc, x_d.ap(), repeat=repeat)
                elif mode == "probe_dma2":
                    emit_probe_dma(tc, x_d.ap(), repeat=repeat, rings=2)
                else:
                    emit_probe_pe(tc, x_d.ap(), wc_sb, repeat=repeat)
    elif mode in ("hilo", "hilo4"):
        x_d = nc.dram_tensor("x", [T, H], F32, kind="ExternalInput")
        whi_d = nc.dram_tensor("whi", [P, J * E], BF16, kind="ExternalInput")
        wlo_d = nc.dram_tensor("wlo", [P, J * E], BF16, kind="ExternalInput")
        with TileContext(nc) as tc:
            for _ in range(repeat):
                emit_gate_hilo(
                    tc, x_d.ap(), whi_d.ap(), wlo_d.ap(), oi_d.ap(), ow_d.ap(),
                    terms=4 if mode == "hilo4" else 3,
                )
    else:
        x_d = nc.dram_tensor("x", [T, H], F32, kind="ExternalInput")
        w_d = nc.dram_tensor("w", [P, J * E], F32, kind="ExternalInput")
        with TileContext(nc) as tc:
            for _ in range(repeat):
                emit_gate(tc, x_d.ap(), w_d.ap(), oi_d.ap(), ow_d.ap())
    nc.compile()
    return nc


def prep_weight(weight: np.ndarray) -> np.ndarray:
    """[160, 5120] -> [128, 40*160] with w[p, j*E + e] = W[e, p*40 + j]."""
    wt = np.asarray(weight, dtype=np.float32).T  # [H, E]
    return np.ascontiguousarray(wt.reshape(P, J, E)).reshape(P, J * E)


def prep_weight_f32r(weight: np.ndarray) -> np.ndarray:
    """[160, 5120] -> [128, 40*256], w[p, j*E_PAD + e] = W[e, p*40 + j]
    (zero for e >= 160)."""
    wt = np.asarray(weight, dtype=np.float32).T  # [H, E]
    wp = np.zeros((H, E_PAD), np.float32)
    wp[:, :E] = wt
    return np.ascontiguousarray(wp.reshape(P, J, E_PAD)).reshape(P, J * E_PAD)


def prep_weight_f16(weight: np.ndarray) -> np.ndarray:
    """[160, 5120] -> [P, J*2E] fp16: per j-block [Whi | Wlo * 2^11].

    Whi is fp16(W) with denormals flushed to zero host-side (so a PE that
    flushes fp16 denormals sees exactly the value Wlo was computed
    against); Wlo is scaled by 2^11 into fp16 normal range and un-scaled
    in the kernel epilogue. W split error ~2^-22."""
    w = np.asarray(weight, dtype=np.float32)
    whi = w.astype(np.float16)
    whi_f = np.where(np.abs(whi.astype(np.float32)) < 6.104e-5, 0.0, whi.astype(np.float32))
    whi = whi_f.astype(np.float16)
    wlo = ((w - whi.astype(np.float32)) * 2048.0).astype(np.float16)

    def perm(a):
        return np.ascontiguousarray(a.astype(np.float16).T.reshape(P, J, E))

    return np.ascontiguousarray(
        np.concatenate([perm(whi), perm(wlo)], axis=2)
    ).reshape(P, J * 2 * E)


def prep_weight_f8abc(weight: np.ndarray) -> np.ndarray:
    """[160, 5120] -> [P, J*3E] fp8e3m4: per j-block [A | B | C] with
    A = e3m4(W*2^6), B = e3m4((W - A/2^6)*2^11), C = e3m4(residual*2^16).
    Three 5-bit terms -> ~15 bits of W."""
    import ml_dtypes

    w = np.asarray(weight, dtype=np.float32)
    A = (w * 64.0).astype(ml_dtypes.float8_e3m4)
    rB = w - A.astype(np.float32) / 64.0
    B = (rB * 2.0 ** 11).astype(ml_dtypes.float8_e3m4)
    rC = rB - B.astype(np.float32) * 2.0 ** -11
    C = (rC * 2.0 ** 16).astype(ml_dtypes.float8_e3m4)

    def perm(a):
        return a.T.reshape(P, J, E)

    return np.ascontiguousarray(
        np.concatenate([perm(A), perm(B), perm(C)], axis=2)
    ).reshape(P, J * 3 * E)


def prep_weight_f8(weight: np.ndarray) -> np.ndarray:
    """[160, 5120] -> [P, J*E] fp8e3m4 of W * 2^6, for the lo term."""
    import ml_dtypes

    w = np.asarray(weight, dtype=np.float32) * 64.0
    w8 = w.astype(ml_dtypes.float8_e3m4)
    return np.ascontiguousarray(w8.T.reshape(P, J, E)).reshape(P, J * E)


def prep_weight_hilo(weight: np.ndarray):
    import ml_dtypes

    w = np.asarray(weight, dtype=np.float32)
    whi = w.astype(ml_dtypes.bfloat16)
    wlo = (w - whi.astype(np.float32)).astype(ml_dtypes.bfloat16)

    def perm(a):
        return np.ascontiguousarray(a.T.reshape(P, J, E)).reshape(P, J * E)

    return perm(whi), perm(wlo)


_NC_CACHE = {}


# "hilo3g" = 3-term bf16 split matmul on the fast-DMA [p, tile, j, t]
# layout (line-rate 20KB-contiguous x loads, contiguous per-j stationary
# slices), with Whi|Wlo fused into one N=320 moving operand (2 matmuls per
# k-tile), the weight tile resident across repeats, and one 5.24MB DMA per
# token-tile pair. Measured 94.1us vs hilo4's 278.6us baseline; 6/98304
# near-tie index swaps, rel err 4.8e-3 (gate is 2e-2). "hilo4" kept as the
# old fallback; "f32r" is faster on paper but its ~11-bit operand
# truncation puts rel err at 1.99e-2 — disqualified.
MODE = "f16x"


def make_in_maps(hidden_states, weight, mode=None):
    mode = mode or MODE
    hs = np.ascontiguousarray(
        np.asarray(hidden_states, dtype=np.float32).reshape(T_TOTAL, H)
    )
    shards = hs.reshape(N_CORES, T_CORE, H)
    if mode in ("f16h", "f16l", "f16x", "f16xg", "f8w", "probe_pe16"):
        # fast-DMA layout, fp16: xp[p, ((tile*J)+j)*P + t] = x[tile*P+t, p*J+j]
        n_tiles = T_CORE // P
        xs = hs.reshape(N_CORES, n_tiles, P, P, J)  # [c, tile, t, p, j]
        wc = prep_weight_f16(weight)
        maps = []
        for c in range(N_CORES):
            xc = np.ascontiguousarray(xs[c].transpose(2, 0, 3, 1)).reshape(
                P, T_CORE * J
            )
            xh = xc.astype(np.float16)
            # flush fp16 denormals host-side so a PE that FTZs sees the
            # exact value the lo residual was computed against
            xh = np.where(
                np.abs(xh.astype(np.float32)) < 6.104e-5, 0, xh
            ).astype(np.float16)
            if mode == "f8w":
                import ml_dtypes

                lo = (xc - xh.astype(np.float32)) * 4096.0  # 2^12
                m = {
                    "x": xh,
                    "xlo": lo.astype(ml_dtypes.float8_e3m4),
                    "wabc": prep_weight_f8abc(weight),
                    "w8": prep_weight_f8(weight),
                }
            elif mode in ("f16x", "f16xg"):
                import ml_dtypes

                lo = (xc - xh.astype(np.float32)) * 4096.0  # 2^12
                m = {"x": xh, "xlo": lo.astype(ml_dtypes.float8_e3m4)}
                w = np.asarray(weight, dtype=np.float32)
                whi = w.astype(np.float16)
                whi = np.where(
                    np.abs(whi.astype(np.float32)) < 6.104e-5, 0, whi
                ).astype(np.float16)
                wl8 = ((w - whi.astype(np.float32)) * 2.0 ** 17).astype(
                    ml_dtypes.float8_e3m4
                )

                def perm(a):
                    return np.ascontiguousarray(a.T.reshape(P, J, E)).reshape(
                        P, J * E
                    )

                m["wh"] = perm(whi)
                m["wl8"] = perm(wl8)
                m["w8"] = prep_weight_f8(weight)
            else:
                m = {"x": xh, "wc": wc}
                if mode == "f16l":
                    import ml_dtypes

                    lo = (xc - xh.astype(np.float32)) * 4096.0  # 2^12
                    m["xlo"] = lo.astype(ml_dtypes.float8_e3m4)
                    m["w8"] = prep_weight_f8(weight)
            maps.append(m)
        return maps
    if mode in ("probe_pe_f16l", "probe_pe_lo"):
        maps = make_in_maps(hidden_states, weight, "f16l")
        if mode == "probe_pe_lo":
            maps = [{"xlo": m["xlo"], "w8": m["w8"]} for m in maps]
        return maps
    if mode in ("f32r", "hilo3f", "hilo3w", "hilo3g", "hilo3h", "probe_dma", "probe_dma2", "probe_pe"):
        # x[tile*P + t, p*J + j] -> xp[p, ((tile*J)+j)*P + t]: every
        # token-tile DMA is one contiguous 20KB run per partition, and each
        # k-tile's stationary slice is contiguous in SBUF.
        n_tiles = T_CORE // P
        xs = shards.reshape(N_CORES, n_tiles, P, P, J)  # [c, tile, t, p, j]
        xps = [
            np.ascontiguousarray(xs[c].transpose(2, 0, 3, 1)).reshape(
                P, T_CORE * J
            )
            for c in range(N_CORES)
        ]
        if mode == "f32r":
            wf = prep_weight_f32r(weight)
            return [{"x": xps[c], "w": wf} for c in range(N_CORES)]
        whi, wlo = prep_weight_hilo(weight)
        if mode == "hilo3h":
            import ml_dtypes

            wc = np.ascontiguousarray(
                np.concatenate(
                    [whi.reshape(P, J, E), wlo.reshape(P, J, E)], axis=2
                ).reshape(P, J * 2 * E)
            )
            n_pairs = T_CORE // P // 2
            maps = []
            for c in range(N_CORES):
                hi = xps[c].astype(ml_dtypes.bfloat16)
                lo = (xps[c] - hi.astype(np.float32)).astype(ml_dtypes.bfloat16)
                h3 = hi.reshape(P, n_pairs, 2 * J * P)
                l3 = lo.reshape(P, n_pairs, 2 * J * P)
                xc = np.concatenate(
                    [h3[:, :, None, :], l3[:, :, None, :]], axis=2
                ).reshape(P, 2 * T_CORE * J)
                maps.append({"x": np.ascontiguousarray(xc), "wc": wc})
            return maps
        if mode in ("hilo3w", "hilo3g", "probe_dma", "probe_dma2", "probe_pe"):
            wc = np.concatenate(
                [whi.reshape(P, J, E), wlo.reshape(P, J, E)], axis=2
            ).reshape(P, J * 2 * E)
            return [
                {"x": xps[c], "wc": np.ascontiguousarray(wc)}
                for c in range(N_CORES)
            ]
        return [
            {"x": xps[c], "whi": whi, "wlo": wlo} for c in range(N_CORES)
        ]
    if mode == "hilo4w":
        whi, wlo = prep_weight_hilo(weight)
        wc = np.concatenate(
            [whi.reshape(P, J, E), wlo.reshape(P, J, E)], axis=2
        ).reshape(P, J * 2 * E)
        wc = np.ascontiguousarray(wc)
        return [{"x": shards[c], "wc": wc} for c in range(N_CORES)]
    if mode in ("hilo", "hilo4"):
        whi, wlo = prep_weight_hilo(weight)
        return [
            {"x": shards[c], "whi": whi, "wlo": wlo} for c in range(N_CORES)
        ]
    wr = prep_weight(weight)
    return [{"x": shards[c], "w": wr} for c in range(N_CORES)]


def run(hidden_states, weight, trace=False, mode=None):
    mode = mode or MODE
    in_maps = make_in_maps(hidden_states, weight, mode)
    if mode not in _NC_CACHE:
        _NC_CACHE[mode] = build_gate_kernel(mode=mode)
    nc = _NC_CACHE[mode]
    res = bass_utils.run_bass_kernel_spmd(
        nc, in_maps, core_ids=list(range(N_CORES)), trace=trace
    )
    idx = np.concatenate([r["oi"].astype(np.int32) for r in res.results], axis=0)
    wts = np.concatenate([r["ow"] for r in res.results], axis=0)
    return (idx, wts), res


def kernel(hidden_states, weight):
    (idx, wts), _ = run(hidden_states, weight)
    return idx, wts

